# revision 25
# baseline (speedup 1.0000x reference)
"""Trainium2 Bass kernel for nn_EncoderWav (wavelet CNN encoder).

Strategy: pure data parallelism — 8 images, one per NeuronCore. Sync-BN
batch statistics are combined with tiny AllReduce collectives (2 per
residual block). Convolutions run as fp32r matmuls on the tensor engine
with taps accumulated in PSUM; the first block folds (dx, cin) into the
contraction dim to use the 128-wide PE array despite cin=15.
"""

import sys
from contextlib import ExitStack

sys.path.insert(0, "/opt/trn_rl_repo")

import numpy as np  # noqa: E402

import concourse.bass as bass  # noqa: E402
import concourse.bacc as bacc  # noqa: E402
import concourse.tile as tile  # noqa: E402
import concourse.mybir as mybir  # noqa: E402
from concourse.bass_utils import run_bass_kernel_spmd  # noqa: E402

F32 = mybir.dt.float32
F32R = mybir.dt.float32r
BF16 = mybir.dt.bfloat16
OP = mybir.AluOpType
ACTF = mybir.ActivationFunctionType
BN_EPS = 1e-5
N_CORES = 8

# block specs (k>=2): (Cp pool chans, Cout, S spatial, rows-per-matmul)
BLOCKS = {
    2: dict(Cp=64, Cout=128, S=128, nr=4),
    3: dict(Cp=128, Cout=256, S=64, nr=8),
    4: dict(Cp=256, Cout=512, S=32, nr=16),
    5: dict(Cp=512, Cout=1024, S=16, nr=16),
}
T_ORDER = [4, 0, 1, 2, 3, 5, 6, 7, 8]  # tap (1,1) first so sc rhs has base partition 0
OUT_SHAPES = {
    1: (64, 256, 256), 2: (128, 128, 128), 3: (256, 64, 64),
    4: (512, 32, 32), 5: (1024, 16, 16),
}


def r32(ap):
    return ap.bitcast(F32R)


# ---------------------------------------------------------------------------
# host-side weight packing
# ---------------------------------------------------------------------------

def _prep_weights(params):
    """Transform conv weights into lhsT tensors for the kernel.

    Wavelet inputs are computed unnormalized on device (plain subband sums,
    no 0.5 factors); the 2^-k scale of level-k wavelets is folded into the
    conv weights that consume them. Conv biases are dropped entirely:
    train-mode BN directly follows every conv, and BN(y + b) == BN(y).
    """
    P = {}
    f32 = lambda a: np.ascontiguousarray(np.asarray(a, dtype=np.float32))

    # ---- block 1 ----
    b = params["blk1"]
    w1 = f32(b["w1"]).copy()              # [64, 15, 3, 3]
    ws = f32(b["ws"])[:, :, 0, 0].copy()  # [64, 15]
    w1[:, 3:, :, :] *= 0.5                # wav1 channels carry 2x scale
    ws[:, 3:] *= 0.5
    w1_1 = np.zeros((3, 45, 128), np.float32)
    for dy in range(3):
        for dx in range(3):
            w1_1[dy, dx * 15:dx * 15 + 15, 0:64] = w1[:, :, dy, dx].T
    w1_1[1, 15:30, 64:128] = ws.T          # shortcut rides tap (dy=1, dx=1)
    P["w1_1"] = w1_1
    w2 = f32(b["w2"])                      # [64, 64, 3, 3]
    w2_1 = np.zeros((9, 64, 64), np.float32)
    for dy in range(3):
        for dx in range(3):
            w2_1[dy * 3 + dx] = w2[:, :, dy, dx].T
    P["w2_1"] = w2_1
    gb = np.zeros((192, 2), np.float32)
    gb[0:64, 0], gb[0:64, 1] = f32(b["g1"]), f32(b["be1"])
    gb[64:128, 0], gb[64:128, 1] = f32(b["gs"]), f32(b["bes"])
    gb[128:192, 0], gb[128:192, 1] = f32(b["g2"]), f32(b["be2"])
    P["gb_1"] = gb

    # ---- block 2 ----
    b = params["blk2"]
    w1 = f32(b["w1"]).copy()               # [128, 76, 3, 3]
    ws = f32(b["ws"])[:, :, 0, 0].copy()
    w1[:, 64:, :, :] *= 0.25               # wav2 carries 4x
    ws[:, 64:] *= 0.25
    w1_2 = np.zeros((10, 76, 128), np.float32)
    for dy in range(3):
        for dx in range(3):
            w1_2[dy * 3 + dx] = w1[:, :, dy, dx].T
    w1_2[9] = ws.T
    P["w1_2"] = w1_2
    w2 = f32(b["w2"])
    w2_2 = np.zeros((9, 128, 128), np.float32)
    for dy in range(3):
        for dx in range(3):
            w2_2[dy * 3 + dx] = w2[:, :, dy, dx].T
    P["w2_2"] = w2_2
    gb = np.zeros((384, 2), np.float32)
    gb[0:128, 0], gb[0:128, 1] = f32(b["g1"]), f32(b["be1"])
    gb[128:256, 0], gb[128:256, 1] = f32(b["gs"]), f32(b["bes"])
    gb[256:384, 0], gb[256:384, 1] = f32(b["g2"]), f32(b["be2"])
    P["gb_2"] = gb

    # ---- blocks 3..5 ----
    for k in (3, 4, 5):
        spec = BLOCKS[k]
        Cp, Cout = spec["Cp"], spec["Cout"]
        Mt, Ktp = Cout // 128, Cp // 128
        b = params[f"blk{k}"]
        w1 = f32(b["w1"]).copy()
        ws = f32(b["ws"])[:, :, 0, 0].copy()
        wavscale = 0.5 ** k
        w1[:, Cp:, :, :] *= wavscale
        ws[:, Cp:] *= wavscale
        w1p = np.zeros((Mt, Ktp, 9, 128, 128), np.float32)
        w1w = np.zeros((Mt, 108, 128), np.float32)
        wsp = np.zeros((Mt, Ktp, 128, 128), np.float32)
        wsw = np.zeros((Mt, 12, 128), np.float32)
        for m in range(Mt):
            wm = w1[m * 128:(m + 1) * 128]
            for kt in range(Ktp):
                for dy in range(3):
                    for dx in range(3):
                        w1p[m, kt, dy * 3 + dx] = wm[:, kt * 128:(kt + 1) * 128, dy, dx].T
                wsp[m, kt] = ws[m * 128:(m + 1) * 128, kt * 128:(kt + 1) * 128].T
            for ti, t in enumerate(T_ORDER):
                dy, dx = t // 3, t % 3
                w1w[m, ti * 12:(ti + 1) * 12, :] = wm[:, Cp:, dy, dx].T
            wsw[m] = ws[m * 128:(m + 1) * 128, Cp:].T
        P[f"w1_{k}"] = w1p
        P[f"w1w_{k}"] = w1w
        P[f"ws_{k}"] = wsp
        P[f"wsw_{k}"] = wsw
        w2 = f32(b["w2"])
        w2p = np.zeros((Mt, Mt, 9, 128, 128), np.float32)
        for m in range(Mt):
            for kt in range(Mt):
                for dy in range(3):
                    for dx in range(3):
                        w2p[m, kt, dy * 3 + dx] = \
                            w2[m * 128:(m + 1) * 128, kt * 128:(kt + 1) * 128, dy, dx].T
        P[f"w2_{k}"] = w2p
        gb = np.zeros((3 * Cout, 2), np.float32)
        gb[0:Cout, 0], gb[0:Cout, 1] = f32(b["g1"]), f32(b["be1"])
        gb[Cout:2 * Cout, 0], gb[Cout:2 * Cout, 1] = f32(b["gs"]), f32(b["bes"])
        gb[2 * Cout:, 0], gb[2 * Cout:, 1] = f32(b["g2"]), f32(b["be2"])
        P[f"gb_{k}"] = gb
    return P


# ---------------------------------------------------------------------------
# device-side helpers
# ---------------------------------------------------------------------------

class Seg:
    """Accumulates bn_stats chunks for one [P, npix] conv-output segment."""

    def __init__(self, pool, P, nchunks, tag):
        self.P = P
        self.buf = pool.tile([P, max(nchunks, 1), 6], F32, tag=tag)
        self.n = 0

    def add(self, nc, src_flat):
        nc.vector.bn_stats(out=self.buf[:, self.n, :], in_=src_flat)
        self.n += 1

    def finish(self, nc, pool, tag):
        mv = pool.tile([self.P, 2], F32, tag=tag + "_mv")
        nc.vector.bn_aggr(out=mv, in_=self.buf[:, :self.n, :])
        return mv


def _pack_stats(nc, pool, mv, P, tag):
    """[P,2] (mean,var) -> [P,2] (mean/8, (var+mean^2)/8)."""
    pk = pool.tile([P, 2], F32, tag=tag + "_pk")
    tmp = pool.tile([P, 1], F32, tag=tag + "_tmp")
    nc.vector.tensor_tensor(out=tmp, in0=mv[:, 0:1], in1=mv[:, 0:1], op=OP.mult)
    nc.vector.tensor_tensor(out=pk[:, 1:2], in0=mv[:, 1:2], in1=tmp, op=OP.add)
    nc.scalar.mul(pk[:, 1:2], pk[:, 1:2], 1.0 / N_CORES)
    nc.scalar.mul(pk[:, 0:1], mv[:, 0:1], 1.0 / N_CORES)
    return pk


def _unpack_stats(nc, pool, g, gamma_beta, P, tag):
    """g [P,2] = (mean, E[x^2]) -> ac [P,2] = (a, c):
    a = gamma * rsqrt(var + eps), c = beta - mean * a."""
    ac = pool.tile([P, 2], F32, tag=tag + "_ac")
    tmp = pool.tile([P, 1], F32, tag=tag + "_t1")
    var = pool.tile([P, 1], F32, tag=tag + "_t2")
    nc.vector.tensor_tensor(out=tmp, in0=g[:, 0:1], in1=g[:, 0:1], op=OP.mult)
    nc.vector.tensor_tensor(out=var, in0=g[:, 1:2], in1=tmp, op=OP.subtract)
    eps = pool.tile([P, 1], F32, tag=tag + "_eps")
    nc.vector.memset(eps, BN_EPS)
    nc.scalar.activation(out=var, in_=var, func=ACTF.Sqrt, bias=eps, scale=1.0)
    nc.vector.reciprocal(out=var, in_=var)
    nc.vector.tensor_tensor(out=ac[:, 0:1], in0=var, in1=gamma_beta[:, 0:1], op=OP.mult)
    nc.vector.tensor_tensor(out=tmp, in0=g[:, 0:1], in1=ac[:, 0:1], op=OP.mult)
    nc.vector.tensor_tensor(out=ac[:, 1:2], in0=gamma_beta[:, 1:2], in1=tmp, op=OP.subtract)
    return ac


def _allreduce(nc, pool, segs_pk, cc_pair, replica, tag):
    """Pack per-segment [P,2] tiles into cci, AllReduce, read back tiles of
    (global mean, global E[x^2])."""
    cci, cco = cc_pair
    off = 0
    for pk in segs_pk:
        Pp = pk.shape[0]
        nc.sync.dma_start(out=cci[0, off:off + 2 * Pp], in_=pk)
        off += 2 * Pp
    nc.gpsimd.collective_compute(
        "AllReduce", OP.add, ins=[cci[:, :]], outs=[cco[:, :]],
        replica_groups=[replica])
    outs = []
    off = 0
    for i, pk in enumerate(segs_pk):
        Pp = pk.shape[0]
        g = pool.tile([Pp, 2], F32, tag=f"{tag}_g{i}")
        nc.sync.dma_start(out=g, in_=cco[0, off:off + 2 * Pp])
        off += 2 * Pp
        outs.append(g)
    return outs


def _load_gb(nc, pool, dram, row0, P, tag):
    t = pool.tile([P, 2], F32, tag=tag)
    nc.sync.dma_start(out=t, in_=dram[row0:row0 + P, :])
    return t


def _pad_memset(nc, t):
    S2 = t.shape[1]
    tb = t.bitcast(F32) if t.dtype == F32R else t
    nc.vector.memset(tb[:, 0, :], 0.0)
    nc.vector.memset(tb[:, S2 - 1, :], 0.0)
    nc.vector.memset(tb[:, :, 0], 0.0)
    nc.vector.memset(tb[:, :, S2 - 1], 0.0)


# ---------------------------------------------------------------------------
# kernel body
# ---------------------------------------------------------------------------

def _build_nc(num_devices=N_CORES, replica=None):
    if replica is None:
        replica = list(range(num_devices))
    nc = bacc.Bacc("TRN2", target_bir_lowering=False, debug=False,
                   num_devices=num_devices)

    x = nc.dram_tensor("x", [3, 512, 512], F32, kind="ExternalInput")
    wt = {}
    wt["w1_1"] = nc.dram_tensor("w1_1", [3, 45, 128], F32, kind="ExternalInput")
    wt["w2_1"] = nc.dram_tensor("w2_1", [9, 64, 64], F32, kind="ExternalInput")
    wt["gb_1"] = nc.dram_tensor("gb_1", [192, 2], F32, kind="ExternalInput")
    wt["w1_2"] = nc.dram_tensor("w1_2", [10, 76, 128], F32, kind="ExternalInput")
    wt["w2_2"] = nc.dram_tensor("w2_2", [9, 128, 128], F32, kind="ExternalInput")
    wt["gb_2"] = nc.dram_tensor("gb_2", [384, 2], F32, kind="ExternalInput")
    for k in (3, 4, 5):
        Cp, Cout = BLOCKS[k]["Cp"], BLOCKS[k]["Cout"]
        Mt, Ktp = Cout // 128, Cp // 128
        wt[f"w1_{k}"] = nc.dram_tensor(f"w1_{k}", [Mt, Ktp, 9, 128, 128], F32, kind="ExternalInput")
        wt[f"w1w_{k}"] = nc.dram_tensor(f"w1w_{k}", [Mt, 108, 128], F32, kind="ExternalInput")
        wt[f"ws_{k}"] = nc.dram_tensor(f"ws_{k}", [Mt, Ktp, 128, 128], F32, kind="ExternalInput")
        wt[f"wsw_{k}"] = nc.dram_tensor(f"wsw_{k}", [Mt, 12, 128], F32, kind="ExternalInput")
        wt[f"w2_{k}"] = nc.dram_tensor(f"w2_{k}", [Mt, Mt, 9, 128, 128], F32, kind="ExternalInput")
        wt[f"gb_{k}"] = nc.dram_tensor(f"gb_{k}", [3 * Cout, 2], F32, kind="ExternalInput")

    eout = {k: nc.dram_tensor(f"e{k}", list(OUT_SHAPES[k]), F32, kind="ExternalOutput")
            for k in range(1, 6)}

    in1d = nc.dram_tensor("in1d", [15, 256, 256], F32)
    ll_d = {1: nc.dram_tensor("ll1", [3, 256, 256], F32),
            2: nc.dram_tensor("ll2", [3, 128, 128], F32),
            3: nc.dram_tensor("ll3", [3, 64, 64], F32),
            4: nc.dram_tensor("ll4", [3, 32, 32], F32)}
    wav_d = {2: nc.dram_tensor("wav2", [12, 128, 128], F32),
             3: nc.dram_tensor("wav3", [12, 64, 64], F32),
             4: nc.dram_tensor("wav4", [12, 32, 32], F32),
             5: nc.dram_tensor("wav5", [12, 16, 16], F32)}
    yscD1 = nc.dram_tensor("yscD1", [64, 256, 256], BF16)
    y2D1 = nc.dram_tensor("y2D1", [64, 256, 256], BF16)
    yscD2 = nc.dram_tensor("yscD2", [128, 128, 128], BF16)
    cc = {}
    for tag, n in (("ar1_1", 256), ("ar2_1", 128), ("ar1_2", 512), ("ar2_2", 256),
                   ("ar1_3", 1024), ("ar2_3", 512), ("ar1_4", 2048), ("ar2_4", 1024),
                   ("ar1_5", 4096), ("ar2_5", 2048)):
        cc[tag] = (nc.dram_tensor(f"cci_{tag}", [1, n], F32),
                   nc.dram_tensor(f"cco_{tag}", [1, n], F32, addr_space="Shared"))

    with tile.TileContext(nc, pool_alloc_mode="queue") as tc:
        with ExitStack() as ctx:
            _emit(ctx, tc, nc, x, wt, eout, in1d, ll_d, wav_d,
                  yscD1, y2D1, yscD2, cc, replica)
    nc.compile()
    return nc


def _phase_wavelets(tc, nc, x, in1d, ll_d, wav_d):
    with tc.tile_pool(name="wv", bufs=3) as wv:
        # level 1: x [3,512,512] -> in1d channels (pool + wav1) + ll1
        for g in range(6):
            c, half = g // 2, g % 2
            y0 = 128 * half
            E = wv.tile([128, 512], F32, tag="E")
            Ot = wv.tile([128, 512], F32, tag="O")
            nc.sync.dma_start(out=E, in_=x[c, 2 * y0:2 * y0 + 256:2, :])
            nc.sync.dma_start(out=Ot, in_=x[c, 2 * y0 + 1:2 * y0 + 256:2, :])
            Ev = E.rearrange("p (a b) -> p a b", b=2)
            Ov = Ot.rearrange("p (a b) -> p a b", b=2)
            p_ = wv.tile([128, 256], F32, tag="p_")
            m_ = wv.tile([128, 256], F32, tag="m_")
            q_ = wv.tile([128, 256], F32, tag="q_")
            n_ = wv.tile([128, 256], F32, tag="n_")
            nc.vector.tensor_tensor(out=p_, in0=Ev[:, :, 0], in1=Ev[:, :, 1], op=OP.add)
            nc.vector.tensor_tensor(out=m_, in0=Ev[:, :, 0], in1=Ev[:, :, 1], op=OP.subtract)
            nc.vector.tensor_tensor(out=q_, in0=Ov[:, :, 0], in1=Ov[:, :, 1], op=OP.add)
            nc.vector.tensor_tensor(out=n_, in0=Ov[:, :, 0], in1=Ov[:, :, 1], op=OP.subtract)
            sb = wv.tile([128, 4, 256], F32, tag="sb")
            nc.vector.tensor_tensor(out=sb[:, 0, :], in0=p_, in1=q_, op=OP.add)
            nc.vector.tensor_tensor(out=sb[:, 1, :], in0=p_, in1=q_, op=OP.subtract)
            nc.vector.tensor_tensor(out=sb[:, 2, :], in0=m_, in1=n_, op=OP.add)
            nc.vector.tensor_tensor(out=sb[:, 3, :], in0=m_, in1=n_, op=OP.subtract)
            pl = wv.tile([128, 256], F32, tag="pl")
            nc.vector.tensor_tensor(out=p_, in0=Ev[:, :, 0], in1=Ev[:, :, 1], op=OP.max)
            nc.vector.tensor_tensor(out=q_, in0=Ov[:, :, 0], in1=Ov[:, :, 1], op=OP.max)
            nc.vector.tensor_tensor(out=pl, in0=p_, in1=q_, op=OP.max)
            nc.sync.dma_start(out=in1d[c, y0:y0 + 128, :], in_=pl)
            for s in range(4):
                nc.sync.dma_start(out=in1d[3 + 4 * c + s, y0:y0 + 128, :],
                                  in_=sb[:, s, :])
            nc.sync.dma_start(out=ll_d[1][c, y0:y0 + 128, :], in_=sb[:, 0, :])

        # levels 2..5 -> wav_d planes (+ ll chain)
        for lvl in (2, 3, 4, 5):
            src = ll_d[lvl - 1]
            Sin = src.shape[1]
            Sout = Sin // 2
            for c in range(3):
                ng = (Sout + 127) // 128
                for gi in range(ng):
                    y0 = gi * 128
                    nrr = min(128, Sout - y0)
                    E = wv.tile([128, 512], F32, tag="E")
                    Ot = wv.tile([128, 512], F32, tag="O")
                    nc.sync.dma_start(out=E[:nrr, :Sin], in_=src[c, 2 * y0:2 * (y0 + nrr):2, :])
                    nc.sync.dma_start(out=Ot[:nrr, :Sin], in_=src[c, 2 * y0 + 1:2 * (y0 + nrr):2, :])
                    Ev = E[:nrr, :Sin].rearrange("p (a b) -> p a b", b=2)
                    Ov = Ot[:nrr, :Sin].rearrange("p (a b) -> p a b", b=2)
                    p_ = wv.tile([128, 256], F32, tag="p_")
                    m_ = wv.tile([128, 256], F32, tag="m_")
                    q_ = wv.tile([128, 256], F32, tag="q_")
                    n_ = wv.tile([128, 256], F32, tag="n_")
                    nc.vector.tensor_tensor(out=p_[:nrr, :Sout], in0=Ev[:, :, 0], in1=Ev[:, :, 1], op=OP.add)
                    nc.vector.tensor_tensor(out=m_[:nrr, :Sout], in0=Ev[:, :, 0], in1=Ev[:, :, 1], op=OP.subtract)
                    nc.vector.tensor_tensor(out=q_[:nrr, :Sout], in0=Ov[:, :, 0], in1=Ov[:, :, 1], op=OP.add)
                    nc.vector.tensor_tensor(out=n_[:nrr, :Sout], in0=Ov[:, :, 0], in1=Ov[:, :, 1], op=OP.subtract)
                    sb = wv.tile([128, 4, 256], F32, tag="sb")
                    nc.vector.tensor_tensor(out=sb[:nrr, 0, :Sout], in0=p_[:nrr, :Sout], in1=q_[:nrr, :Sout], op=OP.add)
                    nc.vector.tensor_tensor(out=sb[:nrr, 1, :Sout], in0=p_[:nrr, :Sout], in1=q_[:nrr, :Sout], op=OP.subtract)
                    nc.vector.tensor_tensor(out=sb[:nrr, 2, :Sout], in0=m_[:nrr, :Sout], in1=n_[:nrr, :Sout], op=OP.add)
                    nc.vector.tensor_tensor(out=sb[:nrr, 3, :Sout], in0=m_[:nrr, :Sout], in1=n_[:nrr, :Sout], op=OP.subtract)
                    for s in range(4):
                        nc.sync.dma_start(out=wav_d[lvl][4 * c + s, y0:y0 + nrr, :],
                                          in_=sb[:nrr, s, :Sout])
                    if lvl < 5:
                        nc.sync.dma_start(out=ll_d[lvl][c, y0:y0 + nrr, :],
                                          in_=sb[:nrr, 0, :Sout])


def _emit(ctx, tc, nc, x, wt, eout, in1d, ll_d, wav_d, yscD1, y2D1, yscD2,
          cc, replica):
    _NEXT_IN.clear()
    small = ctx.enter_context(tc.tile_pool(name="small", bufs=1))

    _phase_wavelets(tc, nc, x, in1d, ll_d, wav_d)

    # =====================================================================
    # Block 1 (strip-tiled; y1 SBUF bf16; ysc & y2 staged in DRAM bf16)
    # =====================================================================

    with tc.tile_pool(name="b1w", bufs=1) as b1w:
        w1l = b1w.tile([45, 3, 128], F32R, tag="w1l")
        nc.sync.dma_start(out=w1l, in_=wt["w1_1"][:, :, :].rearrange("a b c -> b a c").bitcast(F32R))
        w2l = b1w.tile([64, 9, 64], F32R, tag="w2l")
        nc.sync.dma_start(out=w2l, in_=wt["w2_1"][:, :, :].rearrange("a b c -> b a c").bitcast(F32R))

        with tc.tile_pool(name="pY1", bufs=1) as pY1:
            Y1 = pY1.tile([128, 34816], BF16, tag="Y1")  # part 64h+c; free (r%128)*256+x

            # ---------------- pass A: conv1 + shortcut ----------------
            with tc.tile_pool(name="b1a", bufs=2) as b1a, \
                 tc.tile_pool(name="b1ps", bufs=4, space="PSUM") as b1ps, \
                 tc.tile_pool(name="b1st", bufs=2) as b1st:
                seg1 = Seg(small, 128, 128, "seg1")
                for s in range(8):
                    # T45[(dx,c), yy, x] = in1[c, 32s-1+yy, x-1+dx] with zero pad
                    T45 = b1a.tile([45, 34, 256], F32R, tag="T45")
                    r0, r1 = 32 * s - 1, 32 * s + 33
                    if r0 < 0:
                        nc.vector.memset(T45.bitcast(F32)[:, 0, :], 0.0)
                        r0 = 0
                    if r1 > 256:
                        nc.vector.memset(T45.bitcast(F32)[:, 33, :], 0.0)
                        r1 = 256
                    yy0 = r0 - (32 * s - 1)
                    nrr = r1 - r0
                    # zero cols 0 & 255 across all dx blocks (base partition 0),
                    # then the DMAs below overwrite where real data exists
                    nc.vector.memset(T45.bitcast(F32)[:, :, 0], 0.0)
                    nc.vector.memset(T45.bitcast(F32)[:, :, 255], 0.0)
                    # dx=0: src cols [0,255) -> dest cols [1,256), col 0 zero
                    nc.sync.dma_start(out=T45[0:15, yy0:yy0 + nrr, 1:256],
                                      in_=in1d[:, r0:r1, 0:255].bitcast(F32R))
                    # dx=1: full
                    nc.sync.dma_start(out=T45[15:30, yy0:yy0 + nrr, :],
                                      in_=in1d[:, r0:r1, :].bitcast(F32R))
                    # dx=2: src cols [1,256) -> dest cols [0,255), col 255 zero
                    nc.sync.dma_start(out=T45[30:45, yy0:yy0 + nrr, 0:255],
                                      in_=in1d[:, r0:r1, 1:256].bitcast(F32R))
                    yscS = b1st.tile([64, 32, 256], BF16, tag="yscS")
                    h = s // 4
                    for i in range(16):
                        yo = 2 * i
                        ps = b1ps.tile([128, 2, 256], F32, tag="ps1")
                        for dy in range(3):
                            nc.tensor.matmul(ps, r32(w1l[:, dy, :]),
                                             r32(T45[:, yo + dy:yo + dy + 2, :]),
                                             start=(dy == 0), stop=(dy == 2))
                        psf = ps.rearrange("p a b -> p (a b)")
                        seg1.add(nc, psf)
                        rr = (32 * s + yo) % 128
                        nc.scalar.copy(Y1[64 * h:64 * h + 64, rr * 256:(rr + 2) * 256],
                                       psf[0:64, :])
                        nc.scalar.copy(yscS[:, yo:yo + 2, :], ps[64:128, :, :])
                    nc.sync.dma_start(out=yscD1[:, 32 * s:32 * s + 32, :], in_=yscS)
                mv1 = seg1.finish(nc, small, "seg1")
                pk1 = _pack_stats(nc, small, mv1, 128, "pk1")
            g1 = _allreduce(nc, small, [pk1], cc["ar1_1"], replica, "ar11")[0]
            gb1a = _load_gb(nc, small, wt["gb_1"], 0, 128, "gb1a")
            ac1 = _unpack_stats(nc, small, g1, gb1a, 128, "ac1")
            acs_d = small.tile([128, 2], F32, tag="acs_d")
            nc.scalar.copy(acs_d[0:64, :], ac1[64:128, :])
            nc.scalar.copy(acs_d[64:128, :], ac1[64:128, :])

            # ---------------- pass B: t1 = relu(bn(y1)); conv2 ----------------
            with tc.tile_pool(name="b1b", bufs=2) as b1b, \
                 tc.tile_pool(name="b1ps2", bufs=4, space="PSUM") as b1ps2, \
                 tc.tile_pool(name="b1st2", bufs=2) as b1st2:
                seg2 = Seg(small, 64, 128, "seg2")
                for s in range(8):
                    t1 = b1b.tile([64, 34, 258], F32R, tag="t1")
                    nc.vector.memset(t1.bitcast(F32)[:, :, 0], 0.0)
                    nc.vector.memset(t1.bitcast(F32)[:, :, 257], 0.0)
                    r0, r1 = 32 * s - 1, 32 * s + 33
                    if r0 < 0:
                        nc.vector.memset(t1.bitcast(F32)[:, 0, :], 0.0)
                        r0 = 0
                    if r1 > 256:
                        nc.vector.memset(t1.bitcast(F32)[:, 33, :], 0.0)
                        r1 = 256
                    spans = []
                    if r0 < 128:
                        spans.append((0, r0, min(r1, 128)))
                    if r1 > 128:
                        spans.append((1, max(r0, 128), r1))
                    for h, a, bnd in spans:
                        Yv = Y1[64 * h:64 * h + 64,
                                (a % 128) * 256:((a % 128) + (bnd - a)) * 256]
                        yy = a - (32 * s - 1)
                        nc.scalar.activation(
                            out=t1[:, yy:yy + (bnd - a), 1:257],
                            in_=Yv.rearrange("p (r c) -> p r c", c=256),
                            func=ACTF.Relu,
                            bias=ac1[0:64, 1:2], scale=ac1[0:64, 0:1])
                    y2S = b1st2.tile([64, 32, 256], BF16, tag="y2S")
                    for i in range(16):
                        yo = 2 * i
                        ps = b1ps2.tile([64, 2, 256], F32, tag="ps2")
                        first = True
                        for dy in range(3):
                            for dx in range(3):
                                nc.tensor.matmul(
                                    ps, r32(w2l[:, dy * 3 + dx, :]),
                                    r32(t1[:, yo + dy:yo + dy + 2, dx:dx + 256]),
                                    start=first, stop=(dy == 2 and dx == 2))
                                first = False
                        psf = ps.rearrange("p a b -> p (a b)")
                        seg2.add(nc, psf)
                        nc.scalar.copy(y2S[:, yo:yo + 2, :], ps)
                    nc.sync.dma_start(out=y2D1[:, 32 * s:32 * s + 32, :], in_=y2S)
                mv2 = seg2.finish(nc, small, "seg2")
                pk2 = _pack_stats(nc, small, mv2, 64, "pk2")
        # Y1 pool closed here
        g2 = _allreduce(nc, small, [pk2], cc["ar2_1"], replica, "ar21")[0]
        gb1b = _load_gb(nc, small, wt["gb_1"], 128, 64, "gb1b")
        ac2 = _unpack_stats(nc, small, g2, gb1b, 64, "ac2")
        ac2_d = small.tile([128, 2], F32, tag="ac2_d")
        nc.scalar.copy(ac2_d[0:64, :], ac2)
        nc.scalar.copy(ac2_d[64:128, :], ac2)
        ccs1 = small.tile([128, 1], F32, tag="ccs1")
        nc.vector.tensor_tensor(out=ccs1, in0=acs_d[:, 1:2], in1=ac2_d[:, 1:2], op=OP.add)

    # in2sb spans blk1 pass C .. blk2 pass A
    p_in2 = tc.tile_pool(name="p_in2", bufs=1, side="right")
    in2p = p_in2.__enter__()
    in2sb = in2p.tile([76, 130, 130], F32R, tag="in2sb")
    _pad_memset(nc, in2sb)
    # wav2 channels
    nc.sync.dma_start(out=in2sb[64:76, 1:129, 1:129], in_=wav_d[2][:, :, :].bitcast(F32R))

    # ---------------- blk1 pass C ----------------
    with tc.tile_pool(name="b1c", bufs=2) as b1c:
        for j in range(16):
            # chunk covers rows [8j,8j+8) of each half
            y2c = b1c.tile([128, 8, 256], BF16, tag="y2c")
            ysc = b1c.tile([128, 8, 256], BF16, tag="ysc")
            for h in range(2):
                rb = 128 * h + 8 * j
                nc.sync.dma_start(out=y2c[64 * h:64 * h + 64, :, :],
                                  in_=y2D1[:, rb:rb + 8, :])
                nc.sync.dma_start(out=ysc[64 * h:64 * h + 64, :, :],
                                  in_=yscD1[:, rb:rb + 8, :])
            tmp = b1c.tile([128, 8, 256], F32, tag="tmpc")
            nc.scalar.activation(out=tmp, in_=ysc, func=ACTF.Identity,
                                 bias=ccs1[:, 0:1], scale=acs_d[:, 0:1])
            nc.vector.scalar_tensor_tensor(
                out=tmp, in0=y2c, scalar=ac2_d[:, 0:1], in1=tmp,
                op0=OP.mult, op1=OP.add)
            nc.scalar.activation(out=tmp, in_=tmp, func=ACTF.Relu)
            for h in range(2):
                rb = 128 * h + 8 * j
                nc.sync.dma_start(out=eout[1][:, rb:rb + 8, :],
                                  in_=tmp[64 * h:64 * h + 64, :, :])
            m1 = b1c.tile([128, 8, 128], F32, tag="m1")
            tv = tmp.rearrange("p r (c d) -> p r c d", d=2)
            nc.vector.tensor_tensor(out=m1, in0=tv[:, :, :, 0], in1=tv[:, :, :, 1], op=OP.max)
            m2 = b1c.tile([128, 4, 128], F32, tag="m2")
            m1v = m1.rearrange("p (r d) c -> p r d c", d=2)
            nc.vector.tensor_tensor(out=m2, in0=m1v[:, :, 0, :], in1=m1v[:, :, 1, :], op=OP.max)
            for h in range(2):
                rb = 64 * h + 4 * j
                nc.sync.dma_start(out=in2sb[0:64, 1 + rb:1 + rb + 4, 1:129],
                                  in_=m2[64 * h:64 * h + 64, :, :].bitcast(F32R))

    # =====================================================================
    # Block 2
    # =====================================================================
    _emit_block2(tc, nc, wt, eout, in2sb, p_in2, yscD2, wav_d, cc, replica, small)

    # =====================================================================
    # Blocks 3..5
    # =====================================================================
    # in3a was created by _emit_block2 pass C (returned via small registry)
    _emit_blockk(tc, nc, 3, wt, eout, cc, replica, small, wav_d,
                 stream_w1=False, stream_w2=False)
    _emit_blockk(tc, nc, 4, wt, eout, cc, replica, small, wav_d,
                 stream_w1=True, stream_w2=True)
    _emit_blockk(tc, nc, 5, wt, eout, cc, replica, small, wav_d,
                 stream_w1=True, stream_w2=True)


_NEXT_IN = {}  # k -> list of [128, Spad, Spad] pool-input tiles (built by k-1)


def _open_next_in(tc, nc, k):
    """Create block k's pool-channel input tiles (padded, borders zeroed)."""
    spec = BLOCKS[k]
    S, Ktp = spec["S"], spec["Cp"] // 128 if k >= 3 else 1
    Spad = S + 2
    pool_cm = tc.tile_pool(name=f"p_in{k}", bufs=1, side="right")
    p = pool_cm.__enter__()
    nt = max(1, spec["Cp"] // 128)
    npart = 128 if spec["Cp"] >= 128 else spec["Cp"]
    tiles = []
    for i in range(nt):
        t = p.tile([npart, Spad, Spad], F32R, tag=f"in{k}_{i}", name=f"in{k}_{i}")
        tiles.append(t)
    for t in tiles:
        _pad_memset(nc, t)
    _NEXT_IN[k] = (tiles, pool_cm, p)
    return tiles


def _emit_block2(tc, nc, wt, eout, in2sb, p_in2_cm, yscD2, wav_d, cc, replica, small):
    S, Spad, nr = 128, 130, 4
    ngr = S // nr

    with tc.tile_pool(name="b2y", bufs=1) as b2y:
        y1b = b2y.tile([128, S * S], F32, tag="y1b2")
        with tc.tile_pool(name="b2w", bufs=1) as b2w:
            w1l = b2w.tile([76, 10, 128], F32R, tag="w1l2")
            nc.sync.dma_start(out=w1l, in_=wt["w1_2"][:, :, :].rearrange("a b c -> b a c").bitcast(F32R))
            w2l = b2w.tile([128, 9, 128], F32R, tag="w2l2")
            nc.sync.dma_start(out=w2l, in_=wt["w2_2"][:, :, :].rearrange("a b c -> b a c").bitcast(F32R))

            # ---- pass A: conv1 + sc ----
            with tc.tile_pool(name="b2ps", bufs=4, space="PSUM") as psp, \
                 tc.tile_pool(name="b2st", bufs=2) as stp:
                seg1 = Seg(small, 128, ngr, "b2seg1")
                segs = Seg(small, 128, ngr, "b2segs")
                for g in range(ngr):
                    y0 = g * nr
                    ps = psp.tile([128, nr, S], F32, tag="psA")
                    first = True
                    for dy in range(3):
                        for dx in range(3):
                            nc.tensor.matmul(
                                ps, r32(w1l[:, dy * 3 + dx, :]),
                                r32(in2sb[:, y0 + dy:y0 + dy + nr, dx:dx + S]),
                                start=first, stop=(dy == 2 and dx == 2))
                            first = False
                    pss = psp.tile([128, nr, S], F32, tag="psS")
                    nc.tensor.matmul(pss, r32(w1l[:, 9, :]),
                                     r32(in2sb[:, 1 + y0:1 + y0 + nr, 1:1 + S]),
                                     start=True, stop=True)
                    seg1.add(nc, ps.rearrange("p a b -> p (a b)"))
                    segs.add(nc, pss.rearrange("p a b -> p (a b)"))
                    nc.scalar.copy(y1b[:, y0 * S:(y0 + nr) * S],
                                   ps.rearrange("p a b -> p (a b)"))
                    yscS = stp.tile([128, nr, S], BF16, tag="yscS2")
                    nc.scalar.copy(yscS, pss)
                    nc.sync.dma_start(out=yscD2[:, y0:y0 + nr, :], in_=yscS)
                mv1 = seg1.finish(nc, small, "b2seg1")
                mvs = segs.finish(nc, small, "b2segs")
                pk1 = _pack_stats(nc, small, mv1, 128, "b2pk1")
                pks = _pack_stats(nc, small, mvs, 128, "b2pks")
            # in2sb dead from here
            p_in2_cm.__exit__(None, None, None)
            gars = _allreduce(nc, small, [pk1, pks], cc["ar1_2"], replica, "b2ar1")
            ac1 = _unpack_stats(nc, small, gars[0], _load_gb(nc, small, wt["gb_2"], 0, 128, "gb2_1"), 128, "b2ac1")
            acs = _unpack_stats(nc, small, gars[1], _load_gb(nc, small, wt["gb_2"], 128, 128, "gb2_s"), 128, "b2acs")

            # ---- pass B: t1 strips + conv2 ----
            with tc.tile_pool(name="b2y2", bufs=1) as b2y2:
                y2b = b2y2.tile([128, S * S], BF16, tag="y2b2")
                with tc.tile_pool(name="b2b", bufs=3) as b2b, \
                     tc.tile_pool(name="b2ps2", bufs=4, space="PSUM") as psp2:
                    seg2 = Seg(small, 128, ngr, "b2seg2")
                    y1v = y1b.rearrange("p (r c) -> p r c", c=S)
                    for g in range(ngr):
                        y0 = g * nr
                        t1 = b2b.tile([128, nr + 2, Spad], F32R, tag="t1s2")
                        nc.vector.memset(t1.bitcast(F32)[:, :, 0], 0.0)
                        nc.vector.memset(t1.bitcast(F32)[:, :, Spad - 1], 0.0)
                        r0, r1 = y0 - 1, y0 + nr + 1
                        if r0 < 0:
                            nc.vector.memset(t1.bitcast(F32)[:, 0, :], 0.0)
                            r0 = 0
                        if r1 > S:
                            nc.vector.memset(t1.bitcast(F32)[:, nr + 1, :], 0.0)
                            r1 = S
                        nc.scalar.activation(
                            out=t1[:, r0 - (y0 - 1):r1 - (y0 - 1), 1:1 + S],
                            in_=y1v[:, r0:r1, :], func=ACTF.Relu,
                            bias=ac1[:, 1:2], scale=ac1[:, 0:1])
                        ps = psp2.tile([128, nr, S], F32, tag="psB")
                        first = True
                        for dy in range(3):
                            for dx in range(3):
                                nc.tensor.matmul(
                                    ps, r32(w2l[:, dy * 3 + dx, :]),
                                    r32(t1[:, dy:dy + nr, dx:dx + S]),
                                    start=first, stop=(dy == 2 and dx == 2))
                                first = False
                        seg2.add(nc, ps.rearrange("p a b -> p (a b)"))
                        nc.scalar.copy(
                            y2b.rearrange("p (r c) -> p r c", c=S)[:, y0:y0 + nr, :], ps)
                    mv2 = seg2.finish(nc, small, "b2seg2")
                    pk2 = _pack_stats(nc, small, mv2, 128, "b2pk2")
                # y1b dead
                g2 = _allreduce(nc, small, [pk2], cc["ar2_2"], replica, "b2ar2")[0]
                ac2 = _unpack_stats(nc, small, g2, _load_gb(nc, small, wt["gb_2"], 256, 128, "gb2_2"), 128, "b2ac2")
                ccs = small.tile([128, 1], F32, tag="b2ccs")
                nc.vector.tensor_tensor(out=ccs, in0=acs[:, 1:2], in1=ac2[:, 1:2], op=OP.add)

                # ---- pass C ----
                in3 = _open_next_in(tc, nc, 3)
                with tc.tile_pool(name="b2c", bufs=2) as bc:
                    for j in range(32):
                        rb = 4 * j
                        ysc = bc.tile([128, 4, S], BF16, tag="yscC2")
                        nc.sync.dma_start(out=ysc, in_=yscD2[:, rb:rb + 4, :])
                        tmp = bc.tile([128, 4, S], F32, tag="tmpC2")
                        nc.scalar.activation(out=tmp, in_=ysc, func=ACTF.Identity,
                                             bias=ccs[:, 0:1], scale=acs[:, 0:1])
                        y2v = y2b.rearrange("p (r c) -> p r c", c=S)[:, rb:rb + 4, :]
                        nc.vector.scalar_tensor_tensor(
                            out=tmp, in0=y2v, scalar=ac2[:, 0:1], in1=tmp,
                            op0=OP.mult, op1=OP.add)
                        nc.scalar.activation(out=tmp, in_=tmp, func=ACTF.Relu)
                        nc.sync.dma_start(out=eout[2][:, rb:rb + 4, :], in_=tmp)
                        m1 = bc.tile([128, 4, 64], F32, tag="m1C2")
                        tv = tmp.rearrange("p r (c d) -> p r c d", d=2)
                        nc.vector.tensor_tensor(out=m1, in0=tv[:, :, :, 0],
                                                in1=tv[:, :, :, 1], op=OP.max)
                        m2 = bc.tile([128, 2, 64], F32, tag="m2C2")
                        m1v = m1.rearrange("p (r d) c -> p r d c", d=2)
                        nc.vector.tensor_tensor(out=m2, in0=m1v[:, :, 0, :],
                                                in1=m1v[:, :, 1, :], op=OP.max)
                        nc.sync.dma_start(out=in3[0][:, 1 + 2 * j:1 + 2 * j + 2, 1:65],
                                          in_=m2.bitcast(F32R))


def _emit_blockk(tc, nc, k, wt, eout, cc, replica, small, wav_d,
                 stream_w1=False, stream_w2=False):
    spec = BLOCKS[k]
    Cp, Cout, S, nr = spec["Cp"], spec["Cout"], spec["S"], spec["nr"]
    Spad = S + 2
    Mt, Ktp = Cout // 128, Cp // 128
    ngr = S // nr
    inP, inP_cm, inP_pool = _NEXT_IN[k]

    with tc.tile_pool(name=f"bk{k}w", bufs=1) as bkw:
        bkT_cm = tc.tile_pool(name=f"bk{k}T", bufs=1, side="right")
        bkT = bkT_cm.__enter__()
        # wavelet im2col from DRAM (padded implicitly via shifts + zero pad)
        T108 = bkT.tile([108, S, S], F32R, tag=f"T108_{k}")
        wpad = bkT.tile([12, Spad, Spad], F32R, tag=f"wpad{k}")
        _pad_memset(nc, wpad)
        nc.sync.dma_start(out=wpad[:, 1:1 + S, 1:1 + S], in_=wav_d[k][:, :, :].bitcast(F32R))
        for ti, t in enumerate(T_ORDER):
            dy, dx = t // 3, t % 3
            nc.sync.dma_start(out=T108[12 * ti:12 * ti + 12, :, :],
                              in_=wpad[:, dy:dy + S, dx:dx + S])
        w1wl = bkw.tile([108, Mt, 128], F32R, tag=f"w1wl{k}")
        nc.sync.dma_start(out=w1wl, in_=wt[f"w1w_{k}"][:, :, :].rearrange("a b c -> b a c").bitcast(F32R))
        wswl = bkw.tile([12, Mt, 128], F32R, tag=f"wswl{k}")
        nc.sync.dma_start(out=wswl, in_=wt[f"wsw_{k}"][:, :, :].rearrange("a b c -> b a c").bitcast(F32R))
        wspl = bkw.tile([128, Mt, Ktp, 128], F32R, tag=f"wspl{k}")
        nc.sync.dma_start(out=wspl, in_=wt[f"ws_{k}"][:, :, :, :].rearrange("a b c d -> c a b d").bitcast(F32R))
        if not stream_w1:
            w1pl = bkw.tile([128, Mt, Ktp, 9, 128], F32R, tag=f"w1pl{k}")
            nc.sync.dma_start(out=w1pl,
                              in_=wt[f"w1_{k}"][:, :, :, :, :].rearrange("a b c d e -> d a b c e").bitcast(F32R))

        with tc.tile_pool(name=f"bk{k}ys", bufs=1) as bkys:
            y1b = [inP_pool.tile([128, S * S], F32, tag=f"y1b{k}_{m}", name=f"y1b{k}_{m}") for m in range(Mt)]
            yscb = [bkys.tile([128, S * S], F32, tag=f"yscb{k}_{m}", name=f"yscb{k}_{m}") for m in range(Mt)]

            # ---- pass A ----
            with tc.tile_pool(name=f"b{k}ps", bufs=4, space="PSUM") as psp, \
                 tc.tile_pool(name=f"b{k}wst", bufs=2) as wstr:
                seg1 = [Seg(small, 128, ngr, f"b{k}seg1_{m}") for m in range(Mt)]
                segs = [Seg(small, 128, ngr, f"b{k}segs_{m}") for m in range(Mt)]
                for m in range(Mt):
                    w1m = {}
                    if stream_w1:
                        for kt in range(Ktp):
                            w1kt = wstr.tile([128, 9, 128], F32R, tag=f"w1m{kt % 2}",
                                             name=f"w1m_{m}_{kt}")
                            nc.sync.dma_start(out=w1kt,
                                              in_=wt[f"w1_{k}"][m, kt].rearrange("d e f -> e d f").bitcast(F32R))
                            w1m[kt] = w1kt
                    for g in range(ngr):
                        y0 = g * nr
                        ps = psp.tile([128, nr, S], F32, tag="psA")
                        first = True
                        for kt in range(Ktp):
                            for dy in range(3):
                                for dx in range(3):
                                    lw = (w1m[kt][:, dy * 3 + dx, :] if stream_w1
                                          else w1pl[:, m, kt, dy * 3 + dx, :])
                                    nc.tensor.matmul(
                                        ps, r32(lw),
                                        r32(inP[kt][:, y0 + dy:y0 + dy + nr, dx:dx + S]),
                                        start=first, stop=False)
                                    first = False
                        nc.tensor.matmul(ps, r32(w1wl[:, m, :]),
                                         r32(T108[:, y0:y0 + nr, :]),
                                         start=False, stop=True)
                        pss = psp.tile([128, nr, S], F32, tag="psS")
                        for kt in range(Ktp):
                            nc.tensor.matmul(pss, r32(wspl[:, m, kt, :]),
                                             r32(inP[kt][:, 1 + y0:1 + y0 + nr, 1:1 + S]),
                                             start=(kt == 0), stop=False)
                        nc.tensor.matmul(pss, r32(wswl[:, m, :]),
                                         r32(T108[0:12, y0:y0 + nr, :]),
                                         start=False, stop=True)
                        seg1[m].add(nc, ps.rearrange("p a b -> p (a b)"))
                        segs[m].add(nc, pss.rearrange("p a b -> p (a b)"))
                        nc.scalar.copy(y1b[m][:, y0 * S:(y0 + nr) * S],
                                       ps.rearrange("p a b -> p (a b)"))
                        nc.scalar.copy(yscb[m][:, y0 * S:(y0 + nr) * S],
                                       pss.rearrange("p a b -> p (a b)"))
                pks = [_pack_stats(nc, small, seg1[m].finish(nc, small, f"b{k}seg1_{m}"),
                                   128, f"b{k}p1{m}") for m in range(Mt)] + \
                      [_pack_stats(nc, small, segs[m].finish(nc, small, f"b{k}segs_{m}"),
                                   128, f"b{k}ps{m}") for m in range(Mt)]
            bkT_cm.__exit__(None, None, None)  # T108/wpad dead after pass A
            gl = _allreduce(nc, small, pks, cc[f"ar1_{k}"], replica, f"b{k}ar1")
            ac1 = [_unpack_stats(nc, small, gl[m],
                                 _load_gb(nc, small, wt[f"gb_{k}"], m * 128, 128, f"gbl{k}1{m}"),
                                 128, f"b{k}ac1{m}") for m in range(Mt)]
            acs = [_unpack_stats(nc, small, gl[Mt + m],
                                 _load_gb(nc, small, wt[f"gb_{k}"], Cout + m * 128, 128, f"gbl{k}s{m}"),
                                 128, f"b{k}acs{m}") for m in range(Mt)]

            # ---- pass B ----
            with tc.tile_pool(name=f"bk{k}y2", bufs=1) as bky2:
                y2b = [bky2.tile([128, S * S], F32, tag=f"y2b{k}_{m}", name=f"y2b{k}_{m}") for m in range(Mt)]
                with tc.tile_pool(name=f"b{k}t1s", bufs=2) as bt1s, \
                     tc.tile_pool(name=f"b{k}ps2", bufs=4, space="PSUM") as psp2, \
                     tc.tile_pool(name=f"b{k}wst2", bufs=2) as wstr2:
                    seg2 = [Seg(small, 128, ngr, f"b{k}seg2_{m}") for m in range(Mt)]
                    if not stream_w2:
                        w2lf = bkw.tile([128, Mt, Mt, 9, 128], F32R, tag=f"w2l{k}")
                        nc.sync.dma_start(out=w2lf,
                                          in_=wt[f"w2_{k}"][:, :, :, :, :].rearrange("a b c d e -> d a b c e").bitcast(F32R))
                    for m in range(Mt):
                        w2m = {}
                        if stream_w2:
                            for kt in range(Mt):
                                w2kt = wstr2.tile([128, 9, 128], F32R, tag=f"w2m{kt % 2}",
                                                  name=f"w2m_{m}_{kt}")
                                nc.sync.dma_start(out=w2kt,
                                                  in_=wt[f"w2_{k}"][m, kt].rearrange("d e f -> e d f").bitcast(F32R))
                                w2m[kt] = w2kt
                        for g in range(ngr):
                            y0 = g * nr
                            t1s = []
                            for kt in range(Mt):
                                t1k = bt1s.tile([128, nr + 2, Spad], F32R,
                                                tag=f"t1s{kt}", name=f"t1s{kt}")
                                nc.vector.memset(t1k.bitcast(F32)[:, :, 0], 0.0)
                                nc.vector.memset(t1k.bitcast(F32)[:, :, Spad - 1], 0.0)
                                r0, r1 = y0 - 1, y0 + nr + 1
                                if r0 < 0:
                                    nc.vector.memset(t1k.bitcast(F32)[:, 0, :], 0.0)
                                    r0 = 0
                                if r1 > S:
                                    nc.vector.memset(t1k.bitcast(F32)[:, nr + 1, :], 0.0)
                                    r1 = S
                                nc.scalar.activation(
                                    out=t1k[:, r0 - (y0 - 1):r1 - (y0 - 1), 1:1 + S],
                                    in_=y1b[kt].rearrange("p (r c) -> p r c", c=S)[:, r0:r1, :],
                                    func=ACTF.Relu, bias=ac1[kt][:, 1:2], scale=ac1[kt][:, 0:1])
                                t1s.append(t1k)
                            ps = psp2.tile([128, nr, S], F32, tag="psB")
                            first = True
                            for kt in range(Mt):
                                for dy in range(3):
                                    for dx in range(3):
                                        lw = (w2m[kt][:, dy * 3 + dx, :] if stream_w2
                                              else w2lf[:, m, kt, dy * 3 + dx, :])
                                        nc.tensor.matmul(
                                            ps, r32(lw),
                                            r32(t1s[kt][:, dy:dy + nr, dx:dx + S]),
                                            start=first,
                                            stop=(kt == Mt - 1 and dy == 2 and dx == 2))
                                        first = False
                            seg2[m].add(nc, ps.rearrange("p a b -> p (a b)"))
                            nc.scalar.copy(y2b[m][:, y0 * S:(y0 + nr) * S],
                                           ps.rearrange("p a b -> p (a b)"))
                    pk2 = [_pack_stats(nc, small, seg2[m].finish(nc, small, f"b{k}seg2_{m}"),
                                       128, f"b{k}p2{m}") for m in range(Mt)]
                inP_cm.__exit__(None, None, None)  # in-tiles + y1b dead
                gl2 = _allreduce(nc, small, pk2, cc[f"ar2_{k}"], replica, f"b{k}ar2")
                ac2 = [_unpack_stats(nc, small, gl2[m],
                                     _load_gb(nc, small, wt[f"gb_{k}"], 2 * Cout + m * 128, 128, f"gbl{k}2{m}"),
                                     128, f"b{k}ac2{m}") for m in range(Mt)]

                # ---- pass C ----
                outP = _open_next_in(tc, nc, k + 1) if k < 5 else None
                ncch = S // 16 if S >= 32 else 1   # row chunks
                rch = S // ncch
                with tc.tile_pool(name=f"b{k}c", bufs=2) as bc:
                    for m in range(Mt):
                        ccs = small.tile([128, 1], F32, tag=f"b{k}ccs{m}")
                        nc.vector.tensor_tensor(out=ccs, in0=acs[m][:, 1:2],
                                                in1=ac2[m][:, 1:2], op=OP.add)
                        for ch in range(ncch):
                            rb = ch * rch
                            tmp = bc.tile([128, rch, S], F32, tag="tmpC")
                            nc.scalar.activation(
                                out=tmp,
                                in_=yscb[m].rearrange("p (r c) -> p r c", c=S)[:, rb:rb + rch, :],
                                func=ACTF.Identity, bias=ccs[:, 0:1], scale=acs[m][:, 0:1])
                            nc.vector.scalar_tensor_tensor(
                                out=tmp,
                                in0=y2b[m].rearrange("p (r c) -> p r c", c=S)[:, rb:rb + rch, :],
                                scalar=ac2[m][:, 0:1], in1=tmp, op0=OP.mult, op1=OP.add)
                            nc.scalar.activation(out=tmp, in_=tmp, func=ACTF.Relu)
                            nc.sync.dma_start(out=eout[k][m * 128:(m + 1) * 128, rb:rb + rch, :], in_=tmp)
                            if k < 5:
                                m1 = bc.tile([128, rch, S // 2], F32, tag="m1C")
                                tv = tmp.rearrange("p r (c d) -> p r c d", d=2)
                                nc.vector.tensor_tensor(out=m1, in0=tv[:, :, :, 0],
                                                        in1=tv[:, :, :, 1], op=OP.max)
                                m2 = bc.tile([128, rch // 2, S // 2], F32, tag="m2C")
                                m1v = m1.rearrange("p (r d) c -> p r d c", d=2)
                                nc.vector.tensor_tensor(out=m2, in0=m1v[:, :, 0, :],
                                                        in1=m1v[:, :, 1, :], op=OP.max)
                                nc.sync.dma_start(
                                    out=outP[m][:, 1 + rb // 2:1 + rb // 2 + rch // 2, 1:1 + S // 2],
                                    in_=m2.bitcast(F32R))



# ---------------------------------------------------------------------------
# entry point
# ---------------------------------------------------------------------------

_NC_CACHE = {}


def _get_nc():
    if "nc" not in _NC_CACHE:
        _NC_CACHE["nc"] = _build_nc()
    return _NC_CACHE["nc"]


def kernel(x_img, params):
    x_img = np.asarray(x_img, dtype=np.float32)
    P = _prep_weights(params)
    nc = _get_nc()
    in_maps = []
    for i in range(N_CORES):
        m = {"x": np.ascontiguousarray(x_img[i])}
        m.update(P)
        in_maps.append(m)
    res = run_bass_kernel_spmd(nc, in_maps, core_ids=list(range(N_CORES)))
    outs = []
    for k in range(1, 6):
        ek = np.stack([res.results[i][f"e{k}"] for i in range(N_CORES)], axis=0)
        outs.append(ek)
    return (x_img, *outs)


# revision 30
# speedup vs baseline: 1.0092x; 1.0092x over previous
"""Trainium2 Bass kernel for nn_EncoderWav (wavelet CNN encoder).

Strategy: pure data parallelism — 8 images, one per NeuronCore. Sync-BN
batch statistics are combined with tiny AllReduce collectives (2 per
residual block). Convolutions run as fp32r matmuls on the tensor engine
with taps accumulated in PSUM; the first block folds (dx, cin) into the
contraction dim to use the 128-wide PE array despite cin=15.
"""

import sys
from contextlib import ExitStack

sys.path.insert(0, "/opt/trn_rl_repo")

import numpy as np  # noqa: E402

import concourse.bass as bass  # noqa: E402
import concourse.bacc as bacc  # noqa: E402
import concourse.tile as tile  # noqa: E402
import concourse.mybir as mybir  # noqa: E402
from concourse.bass_utils import run_bass_kernel_spmd  # noqa: E402

F32 = mybir.dt.float32
F32R = mybir.dt.float32r
BF16 = mybir.dt.bfloat16
OP = mybir.AluOpType
ACTF = mybir.ActivationFunctionType
BN_EPS = 1e-5
N_CORES = 8

# block specs (k>=2): (Cp pool chans, Cout, S spatial, rows-per-matmul)
BLOCKS = {
    2: dict(Cp=64, Cout=128, S=128, nr=4),
    3: dict(Cp=128, Cout=256, S=64, nr=8),
    4: dict(Cp=256, Cout=512, S=32, nr=16),
    5: dict(Cp=512, Cout=1024, S=16, nr=16),
}
T_ORDER = [4, 0, 1, 2, 3, 5, 6, 7, 8]  # tap (1,1) first so sc rhs has base partition 0
OUT_SHAPES = {
    1: (64, 256, 256), 2: (128, 128, 128), 3: (256, 64, 64),
    4: (512, 32, 32), 5: (1024, 16, 16),
}


def r32(ap):
    return ap.bitcast(F32R)


# ---------------------------------------------------------------------------
# host-side weight packing
# ---------------------------------------------------------------------------

def _prep_weights(params):
    """Transform conv weights into lhsT tensors for the kernel.

    Wavelet inputs are computed unnormalized on device (plain subband sums,
    no 0.5 factors); the 2^-k scale of level-k wavelets is folded into the
    conv weights that consume them. Conv biases are dropped entirely:
    train-mode BN directly follows every conv, and BN(y + b) == BN(y).
    """
    P = {}
    f32 = lambda a: np.ascontiguousarray(np.asarray(a, dtype=np.float32))

    # ---- block 1 ----
    b = params["blk1"]
    w1 = f32(b["w1"]).copy()              # [64, 15, 3, 3]
    ws = f32(b["ws"])[:, :, 0, 0].copy()  # [64, 15]
    w1[:, 3:, :, :] *= 0.5                # wav1 channels carry 2x scale
    ws[:, 3:] *= 0.5
    w1_1 = np.zeros((3, 45, 128), np.float32)
    for dy in range(3):
        for dx in range(3):
            w1_1[dy, dx * 15:dx * 15 + 15, 0:64] = w1[:, :, dy, dx].T
    w1_1[1, 15:30, 64:128] = ws.T          # shortcut rides tap (dy=1, dx=1)
    P["w1_1"] = w1_1
    w2 = f32(b["w2"])                      # [64, 64, 3, 3]
    w2_1 = np.zeros((9, 64, 64), np.float32)
    for dy in range(3):
        for dx in range(3):
            w2_1[dy * 3 + dx] = w2[:, :, dy, dx].T
    P["w2_1"] = w2_1
    gb = np.zeros((192, 2), np.float32)
    gb[0:64, 0], gb[0:64, 1] = f32(b["g1"]), f32(b["be1"])
    gb[64:128, 0], gb[64:128, 1] = f32(b["gs"]), f32(b["bes"])
    gb[128:192, 0], gb[128:192, 1] = f32(b["g2"]), f32(b["be2"])
    P["gb_1"] = gb

    # ---- block 2 ----
    b = params["blk2"]
    w1 = f32(b["w1"]).copy()               # [128, 76, 3, 3]
    ws = f32(b["ws"])[:, :, 0, 0].copy()
    w1[:, 64:, :, :] *= 0.25               # wav2 carries 4x
    ws[:, 64:] *= 0.25
    w1_2 = np.zeros((10, 76, 128), np.float32)
    for dy in range(3):
        for dx in range(3):
            w1_2[dy * 3 + dx] = w1[:, :, dy, dx].T
    w1_2[9] = ws.T
    P["w1_2"] = w1_2
    w2 = f32(b["w2"])
    w2_2 = np.zeros((9, 128, 128), np.float32)
    for dy in range(3):
        for dx in range(3):
            w2_2[dy * 3 + dx] = w2[:, :, dy, dx].T
    P["w2_2"] = w2_2
    gb = np.zeros((384, 2), np.float32)
    gb[0:128, 0], gb[0:128, 1] = f32(b["g1"]), f32(b["be1"])
    gb[128:256, 0], gb[128:256, 1] = f32(b["gs"]), f32(b["bes"])
    gb[256:384, 0], gb[256:384, 1] = f32(b["g2"]), f32(b["be2"])
    P["gb_2"] = gb

    # ---- blocks 3..5 ----
    for k in (3, 4, 5):
        spec = BLOCKS[k]
        Cp, Cout = spec["Cp"], spec["Cout"]
        Mt, Ktp = Cout // 128, Cp // 128
        b = params[f"blk{k}"]
        w1 = f32(b["w1"]).copy()
        ws = f32(b["ws"])[:, :, 0, 0].copy()
        wavscale = 0.5 ** k
        w1[:, Cp:, :, :] *= wavscale
        ws[:, Cp:] *= wavscale
        w1p = np.zeros((Mt, Ktp, 9, 128, 128), np.float32)
        w1w = np.zeros((Mt, 108, 128), np.float32)
        wsp = np.zeros((Mt, Ktp, 128, 128), np.float32)
        wsw = np.zeros((Mt, 12, 128), np.float32)
        for m in range(Mt):
            wm = w1[m * 128:(m + 1) * 128]
            for kt in range(Ktp):
                for dy in range(3):
                    for dx in range(3):
                        w1p[m, kt, dy * 3 + dx] = wm[:, kt * 128:(kt + 1) * 128, dy, dx].T
                wsp[m, kt] = ws[m * 128:(m + 1) * 128, kt * 128:(kt + 1) * 128].T
            for ti, t in enumerate(T_ORDER):
                dy, dx = t // 3, t % 3
                w1w[m, ti * 12:(ti + 1) * 12, :] = wm[:, Cp:, dy, dx].T
            wsw[m] = ws[m * 128:(m + 1) * 128, Cp:].T
        P[f"w1_{k}"] = w1p
        P[f"w1w_{k}"] = w1w
        P[f"ws_{k}"] = wsp
        P[f"wsw_{k}"] = wsw
        w2 = f32(b["w2"])
        w2p = np.zeros((Mt, Mt, 9, 128, 128), np.float32)
        for m in range(Mt):
            for kt in range(Mt):
                for dy in range(3):
                    for dx in range(3):
                        w2p[m, kt, dy * 3 + dx] = \
                            w2[m * 128:(m + 1) * 128, kt * 128:(kt + 1) * 128, dy, dx].T
        P[f"w2_{k}"] = w2p
        gb = np.zeros((3 * Cout, 2), np.float32)
        gb[0:Cout, 0], gb[0:Cout, 1] = f32(b["g1"]), f32(b["be1"])
        gb[Cout:2 * Cout, 0], gb[Cout:2 * Cout, 1] = f32(b["gs"]), f32(b["bes"])
        gb[2 * Cout:, 0], gb[2 * Cout:, 1] = f32(b["g2"]), f32(b["be2"])
        P[f"gb_{k}"] = gb
    return P


# ---------------------------------------------------------------------------
# device-side helpers
# ---------------------------------------------------------------------------

class Seg:
    """Accumulates bn_stats chunks for one [P, npix] conv-output segment."""

    def __init__(self, pool, P, nchunks, tag):
        self.P = P
        self.buf = pool.tile([P, max(nchunks, 1), 6], F32, tag=tag)
        self.n = 0

    def add(self, nc, src_flat):
        nc.vector.bn_stats(out=self.buf[:, self.n, :], in_=src_flat)
        self.n += 1

    def finish(self, nc, pool, tag):
        mv = pool.tile([self.P, 2], F32, tag=tag + "_mv")
        nc.vector.bn_aggr(out=mv, in_=self.buf[:, :self.n, :])
        return mv


def _pack_stats(nc, pool, mv, P, tag):
    """[P,2] (mean,var) -> [P,2] (mean/8, (var+mean^2)/8)."""
    pk = pool.tile([P, 2], F32, tag=tag + "_pk")
    tmp = pool.tile([P, 1], F32, tag=tag + "_tmp")
    nc.vector.tensor_tensor(out=tmp, in0=mv[:, 0:1], in1=mv[:, 0:1], op=OP.mult)
    nc.vector.tensor_tensor(out=pk[:, 1:2], in0=mv[:, 1:2], in1=tmp, op=OP.add)
    nc.scalar.mul(pk[:, 1:2], pk[:, 1:2], 1.0 / N_CORES)
    nc.scalar.mul(pk[:, 0:1], mv[:, 0:1], 1.0 / N_CORES)
    return pk


def _unpack_stats(nc, pool, g, gamma_beta, P, tag):
    """g [P,2] = (mean, E[x^2]) -> ac [P,2] = (a, c):
    a = gamma * rsqrt(var + eps), c = beta - mean * a."""
    ac = pool.tile([P, 2], F32, tag=tag + "_ac")
    tmp = pool.tile([P, 1], F32, tag=tag + "_t1")
    var = pool.tile([P, 1], F32, tag=tag + "_t2")
    nc.vector.tensor_tensor(out=tmp, in0=g[:, 0:1], in1=g[:, 0:1], op=OP.mult)
    nc.vector.tensor_tensor(out=var, in0=g[:, 1:2], in1=tmp, op=OP.subtract)
    eps = pool.tile([P, 1], F32, tag=tag + "_eps")
    nc.vector.memset(eps, BN_EPS)
    nc.scalar.activation(out=var, in_=var, func=ACTF.Sqrt, bias=eps, scale=1.0)
    nc.vector.reciprocal(out=var, in_=var)
    nc.vector.tensor_tensor(out=ac[:, 0:1], in0=var, in1=gamma_beta[:, 0:1], op=OP.mult)
    nc.vector.tensor_tensor(out=tmp, in0=g[:, 0:1], in1=ac[:, 0:1], op=OP.mult)
    nc.vector.tensor_tensor(out=ac[:, 1:2], in0=gamma_beta[:, 1:2], in1=tmp, op=OP.subtract)
    return ac


def _allreduce(nc, pool, segs_pk, cc_pair, replica, tag):
    """Pack per-segment [P,2] tiles into cci, AllReduce, read back tiles of
    (global mean, global E[x^2])."""
    cci, cco = cc_pair
    off = 0
    for pk in segs_pk:
        Pp = pk.shape[0]
        nc.sync.dma_start(out=cci[0, off:off + 2 * Pp], in_=pk)
        off += 2 * Pp
    nc.gpsimd.collective_compute(
        "AllReduce", OP.add, ins=[cci[:, :]], outs=[cco[:, :]],
        replica_groups=[replica])
    outs = []
    off = 0
    for i, pk in enumerate(segs_pk):
        Pp = pk.shape[0]
        g = pool.tile([Pp, 2], F32, tag=f"{tag}_g{i}")
        nc.sync.dma_start(out=g, in_=cco[0, off:off + 2 * Pp])
        off += 2 * Pp
        outs.append(g)
    return outs


def _load_gb(nc, pool, dram, row0, P, tag):
    t = pool.tile([P, 2], F32, tag=tag)
    nc.sync.dma_start(out=t, in_=dram[row0:row0 + P, :])
    return t


def _pad_memset(nc, t):
    S2 = t.shape[1]
    tb = t.bitcast(F32) if t.dtype == F32R else t
    nc.vector.memset(tb[:, 0, :], 0.0)
    nc.vector.memset(tb[:, S2 - 1, :], 0.0)
    nc.vector.memset(tb[:, :, 0], 0.0)
    nc.vector.memset(tb[:, :, S2 - 1], 0.0)


# ---------------------------------------------------------------------------
# kernel body
# ---------------------------------------------------------------------------

def _build_nc(num_devices=N_CORES, replica=None):
    if replica is None:
        replica = list(range(num_devices))
    nc = bacc.Bacc("TRN2", target_bir_lowering=False, debug=False,
                   num_devices=num_devices)

    x = nc.dram_tensor("x", [3, 512, 512], F32, kind="ExternalInput")
    wt = {}
    wt["w1_1"] = nc.dram_tensor("w1_1", [3, 45, 128], F32, kind="ExternalInput")
    wt["w2_1"] = nc.dram_tensor("w2_1", [9, 64, 64], F32, kind="ExternalInput")
    wt["gb_1"] = nc.dram_tensor("gb_1", [192, 2], F32, kind="ExternalInput")
    wt["w1_2"] = nc.dram_tensor("w1_2", [10, 76, 128], F32, kind="ExternalInput")
    wt["w2_2"] = nc.dram_tensor("w2_2", [9, 128, 128], F32, kind="ExternalInput")
    wt["gb_2"] = nc.dram_tensor("gb_2", [384, 2], F32, kind="ExternalInput")
    for k in (3, 4, 5):
        Cp, Cout = BLOCKS[k]["Cp"], BLOCKS[k]["Cout"]
        Mt, Ktp = Cout // 128, Cp // 128
        wt[f"w1_{k}"] = nc.dram_tensor(f"w1_{k}", [Mt, Ktp, 9, 128, 128], F32, kind="ExternalInput")
        wt[f"w1w_{k}"] = nc.dram_tensor(f"w1w_{k}", [Mt, 108, 128], F32, kind="ExternalInput")
        wt[f"ws_{k}"] = nc.dram_tensor(f"ws_{k}", [Mt, Ktp, 128, 128], F32, kind="ExternalInput")
        wt[f"wsw_{k}"] = nc.dram_tensor(f"wsw_{k}", [Mt, 12, 128], F32, kind="ExternalInput")
        wt[f"w2_{k}"] = nc.dram_tensor(f"w2_{k}", [Mt, Mt, 9, 128, 128], F32, kind="ExternalInput")
        wt[f"gb_{k}"] = nc.dram_tensor(f"gb_{k}", [3 * Cout, 2], F32, kind="ExternalInput")

    eout = {k: nc.dram_tensor(f"e{k}", list(OUT_SHAPES[k]), F32, kind="ExternalOutput")
            for k in range(1, 6)}

    in1d = nc.dram_tensor("in1d", [45, 258, 256], F32)  # (dx,c), 1+256+1 rows, shifted cols
    ll_d = {1: nc.dram_tensor("ll1", [3, 256, 256], F32),
            2: nc.dram_tensor("ll2", [3, 128, 128], F32),
            3: nc.dram_tensor("ll3", [3, 64, 64], F32),
            4: nc.dram_tensor("ll4", [3, 32, 32], F32)}
    wav_d = {2: nc.dram_tensor("wav2", [12, 128, 128], F32),
             3: nc.dram_tensor("wav3", [12, 64, 64], F32),
             4: nc.dram_tensor("wav4", [12, 32, 32], F32),
             5: nc.dram_tensor("wav5", [12, 16, 16], F32)}
    yscD1 = nc.dram_tensor("yscD1", [64, 256, 256], BF16)
    y2D1 = nc.dram_tensor("y2D1", [64, 256, 256], BF16)
    yscD2 = nc.dram_tensor("yscD2", [128, 128, 128], BF16)
    cc = {}
    for tag, n in (("ar1_1", 256), ("ar2_1", 128), ("ar1_2", 512), ("ar2_2", 256),
                   ("ar1_3", 1024), ("ar2_3", 512), ("ar1_4", 2048), ("ar2_4", 1024),
                   ("ar1_5", 4096), ("ar2_5", 2048)):
        cc[tag] = (nc.dram_tensor(f"cci_{tag}", [1, n], F32),
                   nc.dram_tensor(f"cco_{tag}", [1, n], F32, addr_space="Shared"))

    with tile.TileContext(nc, pool_alloc_mode="queue") as tc:
        with ExitStack() as ctx:
            _emit(ctx, tc, nc, x, wt, eout, in1d, ll_d, wav_d,
                  yscD1, y2D1, yscD2, cc, replica)
    nc.compile()
    return nc


def _phase_wavelets(tc, nc, x, in1d, ll_d, wav_d, levels=(1, 2, 3, 4, 5)):
    with tc.tile_pool(name="wv", bufs=3) as wv:
        # zero pad rows (r=0, r=257) of the pre-shifted in1d45
        zr = wv.tile([128, 8192], F32, tag="zr", bufs=1)
        nc.vector.memset(zr, 0.0)
        nc.sync.dma_start(out=in1d[:, 0, :], in_=zr[0:45, 0:256])
        nc.sync.dma_start(out=in1d[:, 257, :], in_=zr[0:45, 0:256])
        # level 1: x [3,512,512] -> in1d channels (pool + wav1) + ll1
        for g in (range(6) if 1 in levels else ()):
            c, half = g // 2, g % 2
            y0 = 128 * half
            E = wv.tile([128, 512], F32, tag="E")
            Ot = wv.tile([128, 512], F32, tag="O")
            nc.sync.dma_start(out=E, in_=x[c, 2 * y0:2 * y0 + 256:2, :])
            nc.sync.dma_start(out=Ot, in_=x[c, 2 * y0 + 1:2 * y0 + 256:2, :])
            Ev = E.rearrange("p (a b) -> p a b", b=2)
            Ov = Ot.rearrange("p (a b) -> p a b", b=2)
            p_ = wv.tile([128, 256], F32, tag="p_")
            m_ = wv.tile([128, 256], F32, tag="m_")
            q_ = wv.tile([128, 256], F32, tag="q_")
            n_ = wv.tile([128, 256], F32, tag="n_")
            nc.vector.tensor_tensor(out=p_, in0=Ev[:, :, 0], in1=Ev[:, :, 1], op=OP.add)
            nc.vector.tensor_tensor(out=m_, in0=Ev[:, :, 0], in1=Ev[:, :, 1], op=OP.subtract)
            nc.vector.tensor_tensor(out=q_, in0=Ov[:, :, 0], in1=Ov[:, :, 1], op=OP.add)
            nc.vector.tensor_tensor(out=n_, in0=Ov[:, :, 0], in1=Ov[:, :, 1], op=OP.subtract)
            sb = wv.tile([128, 4, 256], F32, tag="sb")
            nc.vector.tensor_tensor(out=sb[:, 0, :], in0=p_, in1=q_, op=OP.add)
            nc.vector.tensor_tensor(out=sb[:, 1, :], in0=p_, in1=q_, op=OP.subtract)
            nc.vector.tensor_tensor(out=sb[:, 2, :], in0=m_, in1=n_, op=OP.add)
            nc.vector.tensor_tensor(out=sb[:, 3, :], in0=m_, in1=n_, op=OP.subtract)
            pl = wv.tile([128, 256], F32, tag="pl")
            nc.vector.tensor_tensor(out=p_, in0=Ev[:, :, 0], in1=Ev[:, :, 1], op=OP.max)
            nc.vector.tensor_tensor(out=q_, in0=Ov[:, :, 0], in1=Ov[:, :, 1], op=OP.max)
            nc.vector.tensor_tensor(out=pl, in0=p_, in1=q_, op=OP.max)
            # in1d45[(dx,c'), 1+r, x] = in1[c', r, x-1+dx]; c' = channel index (15)
            for ch_, src in [(c, pl)] + [(3 + 4 * c + s, sb[:, s, :]) for s in range(4)]:
                nc.sync.dma_start(out=in1d[15 * 0 + ch_, 1 + y0:1 + y0 + 128, 1:256],
                                  in_=src[:, 0:255])
                nc.sync.dma_start(out=in1d[15 * 0 + ch_, 1 + y0:1 + y0 + 128, 0:1],
                                  in_=zr[0:128, 0:1])
                nc.sync.dma_start(out=in1d[15 * 1 + ch_, 1 + y0:1 + y0 + 128, :],
                                  in_=src)
                nc.sync.dma_start(out=in1d[15 * 2 + ch_, 1 + y0:1 + y0 + 128, 0:255],
                                  in_=src[:, 1:256])
                nc.sync.dma_start(out=in1d[15 * 2 + ch_, 1 + y0:1 + y0 + 128, 255:256],
                                  in_=zr[0:128, 0:1])
            nc.sync.dma_start(out=ll_d[1][c, y0:y0 + 128, :], in_=sb[:, 0, :])

        # levels 2..5 -> wav_d planes (+ ll chain)
        for lvl in (2, 3, 4, 5):
            if lvl not in levels:
                continue
            src = ll_d[lvl - 1]
            Sin = src.shape[1]
            Sout = Sin // 2
            for c in range(3):
                ng = (Sout + 127) // 128
                for gi in range(ng):
                    y0 = gi * 128
                    nrr = min(128, Sout - y0)
                    E = wv.tile([128, 512], F32, tag="E")
                    Ot = wv.tile([128, 512], F32, tag="O")
                    nc.sync.dma_start(out=E[:nrr, :Sin], in_=src[c, 2 * y0:2 * (y0 + nrr):2, :])
                    nc.sync.dma_start(out=Ot[:nrr, :Sin], in_=src[c, 2 * y0 + 1:2 * (y0 + nrr):2, :])
                    Ev = E[:nrr, :Sin].rearrange("p (a b) -> p a b", b=2)
                    Ov = Ot[:nrr, :Sin].rearrange("p (a b) -> p a b", b=2)
                    p_ = wv.tile([128, 256], F32, tag="p_")
                    m_ = wv.tile([128, 256], F32, tag="m_")
                    q_ = wv.tile([128, 256], F32, tag="q_")
                    n_ = wv.tile([128, 256], F32, tag="n_")
                    nc.vector.tensor_tensor(out=p_[:nrr, :Sout], in0=Ev[:, :, 0], in1=Ev[:, :, 1], op=OP.add)
                    nc.vector.tensor_tensor(out=m_[:nrr, :Sout], in0=Ev[:, :, 0], in1=Ev[:, :, 1], op=OP.subtract)
                    nc.vector.tensor_tensor(out=q_[:nrr, :Sout], in0=Ov[:, :, 0], in1=Ov[:, :, 1], op=OP.add)
                    nc.vector.tensor_tensor(out=n_[:nrr, :Sout], in0=Ov[:, :, 0], in1=Ov[:, :, 1], op=OP.subtract)
                    sb = wv.tile([128, 4, 256], F32, tag="sb")
                    nc.vector.tensor_tensor(out=sb[:nrr, 0, :Sout], in0=p_[:nrr, :Sout], in1=q_[:nrr, :Sout], op=OP.add)
                    nc.vector.tensor_tensor(out=sb[:nrr, 1, :Sout], in0=p_[:nrr, :Sout], in1=q_[:nrr, :Sout], op=OP.subtract)
                    nc.vector.tensor_tensor(out=sb[:nrr, 2, :Sout], in0=m_[:nrr, :Sout], in1=n_[:nrr, :Sout], op=OP.add)
                    nc.vector.tensor_tensor(out=sb[:nrr, 3, :Sout], in0=m_[:nrr, :Sout], in1=n_[:nrr, :Sout], op=OP.subtract)
                    for s in range(4):
                        nc.sync.dma_start(out=wav_d[lvl][4 * c + s, y0:y0 + nrr, :],
                                          in_=sb[:nrr, s, :Sout])
                    if lvl < 5:
                        nc.sync.dma_start(out=ll_d[lvl][c, y0:y0 + nrr, :],
                                          in_=sb[:nrr, 0, :Sout])


def _emit(ctx, tc, nc, x, wt, eout, in1d, ll_d, wav_d, yscD1, y2D1, yscD2,
          cc, replica):
    _NEXT_IN.clear()
    small = ctx.enter_context(tc.tile_pool(name="small", bufs=1))

    _phase_wavelets(tc, nc, x, in1d, ll_d, wav_d, levels=(1,))

    # =====================================================================
    # Block 1 (strip-tiled; y1 SBUF bf16; ysc & y2 staged in DRAM bf16)
    # =====================================================================

    with tc.tile_pool(name="b1w", bufs=1) as b1w:
        w1l = b1w.tile([45, 3, 128], F32R, tag="w1l")
        nc.sync.dma_start(out=w1l, in_=wt["w1_1"][:, :, :].rearrange("a b c -> b a c").bitcast(F32R))
        w2l = b1w.tile([64, 9, 64], F32R, tag="w2l")
        nc.sync.dma_start(out=w2l, in_=wt["w2_1"][:, :, :].rearrange("a b c -> b a c").bitcast(F32R))

        _phase_wavelets(tc, nc, x, in1d, ll_d, wav_d, levels=(2, 3, 4, 5))

        with tc.tile_pool(name="pY1", bufs=1) as pY1:
            Y1 = pY1.tile([128, 34816], BF16, tag="Y1")  # part 64h+c; free (r%128)*256+x

            # ---------------- pass A: conv1 + shortcut ----------------
            with tc.tile_pool(name="b1a", bufs=2) as b1a, \
                 tc.tile_pool(name="b1ps", bufs=4, space="PSUM") as b1ps, \
                 tc.tile_pool(name="b1st", bufs=2) as b1st:
                seg1 = Seg(small, 128, 128, "seg1")
                for s in range(8):
                    # T45[(dx,c), yy, x] = in1[c, 32s-1+yy, x-1+dx] (pre-shifted DRAM)
                    T45 = b1a.tile([45, 34, 256], F32R, tag="T45")
                    nc.sync.dma_start(out=T45[:, :, :],
                                      in_=in1d[:, 32 * s:32 * s + 34, :].bitcast(F32R))
                    yscS = b1st.tile([64, 32, 256], BF16, tag="yscS")
                    h = s // 4
                    for i in range(16):
                        yo = 2 * i
                        ps = b1ps.tile([128, 2, 256], F32, tag="ps1")
                        for dy in range(3):
                            nc.tensor.matmul(ps, r32(w1l[:, dy, :]),
                                             r32(T45[:, yo + dy:yo + dy + 2, :]),
                                             start=(dy == 0), stop=(dy == 2))
                        psf = ps.rearrange("p a b -> p (a b)")
                        seg1.add(nc, psf)
                        rr = (32 * s + yo) % 128
                        nc.scalar.copy(Y1[64 * h:64 * h + 64, rr * 256:(rr + 2) * 256],
                                       psf[0:64, :])
                        nc.scalar.copy(yscS[:, yo:yo + 2, :], ps[64:128, :, :])
                    nc.sync.dma_start(out=yscD1[:, 32 * s:32 * s + 32, :], in_=yscS)
                mv1 = seg1.finish(nc, small, "seg1")
                pk1 = _pack_stats(nc, small, mv1, 128, "pk1")
            g1 = _allreduce(nc, small, [pk1], cc["ar1_1"], replica, "ar11")[0]
            gb1a = _load_gb(nc, small, wt["gb_1"], 0, 128, "gb1a")
            ac1 = _unpack_stats(nc, small, g1, gb1a, 128, "ac1")
            acs_d = small.tile([128, 2], F32, tag="acs_d")
            nc.scalar.copy(acs_d[0:64, :], ac1[64:128, :])
            nc.scalar.copy(acs_d[64:128, :], ac1[64:128, :])

            # ---------------- pass B: t1 = relu(bn(y1)); conv2 ----------------
            with tc.tile_pool(name="b1b", bufs=2) as b1b, \
                 tc.tile_pool(name="b1ps2", bufs=4, space="PSUM") as b1ps2, \
                 tc.tile_pool(name="b1st2", bufs=2) as b1st2:
                seg2 = Seg(small, 64, 128, "seg2")
                for s in range(8):
                    t1 = b1b.tile([64, 34, 258], F32R, tag="t1")
                    nc.vector.memset(t1.bitcast(F32)[:, :, 0], 0.0)
                    nc.vector.memset(t1.bitcast(F32)[:, :, 257], 0.0)
                    r0, r1 = 32 * s - 1, 32 * s + 33
                    if r0 < 0:
                        nc.vector.memset(t1.bitcast(F32)[:, 0, :], 0.0)
                        r0 = 0
                    if r1 > 256:
                        nc.vector.memset(t1.bitcast(F32)[:, 33, :], 0.0)
                        r1 = 256
                    spans = []
                    if r0 < 128:
                        spans.append((0, r0, min(r1, 128)))
                    if r1 > 128:
                        spans.append((1, max(r0, 128), r1))
                    for h, a, bnd in spans:
                        Yv = Y1[64 * h:64 * h + 64,
                                (a % 128) * 256:((a % 128) + (bnd - a)) * 256]
                        yy = a - (32 * s - 1)
                        nc.scalar.activation(
                            out=t1[:, yy:yy + (bnd - a), 1:257],
                            in_=Yv.rearrange("p (r c) -> p r c", c=256),
                            func=ACTF.Relu,
                            bias=ac1[0:64, 1:2], scale=ac1[0:64, 0:1])
                    y2S = b1st2.tile([64, 32, 256], BF16, tag="y2S")
                    for i in range(16):
                        yo = 2 * i
                        ps = b1ps2.tile([64, 2, 256], F32, tag="ps2")
                        first = True
                        for dy in range(3):
                            for dx in range(3):
                                nc.tensor.matmul(
                                    ps, r32(w2l[:, dy * 3 + dx, :]),
                                    r32(t1[:, yo + dy:yo + dy + 2, dx:dx + 256]),
                                    start=first, stop=(dy == 2 and dx == 2))
                                first = False
                        psf = ps.rearrange("p a b -> p (a b)")
                        seg2.add(nc, psf)
                        nc.scalar.copy(y2S[:, yo:yo + 2, :], ps)
                    nc.sync.dma_start(out=y2D1[:, 32 * s:32 * s + 32, :], in_=y2S)
                mv2 = seg2.finish(nc, small, "seg2")
                pk2 = _pack_stats(nc, small, mv2, 64, "pk2")
        # Y1 pool closed here
        g2 = _allreduce(nc, small, [pk2], cc["ar2_1"], replica, "ar21")[0]
        gb1b = _load_gb(nc, small, wt["gb_1"], 128, 64, "gb1b")
        ac2 = _unpack_stats(nc, small, g2, gb1b, 64, "ac2")
        ac2_d = small.tile([128, 2], F32, tag="ac2_d")
        nc.scalar.copy(ac2_d[0:64, :], ac2)
        nc.scalar.copy(ac2_d[64:128, :], ac2)
        ccs1 = small.tile([128, 1], F32, tag="ccs1")
        nc.vector.tensor_tensor(out=ccs1, in0=acs_d[:, 1:2], in1=ac2_d[:, 1:2], op=OP.add)

    # in2sb spans blk1 pass C .. blk2 pass A
    p_in2 = tc.tile_pool(name="p_in2", bufs=1, side="right")
    in2p = p_in2.__enter__()
    in2sb = in2p.tile([76, 130, 130], F32R, tag="in2sb")
    _pad_memset(nc, in2sb)
    # wav2 channels
    nc.sync.dma_start(out=in2sb[64:76, 1:129, 1:129], in_=wav_d[2][:, :, :].bitcast(F32R))

    # ---------------- blk1 pass C ----------------
    with tc.tile_pool(name="b1c", bufs=2) as b1c:
        for j in range(16):
            # chunk covers rows [8j,8j+8) of each half
            y2c = b1c.tile([128, 8, 256], BF16, tag="y2c")
            ysc = b1c.tile([128, 8, 256], BF16, tag="ysc")
            for h in range(2):
                rb = 128 * h + 8 * j
                nc.sync.dma_start(out=y2c[64 * h:64 * h + 64, :, :],
                                  in_=y2D1[:, rb:rb + 8, :])
                nc.sync.dma_start(out=ysc[64 * h:64 * h + 64, :, :],
                                  in_=yscD1[:, rb:rb + 8, :])
            tmp = b1c.tile([128, 8, 256], F32, tag="tmpc")
            nc.scalar.activation(out=tmp, in_=ysc, func=ACTF.Identity,
                                 bias=ccs1[:, 0:1], scale=acs_d[:, 0:1])
            nc.vector.scalar_tensor_tensor(
                out=tmp, in0=y2c, scalar=ac2_d[:, 0:1], in1=tmp,
                op0=OP.mult, op1=OP.add)
            nc.scalar.activation(out=tmp, in_=tmp, func=ACTF.Relu)
            for h in range(2):
                rb = 128 * h + 8 * j
                nc.sync.dma_start(out=eout[1][:, rb:rb + 8, :],
                                  in_=tmp[64 * h:64 * h + 64, :, :])
            m1 = b1c.tile([128, 8, 128], F32, tag="m1")
            tv = tmp.rearrange("p r (c d) -> p r c d", d=2)
            nc.vector.tensor_tensor(out=m1, in0=tv[:, :, :, 0], in1=tv[:, :, :, 1], op=OP.max)
            m2 = b1c.tile([128, 4, 128], F32, tag="m2")
            m1v = m1.rearrange("p (r d) c -> p r d c", d=2)
            nc.vector.tensor_tensor(out=m2, in0=m1v[:, :, 0, :], in1=m1v[:, :, 1, :], op=OP.max)
            for h in range(2):
                rb = 64 * h + 4 * j
                nc.sync.dma_start(out=in2sb[0:64, 1 + rb:1 + rb + 4, 1:129],
                                  in_=m2[64 * h:64 * h + 64, :, :].bitcast(F32R))

    # =====================================================================
    # Block 2
    # =====================================================================
    _emit_block2(tc, nc, wt, eout, in2sb, p_in2, yscD2, wav_d, cc, replica, small)

    # =====================================================================
    # Blocks 3..5
    # =====================================================================
    # in3a was created by _emit_block2 pass C (returned via small registry)
    _emit_blockk(tc, nc, 3, wt, eout, cc, replica, small, wav_d,
                 stream_w1=False, stream_w2=False)
    _emit_blockk(tc, nc, 4, wt, eout, cc, replica, small, wav_d,
                 stream_w1=True, stream_w2=True)
    _emit_blockk(tc, nc, 5, wt, eout, cc, replica, small, wav_d,
                 stream_w1=True, stream_w2=True)


_NEXT_IN = {}  # k -> list of [128, Spad, Spad] pool-input tiles (built by k-1)


def _open_next_in(tc, nc, k):
    """Create block k's pool-channel input tiles (padded, borders zeroed)."""
    spec = BLOCKS[k]
    S, Ktp = spec["S"], spec["Cp"] // 128 if k >= 3 else 1
    Spad = S + 2
    pool_cm = tc.tile_pool(name=f"p_in{k}", bufs=1, side="right")
    p = pool_cm.__enter__()
    nt = max(1, spec["Cp"] // 128)
    npart = 128 if spec["Cp"] >= 128 else spec["Cp"]
    tiles = []
    for i in range(nt):
        t = p.tile([npart, Spad, Spad], F32R, tag=f"in{k}_{i}", name=f"in{k}_{i}")
        tiles.append(t)
    for t in tiles:
        _pad_memset(nc, t)
    _NEXT_IN[k] = (tiles, pool_cm, p)
    return tiles


def _emit_block2(tc, nc, wt, eout, in2sb, p_in2_cm, yscD2, wav_d, cc, replica, small):
    S, Spad, nr = 128, 130, 4
    ngr = S // nr

    with tc.tile_pool(name="b2y", bufs=1) as b2y:
        y1b = b2y.tile([128, S * S], F32, tag="y1b2")
        with tc.tile_pool(name="b2w", bufs=1) as b2w:
            w1l = b2w.tile([76, 10, 128], F32R, tag="w1l2")
            nc.sync.dma_start(out=w1l, in_=wt["w1_2"][:, :, :].rearrange("a b c -> b a c").bitcast(F32R))
            w2l = b2w.tile([128, 9, 128], F32R, tag="w2l2")
            nc.sync.dma_start(out=w2l, in_=wt["w2_2"][:, :, :].rearrange("a b c -> b a c").bitcast(F32R))

            # ---- pass A: conv1 + sc ----
            with tc.tile_pool(name="b2ps", bufs=4, space="PSUM") as psp, \
                 tc.tile_pool(name="b2st", bufs=2) as stp:
                seg1 = Seg(small, 128, ngr, "b2seg1")
                segs = Seg(small, 128, ngr, "b2segs")
                for g in range(ngr):
                    y0 = g * nr
                    ps = psp.tile([128, nr, S], F32, tag="psA")
                    first = True
                    for dy in range(3):
                        for dx in range(3):
                            nc.tensor.matmul(
                                ps, r32(w1l[:, dy * 3 + dx, :]),
                                r32(in2sb[:, y0 + dy:y0 + dy + nr, dx:dx + S]),
                                start=first, stop=(dy == 2 and dx == 2))
                            first = False
                    pss = psp.tile([128, nr, S], F32, tag="psS")
                    nc.tensor.matmul(pss, r32(w1l[:, 9, :]),
                                     r32(in2sb[:, 1 + y0:1 + y0 + nr, 1:1 + S]),
                                     start=True, stop=True)
                    seg1.add(nc, ps.rearrange("p a b -> p (a b)"))
                    segs.add(nc, pss.rearrange("p a b -> p (a b)"))
                    nc.scalar.copy(y1b[:, y0 * S:(y0 + nr) * S],
                                   ps.rearrange("p a b -> p (a b)"))
                    yscS = stp.tile([128, nr, S], BF16, tag="yscS2")
                    nc.scalar.copy(yscS, pss)
                    nc.sync.dma_start(out=yscD2[:, y0:y0 + nr, :], in_=yscS)
                mv1 = seg1.finish(nc, small, "b2seg1")
                mvs = segs.finish(nc, small, "b2segs")
                pk1 = _pack_stats(nc, small, mv1, 128, "b2pk1")
                pks = _pack_stats(nc, small, mvs, 128, "b2pks")
            # in2sb dead from here
            p_in2_cm.__exit__(None, None, None)
            gars = _allreduce(nc, small, [pk1, pks], cc["ar1_2"], replica, "b2ar1")
            ac1 = _unpack_stats(nc, small, gars[0], _load_gb(nc, small, wt["gb_2"], 0, 128, "gb2_1"), 128, "b2ac1")
            acs = _unpack_stats(nc, small, gars[1], _load_gb(nc, small, wt["gb_2"], 128, 128, "gb2_s"), 128, "b2acs")

            # ---- pass B: t1 strips + conv2 ----
            with tc.tile_pool(name="b2y2", bufs=1) as b2y2:
                y2b = b2y2.tile([128, S * S], BF16, tag="y2b2")
                with tc.tile_pool(name="b2b", bufs=3) as b2b, \
                     tc.tile_pool(name="b2ps2", bufs=4, space="PSUM") as psp2:
                    seg2 = Seg(small, 128, ngr, "b2seg2")
                    y1v = y1b.rearrange("p (r c) -> p r c", c=S)
                    for g in range(ngr):
                        y0 = g * nr
                        t1 = b2b.tile([128, nr + 2, Spad], F32R, tag="t1s2")
                        nc.vector.memset(t1.bitcast(F32)[:, :, 0], 0.0)
                        nc.vector.memset(t1.bitcast(F32)[:, :, Spad - 1], 0.0)
                        r0, r1 = y0 - 1, y0 + nr + 1
                        if r0 < 0:
                            nc.vector.memset(t1.bitcast(F32)[:, 0, :], 0.0)
                            r0 = 0
                        if r1 > S:
                            nc.vector.memset(t1.bitcast(F32)[:, nr + 1, :], 0.0)
                            r1 = S
                        nc.scalar.activation(
                            out=t1[:, r0 - (y0 - 1):r1 - (y0 - 1), 1:1 + S],
                            in_=y1v[:, r0:r1, :], func=ACTF.Relu,
                            bias=ac1[:, 1:2], scale=ac1[:, 0:1])
                        ps = psp2.tile([128, nr, S], F32, tag="psB")
                        first = True
                        for dy in range(3):
                            for dx in range(3):
                                nc.tensor.matmul(
                                    ps, r32(w2l[:, dy * 3 + dx, :]),
                                    r32(t1[:, dy:dy + nr, dx:dx + S]),
                                    start=first, stop=(dy == 2 and dx == 2))
                                first = False
                        seg2.add(nc, ps.rearrange("p a b -> p (a b)"))
                        nc.scalar.copy(
                            y2b.rearrange("p (r c) -> p r c", c=S)[:, y0:y0 + nr, :], ps)
                    mv2 = seg2.finish(nc, small, "b2seg2")
                    pk2 = _pack_stats(nc, small, mv2, 128, "b2pk2")
                # y1b dead
                g2 = _allreduce(nc, small, [pk2], cc["ar2_2"], replica, "b2ar2")[0]
                ac2 = _unpack_stats(nc, small, g2, _load_gb(nc, small, wt["gb_2"], 256, 128, "gb2_2"), 128, "b2ac2")
                ccs = small.tile([128, 1], F32, tag="b2ccs")
                nc.vector.tensor_tensor(out=ccs, in0=acs[:, 1:2], in1=ac2[:, 1:2], op=OP.add)

                # ---- pass C ----
                in3 = _open_next_in(tc, nc, 3)
                with tc.tile_pool(name="b2c", bufs=2) as bc:
                    for j in range(32):
                        rb = 4 * j
                        ysc = bc.tile([128, 4, S], BF16, tag="yscC2")
                        nc.sync.dma_start(out=ysc, in_=yscD2[:, rb:rb + 4, :])
                        tmp = bc.tile([128, 4, S], F32, tag="tmpC2")
                        nc.scalar.activation(out=tmp, in_=ysc, func=ACTF.Identity,
                                             bias=ccs[:, 0:1], scale=acs[:, 0:1])
                        y2v = y2b.rearrange("p (r c) -> p r c", c=S)[:, rb:rb + 4, :]
                        nc.vector.scalar_tensor_tensor(
                            out=tmp, in0=y2v, scalar=ac2[:, 0:1], in1=tmp,
                            op0=OP.mult, op1=OP.add)
                        nc.scalar.activation(out=tmp, in_=tmp, func=ACTF.Relu)
                        nc.sync.dma_start(out=eout[2][:, rb:rb + 4, :], in_=tmp)
                        m1 = bc.tile([128, 4, 64], F32, tag="m1C2")
                        tv = tmp.rearrange("p r (c d) -> p r c d", d=2)
                        nc.vector.tensor_tensor(out=m1, in0=tv[:, :, :, 0],
                                                in1=tv[:, :, :, 1], op=OP.max)
                        m2 = bc.tile([128, 2, 64], F32, tag="m2C2")
                        m1v = m1.rearrange("p (r d) c -> p r d c", d=2)
                        nc.vector.tensor_tensor(out=m2, in0=m1v[:, :, 0, :],
                                                in1=m1v[:, :, 1, :], op=OP.max)
                        nc.sync.dma_start(out=in3[0][:, 1 + 2 * j:1 + 2 * j + 2, 1:65],
                                          in_=m2.bitcast(F32R))


def _emit_blockk(tc, nc, k, wt, eout, cc, replica, small, wav_d,
                 stream_w1=False, stream_w2=False):
    spec = BLOCKS[k]
    Cp, Cout, S, nr = spec["Cp"], spec["Cout"], spec["S"], spec["nr"]
    Spad = S + 2
    Mt, Ktp = Cout // 128, Cp // 128
    ngr = S // nr
    inP, inP_cm, inP_pool = _NEXT_IN[k]

    with tc.tile_pool(name=f"bk{k}w", bufs=1) as bkw:
        bkT_cm = tc.tile_pool(name=f"bk{k}T", bufs=1, side="right")
        bkT = bkT_cm.__enter__()
        # wavelet im2col from DRAM (padded implicitly via shifts + zero pad)
        T108 = bkT.tile([108, S, S], F32R, tag=f"T108_{k}")
        wpad = bkT.tile([12, Spad, Spad], F32R, tag=f"wpad{k}")
        _pad_memset(nc, wpad)
        nc.sync.dma_start(out=wpad[:, 1:1 + S, 1:1 + S], in_=wav_d[k][:, :, :].bitcast(F32R))
        for ti, t in enumerate(T_ORDER):
            dy, dx = t // 3, t % 3
            nc.sync.dma_start(out=T108[12 * ti:12 * ti + 12, :, :],
                              in_=wpad[:, dy:dy + S, dx:dx + S])
        w1wl = bkw.tile([108, Mt, 128], F32R, tag=f"w1wl{k}")
        nc.sync.dma_start(out=w1wl, in_=wt[f"w1w_{k}"][:, :, :].rearrange("a b c -> b a c").bitcast(F32R))
        wswl = bkw.tile([12, Mt, 128], F32R, tag=f"wswl{k}")
        nc.sync.dma_start(out=wswl, in_=wt[f"wsw_{k}"][:, :, :].rearrange("a b c -> b a c").bitcast(F32R))
        wspl = bkw.tile([128, Mt, Ktp, 128], F32R, tag=f"wspl{k}")
        nc.sync.dma_start(out=wspl, in_=wt[f"ws_{k}"][:, :, :, :].rearrange("a b c d -> c a b d").bitcast(F32R))
        if not stream_w1:
            w1pl = bkw.tile([128, Mt, Ktp, 9, 128], F32R, tag=f"w1pl{k}")
            nc.sync.dma_start(out=w1pl,
                              in_=wt[f"w1_{k}"][:, :, :, :, :].rearrange("a b c d e -> d a b c e").bitcast(F32R))

        with tc.tile_pool(name=f"bk{k}ys", bufs=1) as bkys:
            y1b = [inP_pool.tile([128, S * S], F32, tag=f"y1b{k}_{m}", name=f"y1b{k}_{m}") for m in range(Mt)]
            yscb = [bkys.tile([128, S * S], F32, tag=f"yscb{k}_{m}", name=f"yscb{k}_{m}") for m in range(Mt)]

            # ---- pass A ----
            with tc.tile_pool(name=f"b{k}ps", bufs=4, space="PSUM") as psp, \
                 tc.tile_pool(name=f"b{k}wst", bufs=2) as wstr:
                seg1 = [Seg(small, 128, ngr, f"b{k}seg1_{m}") for m in range(Mt)]
                segs = [Seg(small, 128, ngr, f"b{k}segs_{m}") for m in range(Mt)]
                for m in range(Mt):
                    w1m = {}
                    if stream_w1:
                        for kt in range(Ktp):
                            w1kt = wstr.tile([128, 9, 128], F32R, tag=f"w1m{kt % 2}",
                                             name=f"w1m_{m}_{kt}")
                            nc.sync.dma_start(out=w1kt,
                                              in_=wt[f"w1_{k}"][m, kt].rearrange("d e f -> e d f").bitcast(F32R))
                            w1m[kt] = w1kt
                    for g in range(ngr):
                        y0 = g * nr
                        ps = psp.tile([128, nr, S], F32, tag="psA")
                        first = True
                        for kt in range(Ktp):
                            for dy in range(3):
                                for dx in range(3):
                                    lw = (w1m[kt][:, dy * 3 + dx, :] if stream_w1
                                          else w1pl[:, m, kt, dy * 3 + dx, :])
                                    nc.tensor.matmul(
                                        ps, r32(lw),
                                        r32(inP[kt][:, y0 + dy:y0 + dy + nr, dx:dx + S]),
                                        start=first, stop=False)
                                    first = False
                        nc.tensor.matmul(ps, r32(w1wl[:, m, :]),
                                         r32(T108[:, y0:y0 + nr, :]),
                                         start=False, stop=True)
                        pss = psp.tile([128, nr, S], F32, tag="psS")
                        for kt in range(Ktp):
                            nc.tensor.matmul(pss, r32(wspl[:, m, kt, :]),
                                             r32(inP[kt][:, 1 + y0:1 + y0 + nr, 1:1 + S]),
                                             start=(kt == 0), stop=False)
                        nc.tensor.matmul(pss, r32(wswl[:, m, :]),
                                         r32(T108[0:12, y0:y0 + nr, :]),
                                         start=False, stop=True)
                        seg1[m].add(nc, ps.rearrange("p a b -> p (a b)"))
                        segs[m].add(nc, pss.rearrange("p a b -> p (a b)"))
                        nc.scalar.copy(y1b[m][:, y0 * S:(y0 + nr) * S],
                                       ps.rearrange("p a b -> p (a b)"))
                        nc.scalar.copy(yscb[m][:, y0 * S:(y0 + nr) * S],
                                       pss.rearrange("p a b -> p (a b)"))
                pks = [_pack_stats(nc, small, seg1[m].finish(nc, small, f"b{k}seg1_{m}"),
                                   128, f"b{k}p1{m}") for m in range(Mt)] + \
                      [_pack_stats(nc, small, segs[m].finish(nc, small, f"b{k}segs_{m}"),
                                   128, f"b{k}ps{m}") for m in range(Mt)]
            bkT_cm.__exit__(None, None, None)  # T108/wpad dead after pass A
            gl = _allreduce(nc, small, pks, cc[f"ar1_{k}"], replica, f"b{k}ar1")
            ac1 = [_unpack_stats(nc, small, gl[m],
                                 _load_gb(nc, small, wt[f"gb_{k}"], m * 128, 128, f"gbl{k}1{m}"),
                                 128, f"b{k}ac1{m}") for m in range(Mt)]
            acs = [_unpack_stats(nc, small, gl[Mt + m],
                                 _load_gb(nc, small, wt[f"gb_{k}"], Cout + m * 128, 128, f"gbl{k}s{m}"),
                                 128, f"b{k}acs{m}") for m in range(Mt)]

            # ---- pass B ----
            with tc.tile_pool(name=f"bk{k}y2", bufs=1) as bky2:
                y2b = [bky2.tile([128, S * S], F32, tag=f"y2b{k}_{m}", name=f"y2b{k}_{m}") for m in range(Mt)]
                with tc.tile_pool(name=f"b{k}t1s", bufs=2) as bt1s, \
                     tc.tile_pool(name=f"b{k}ps2", bufs=4, space="PSUM") as psp2, \
                     tc.tile_pool(name=f"b{k}wst2", bufs=2) as wstr2:
                    seg2 = [Seg(small, 128, ngr, f"b{k}seg2_{m}") for m in range(Mt)]
                    if not stream_w2:
                        w2lf = bkw.tile([128, Mt, Mt, 9, 128], F32R, tag=f"w2l{k}")
                        nc.sync.dma_start(out=w2lf,
                                          in_=wt[f"w2_{k}"][:, :, :, :, :].rearrange("a b c d e -> d a b c e").bitcast(F32R))
                    for m in range(Mt):
                        w2m = {}
                        if stream_w2:
                            for kt in range(Mt):
                                w2kt = wstr2.tile([128, 9, 128], F32R, tag=f"w2m{kt % 2}",
                                                  name=f"w2m_{m}_{kt}")
                                nc.sync.dma_start(out=w2kt,
                                                  in_=wt[f"w2_{k}"][m, kt].rearrange("d e f -> e d f").bitcast(F32R))
                                w2m[kt] = w2kt
                        for g in range(ngr):
                            y0 = g * nr
                            t1s = []
                            for kt in range(Mt):
                                t1k = bt1s.tile([128, nr + 2, Spad], F32R,
                                                tag=f"t1s{kt}", name=f"t1s{kt}")
                                nc.vector.memset(t1k.bitcast(F32)[:, :, 0], 0.0)
                                nc.vector.memset(t1k.bitcast(F32)[:, :, Spad - 1], 0.0)
                                r0, r1 = y0 - 1, y0 + nr + 1
                                if r0 < 0:
                                    nc.vector.memset(t1k.bitcast(F32)[:, 0, :], 0.0)
                                    r0 = 0
                                if r1 > S:
                                    nc.vector.memset(t1k.bitcast(F32)[:, nr + 1, :], 0.0)
                                    r1 = S
                                nc.scalar.activation(
                                    out=t1k[:, r0 - (y0 - 1):r1 - (y0 - 1), 1:1 + S],
                                    in_=y1b[kt].rearrange("p (r c) -> p r c", c=S)[:, r0:r1, :],
                                    func=ACTF.Relu, bias=ac1[kt][:, 1:2], scale=ac1[kt][:, 0:1])
                                t1s.append(t1k)
                            ps = psp2.tile([128, nr, S], F32, tag="psB")
                            first = True
                            for kt in range(Mt):
                                for dy in range(3):
                                    for dx in range(3):
                                        lw = (w2m[kt][:, dy * 3 + dx, :] if stream_w2
                                              else w2lf[:, m, kt, dy * 3 + dx, :])
                                        nc.tensor.matmul(
                                            ps, r32(lw),
                                            r32(t1s[kt][:, dy:dy + nr, dx:dx + S]),
                                            start=first,
                                            stop=(kt == Mt - 1 and dy == 2 and dx == 2))
                                        first = False
                            seg2[m].add(nc, ps.rearrange("p a b -> p (a b)"))
                            nc.scalar.copy(y2b[m][:, y0 * S:(y0 + nr) * S],
                                           ps.rearrange("p a b -> p (a b)"))
                    pk2 = [_pack_stats(nc, small, seg2[m].finish(nc, small, f"b{k}seg2_{m}"),
                                       128, f"b{k}p2{m}") for m in range(Mt)]
                inP_cm.__exit__(None, None, None)  # in-tiles + y1b dead
                gl2 = _allreduce(nc, small, pk2, cc[f"ar2_{k}"], replica, f"b{k}ar2")
                ac2 = [_unpack_stats(nc, small, gl2[m],
                                     _load_gb(nc, small, wt[f"gb_{k}"], 2 * Cout + m * 128, 128, f"gbl{k}2{m}"),
                                     128, f"b{k}ac2{m}") for m in range(Mt)]

                # ---- pass C ----
                outP = _open_next_in(tc, nc, k + 1) if k < 5 else None
                ncch = S // 16 if S >= 32 else 1   # row chunks
                rch = S // ncch
                with tc.tile_pool(name=f"b{k}c", bufs=2) as bc:
                    ccs_l = []
                    for m in range(Mt):
                        ccs = small.tile([128, 1], F32, tag=f"b{k}ccs{m}", name=f"b{k}ccs{m}")
                        nc.vector.tensor_tensor(out=ccs, in0=acs[m][:, 1:2],
                                                in1=ac2[m][:, 1:2], op=OP.add)
                        ccs_l.append(ccs)
                    for ch in range(ncch):
                        for m in range(Mt):
                            ccs = ccs_l[m]
                            rb = ch * rch
                            tmp = bc.tile([128, rch, S], F32, tag="tmpC")
                            nc.scalar.activation(
                                out=tmp,
                                in_=yscb[m].rearrange("p (r c) -> p r c", c=S)[:, rb:rb + rch, :],
                                func=ACTF.Identity, bias=ccs[:, 0:1], scale=acs[m][:, 0:1])
                            nc.vector.scalar_tensor_tensor(
                                out=tmp,
                                in0=y2b[m].rearrange("p (r c) -> p r c", c=S)[:, rb:rb + rch, :],
                                scalar=ac2[m][:, 0:1], in1=tmp, op0=OP.mult, op1=OP.add)
                            nc.scalar.activation(out=tmp, in_=tmp, func=ACTF.Relu)
                            nc.sync.dma_start(out=eout[k][m * 128:(m + 1) * 128, rb:rb + rch, :], in_=tmp)
                            if k < 5:
                                m1 = bc.tile([128, rch, S // 2], F32, tag="m1C")
                                tv = tmp.rearrange("p r (c d) -> p r c d", d=2)
                                nc.vector.tensor_tensor(out=m1, in0=tv[:, :, :, 0],
                                                        in1=tv[:, :, :, 1], op=OP.max)
                                m2 = bc.tile([128, rch // 2, S // 2], F32, tag="m2C")
                                m1v = m1.rearrange("p (r d) c -> p r d c", d=2)
                                nc.vector.tensor_tensor(out=m2, in0=m1v[:, :, 0, :],
                                                        in1=m1v[:, :, 1, :], op=OP.max)
                                nc.sync.dma_start(
                                    out=outP[m][:, 1 + rb // 2:1 + rb // 2 + rch // 2, 1:1 + S // 2],
                                    in_=m2.bitcast(F32R))



# ---------------------------------------------------------------------------
# entry point
# ---------------------------------------------------------------------------

_NC_CACHE = {}


def _get_nc():
    if "nc" not in _NC_CACHE:
        _NC_CACHE["nc"] = _build_nc()
    return _NC_CACHE["nc"]


def kernel(x_img, params):
    x_img = np.asarray(x_img, dtype=np.float32)
    P = _prep_weights(params)
    nc = _get_nc()
    in_maps = []
    for i in range(N_CORES):
        m = {"x": np.ascontiguousarray(x_img[i])}
        m.update(P)
        in_maps.append(m)
    res = run_bass_kernel_spmd(nc, in_maps, core_ids=list(range(N_CORES)))
    outs = []
    for k in range(1, 6):
        ek = np.stack([res.results[i][f"e{k}"] for i in range(N_CORES)], axis=0)
        outs.append(ek)
    return (x_img, *outs)


# revision 33
# speedup vs baseline: 1.0174x; 1.0081x over previous
"""Trainium2 Bass kernel for nn_EncoderWav (wavelet CNN encoder).

Strategy: pure data parallelism — 8 images, one per NeuronCore. Sync-BN
batch statistics are combined with tiny AllReduce collectives (2 per
residual block). Convolutions run as fp32r matmuls on the tensor engine
with taps accumulated in PSUM; the first block folds (dx, cin) into the
contraction dim to use the 128-wide PE array despite cin=15.
"""

import sys
from contextlib import ExitStack

sys.path.insert(0, "/opt/trn_rl_repo")

import numpy as np  # noqa: E402

import concourse.bass as bass  # noqa: E402
import concourse.bacc as bacc  # noqa: E402
import concourse.tile as tile  # noqa: E402
import concourse.mybir as mybir  # noqa: E402
from concourse.bass_utils import run_bass_kernel_spmd  # noqa: E402

F32 = mybir.dt.float32
F32R = mybir.dt.float32r
BF16 = mybir.dt.bfloat16
OP = mybir.AluOpType
ACTF = mybir.ActivationFunctionType
BN_EPS = 1e-5
N_CORES = 8

# block specs (k>=2): (Cp pool chans, Cout, S spatial, rows-per-matmul)
BLOCKS = {
    2: dict(Cp=64, Cout=128, S=128, nr=4),
    3: dict(Cp=128, Cout=256, S=64, nr=8),
    4: dict(Cp=256, Cout=512, S=32, nr=16),
    5: dict(Cp=512, Cout=1024, S=16, nr=16),
}
T_ORDER = [4, 0, 1, 2, 3, 5, 6, 7, 8]  # tap (1,1) first so sc rhs has base partition 0
OUT_SHAPES = {
    1: (64, 256, 256), 2: (128, 128, 128), 3: (256, 64, 64),
    4: (512, 32, 32), 5: (1024, 16, 16),
}


def r32(ap):
    return ap.bitcast(F32R)


# ---------------------------------------------------------------------------
# host-side weight packing
# ---------------------------------------------------------------------------

def _prep_weights(params):
    """Transform conv weights into lhsT tensors for the kernel.

    Wavelet inputs are computed unnormalized on device (plain subband sums,
    no 0.5 factors); the 2^-k scale of level-k wavelets is folded into the
    conv weights that consume them. Conv biases are dropped entirely:
    train-mode BN directly follows every conv, and BN(y + b) == BN(y).
    """
    P = {}
    f32 = lambda a: np.ascontiguousarray(np.asarray(a, dtype=np.float32))

    # ---- block 1 ----
    b = params["blk1"]
    w1 = f32(b["w1"]).copy()              # [64, 15, 3, 3]
    ws = f32(b["ws"])[:, :, 0, 0].copy()  # [64, 15]
    w1[:, 3:, :, :] *= 0.5                # wav1 channels carry 2x scale
    ws[:, 3:] *= 0.5
    w1_1 = np.zeros((3, 45, 128), np.float32)
    for dy in range(3):
        for dx in range(3):
            w1_1[dy, dx * 15:dx * 15 + 15, 0:64] = w1[:, :, dy, dx].T
    w1_1[1, 15:30, 64:128] = ws.T          # shortcut rides tap (dy=1, dx=1)
    P["w1_1"] = w1_1
    w2 = f32(b["w2"])                      # [64, 64, 3, 3]
    w2_1 = np.zeros((9, 64, 64), np.float32)
    for dy in range(3):
        for dx in range(3):
            w2_1[dy * 3 + dx] = w2[:, :, dy, dx].T
    P["w2_1"] = w2_1
    gb = np.zeros((192, 2), np.float32)
    gb[0:64, 0], gb[0:64, 1] = f32(b["g1"]), f32(b["be1"])
    gb[64:128, 0], gb[64:128, 1] = f32(b["gs"]), f32(b["bes"])
    gb[128:192, 0], gb[128:192, 1] = f32(b["g2"]), f32(b["be2"])
    P["gb_1"] = gb

    # ---- block 2 ----
    b = params["blk2"]
    w1 = f32(b["w1"]).copy()               # [128, 76, 3, 3]
    ws = f32(b["ws"])[:, :, 0, 0].copy()
    w1[:, 64:, :, :] *= 0.25               # wav2 carries 4x
    ws[:, 64:] *= 0.25
    w1_2 = np.zeros((10, 76, 128), np.float32)
    for dy in range(3):
        for dx in range(3):
            w1_2[dy * 3 + dx] = w1[:, :, dy, dx].T
    w1_2[9] = ws.T
    P["w1_2"] = w1_2
    w2 = f32(b["w2"])
    w2_2 = np.zeros((9, 128, 128), np.float32)
    for dy in range(3):
        for dx in range(3):
            w2_2[dy * 3 + dx] = w2[:, :, dy, dx].T
    P["w2_2"] = w2_2
    gb = np.zeros((384, 2), np.float32)
    gb[0:128, 0], gb[0:128, 1] = f32(b["g1"]), f32(b["be1"])
    gb[128:256, 0], gb[128:256, 1] = f32(b["gs"]), f32(b["bes"])
    gb[256:384, 0], gb[256:384, 1] = f32(b["g2"]), f32(b["be2"])
    P["gb_2"] = gb

    # ---- blocks 3..5 ----
    for k in (3, 4, 5):
        spec = BLOCKS[k]
        Cp, Cout = spec["Cp"], spec["Cout"]
        Mt, Ktp = Cout // 128, Cp // 128
        b = params[f"blk{k}"]
        w1 = f32(b["w1"]).copy()
        ws = f32(b["ws"])[:, :, 0, 0].copy()
        wavscale = 0.5 ** k
        w1[:, Cp:, :, :] *= wavscale
        ws[:, Cp:] *= wavscale
        w1p = np.zeros((Mt, Ktp, 9, 128, 128), np.float32)
        w1w = np.zeros((Mt, 108, 128), np.float32)
        wsp = np.zeros((Mt, Ktp, 128, 128), np.float32)
        wsw = np.zeros((Mt, 12, 128), np.float32)
        for m in range(Mt):
            wm = w1[m * 128:(m + 1) * 128]
            for kt in range(Ktp):
                for dy in range(3):
                    for dx in range(3):
                        w1p[m, kt, dy * 3 + dx] = wm[:, kt * 128:(kt + 1) * 128, dy, dx].T
                wsp[m, kt] = ws[m * 128:(m + 1) * 128, kt * 128:(kt + 1) * 128].T
            for ti, t in enumerate(T_ORDER):
                dy, dx = t // 3, t % 3
                w1w[m, ti * 12:(ti + 1) * 12, :] = wm[:, Cp:, dy, dx].T
            wsw[m] = ws[m * 128:(m + 1) * 128, Cp:].T
        P[f"w1_{k}"] = w1p
        P[f"w1w_{k}"] = w1w
        P[f"ws_{k}"] = wsp
        P[f"wsw_{k}"] = wsw
        w2 = f32(b["w2"])
        w2p = np.zeros((Mt, Mt, 9, 128, 128), np.float32)
        for m in range(Mt):
            for kt in range(Mt):
                for dy in range(3):
                    for dx in range(3):
                        w2p[m, kt, dy * 3 + dx] = \
                            w2[m * 128:(m + 1) * 128, kt * 128:(kt + 1) * 128, dy, dx].T
        P[f"w2_{k}"] = w2p
        gb = np.zeros((3 * Cout, 2), np.float32)
        gb[0:Cout, 0], gb[0:Cout, 1] = f32(b["g1"]), f32(b["be1"])
        gb[Cout:2 * Cout, 0], gb[Cout:2 * Cout, 1] = f32(b["gs"]), f32(b["bes"])
        gb[2 * Cout:, 0], gb[2 * Cout:, 1] = f32(b["g2"]), f32(b["be2"])
        P[f"gb_{k}"] = gb
    return P


# ---------------------------------------------------------------------------
# device-side helpers
# ---------------------------------------------------------------------------

class Seg:
    """Accumulates bn_stats chunks for one [P, npix] conv-output segment."""

    def __init__(self, pool, P, nchunks, tag):
        self.P = P
        self.buf = pool.tile([P, max(nchunks, 1), 6], F32, tag=tag)
        self.n = 0

    def add(self, nc, src_flat):
        nc.vector.bn_stats(out=self.buf[:, self.n, :], in_=src_flat)
        self.n += 1

    def finish(self, nc, pool, tag):
        mv = pool.tile([self.P, 2], F32, tag=tag + "_mv")
        nc.vector.bn_aggr(out=mv, in_=self.buf[:, :self.n, :])
        return mv


def _pack_stats(nc, pool, mv, P, tag):
    """[P,2] (mean,var) -> [P,2] (mean/8, (var+mean^2)/8)."""
    pk = pool.tile([P, 2], F32, tag=tag + "_pk")
    tmp = pool.tile([P, 1], F32, tag=tag + "_tmp")
    nc.vector.tensor_tensor(out=tmp, in0=mv[:, 0:1], in1=mv[:, 0:1], op=OP.mult)
    nc.vector.tensor_tensor(out=pk[:, 1:2], in0=mv[:, 1:2], in1=tmp, op=OP.add)
    nc.scalar.mul(pk[:, 1:2], pk[:, 1:2], 1.0 / N_CORES)
    nc.scalar.mul(pk[:, 0:1], mv[:, 0:1], 1.0 / N_CORES)
    return pk


def _unpack_stats(nc, pool, g, gamma_beta, P, tag):
    """g [P,2] = (mean, E[x^2]) -> ac [P,2] = (a, c):
    a = gamma * rsqrt(var + eps), c = beta - mean * a."""
    ac = pool.tile([P, 2], F32, tag=tag + "_ac")
    tmp = pool.tile([P, 1], F32, tag=tag + "_t1")
    var = pool.tile([P, 1], F32, tag=tag + "_t2")
    nc.vector.tensor_tensor(out=tmp, in0=g[:, 0:1], in1=g[:, 0:1], op=OP.mult)
    nc.vector.tensor_tensor(out=var, in0=g[:, 1:2], in1=tmp, op=OP.subtract)
    eps = pool.tile([P, 1], F32, tag=tag + "_eps")
    nc.vector.memset(eps, BN_EPS)
    nc.scalar.activation(out=var, in_=var, func=ACTF.Sqrt, bias=eps, scale=1.0)
    nc.vector.reciprocal(out=var, in_=var)
    nc.vector.tensor_tensor(out=ac[:, 0:1], in0=var, in1=gamma_beta[:, 0:1], op=OP.mult)
    nc.vector.tensor_tensor(out=tmp, in0=g[:, 0:1], in1=ac[:, 0:1], op=OP.mult)
    nc.vector.tensor_tensor(out=ac[:, 1:2], in0=gamma_beta[:, 1:2], in1=tmp, op=OP.subtract)
    return ac


def _allreduce(nc, pool, segs_pk, cc_pair, replica, tag):
    """Pack per-segment [P,2] tiles into cci, AllReduce, read back tiles of
    (global mean, global E[x^2])."""
    cci, cco = cc_pair
    off = 0
    for pk in segs_pk:
        Pp = pk.shape[0]
        nc.sync.dma_start(out=cci[0, off:off + 2 * Pp], in_=pk)
        off += 2 * Pp
    nc.gpsimd.collective_compute(
        "AllReduce", OP.add, ins=[cci[:, :]], outs=[cco[:, :]],
        replica_groups=[replica])
    outs = []
    off = 0
    for i, pk in enumerate(segs_pk):
        Pp = pk.shape[0]
        g = pool.tile([Pp, 2], F32, tag=f"{tag}_g{i}")
        nc.sync.dma_start(out=g, in_=cco[0, off:off + 2 * Pp])
        off += 2 * Pp
        outs.append(g)
    return outs


def _load_gb(nc, pool, dram, row0, P, tag):
    t = pool.tile([P, 2], F32, tag=tag)
    nc.sync.dma_start(out=t, in_=dram[row0:row0 + P, :])
    return t


def _pad_memset(nc, t):
    S2 = t.shape[1]
    tb = t.bitcast(F32) if t.dtype == F32R else t
    nc.vector.memset(tb[:, 0, :], 0.0)
    nc.vector.memset(tb[:, S2 - 1, :], 0.0)
    nc.vector.memset(tb[:, :, 0], 0.0)
    nc.vector.memset(tb[:, :, S2 - 1], 0.0)


# ---------------------------------------------------------------------------
# kernel body
# ---------------------------------------------------------------------------

def _build_nc(num_devices=N_CORES, replica=None):
    if replica is None:
        replica = list(range(num_devices))
    nc = bacc.Bacc("TRN2", target_bir_lowering=False, debug=False,
                   num_devices=num_devices)

    x = nc.dram_tensor("x", [3, 512, 512], F32, kind="ExternalInput")
    wt = {}
    wt["w1_1"] = nc.dram_tensor("w1_1", [3, 45, 128], F32, kind="ExternalInput")
    wt["w2_1"] = nc.dram_tensor("w2_1", [9, 64, 64], F32, kind="ExternalInput")
    wt["gb_1"] = nc.dram_tensor("gb_1", [192, 2], F32, kind="ExternalInput")
    wt["w1_2"] = nc.dram_tensor("w1_2", [10, 76, 128], F32, kind="ExternalInput")
    wt["w2_2"] = nc.dram_tensor("w2_2", [9, 128, 128], F32, kind="ExternalInput")
    wt["gb_2"] = nc.dram_tensor("gb_2", [384, 2], F32, kind="ExternalInput")
    for k in (3, 4, 5):
        Cp, Cout = BLOCKS[k]["Cp"], BLOCKS[k]["Cout"]
        Mt, Ktp = Cout // 128, Cp // 128
        wt[f"w1_{k}"] = nc.dram_tensor(f"w1_{k}", [Mt, Ktp, 9, 128, 128], F32, kind="ExternalInput")
        wt[f"w1w_{k}"] = nc.dram_tensor(f"w1w_{k}", [Mt, 108, 128], F32, kind="ExternalInput")
        wt[f"ws_{k}"] = nc.dram_tensor(f"ws_{k}", [Mt, Ktp, 128, 128], F32, kind="ExternalInput")
        wt[f"wsw_{k}"] = nc.dram_tensor(f"wsw_{k}", [Mt, 12, 128], F32, kind="ExternalInput")
        wt[f"w2_{k}"] = nc.dram_tensor(f"w2_{k}", [Mt, Mt, 9, 128, 128], F32, kind="ExternalInput")
        wt[f"gb_{k}"] = nc.dram_tensor(f"gb_{k}", [3 * Cout, 2], F32, kind="ExternalInput")

    eout = {k: nc.dram_tensor(f"e{k}", list(OUT_SHAPES[k]), F32, kind="ExternalOutput")
            for k in range(1, 6)}

    in1d = nc.dram_tensor("in1d", [45, 258, 256], F32)  # (dx,c), 1+256+1 rows, shifted cols
    ll_d = {1: nc.dram_tensor("ll1", [3, 256, 256], F32),
            2: nc.dram_tensor("ll2", [3, 128, 128], F32),
            3: nc.dram_tensor("ll3", [3, 64, 64], F32),
            4: nc.dram_tensor("ll4", [3, 32, 32], F32)}
    wav_d = {2: nc.dram_tensor("wav2", [12, 128, 128], F32),
             3: nc.dram_tensor("wav3", [12, 64, 64], F32),
             4: nc.dram_tensor("wav4", [12, 32, 32], F32),
             5: nc.dram_tensor("wav5", [12, 16, 16], F32)}
    yscD1 = nc.dram_tensor("yscD1", [64, 256, 256], BF16)
    y2D1 = nc.dram_tensor("y2D1", [64, 256, 256], BF16)
    yscD2 = nc.dram_tensor("yscD2", [128, 128, 128], BF16)
    cc = {}
    for tag, n in (("ar1_1", 256), ("ar2_1", 128), ("ar1_2", 512), ("ar2_2", 256),
                   ("ar1_3", 1024), ("ar2_3", 512), ("ar1_4", 2048), ("ar2_4", 1024),
                   ("ar1_5", 4096), ("ar2_5", 2048)):
        cc[tag] = (nc.dram_tensor(f"cci_{tag}", [1, n], F32),
                   nc.dram_tensor(f"cco_{tag}", [1, n], F32, addr_space="Shared"))

    with tile.TileContext(nc, pool_alloc_mode="queue") as tc:
        with ExitStack() as ctx:
            _emit(ctx, tc, nc, x, wt, eout, in1d, ll_d, wav_d,
                  yscD1, y2D1, yscD2, cc, replica)
    nc.compile()
    return nc


def _phase_wavelets(tc, nc, x, in1d, ll_d, wav_d, levels=(1, 2, 3, 4, 5)):
    with tc.tile_pool(name="wv", bufs=3) as wv:
        # zero pad rows (r=0, r=257) of the pre-shifted in1d45
        zr = wv.tile([128, 8192], F32, tag="zr", bufs=1)
        nc.vector.memset(zr, 0.0)
        nc.sync.dma_start(out=in1d[:, 0, :], in_=zr[0:45, 0:256])
        nc.sync.dma_start(out=in1d[:, 257, :], in_=zr[0:45, 0:256])
        # level 1: x [3,512,512] -> in1d channels (pool + wav1) + ll1
        for g in (range(6) if 1 in levels else ()):
            c, half = g // 2, g % 2
            y0 = 128 * half
            E = wv.tile([128, 512], F32, tag="E")
            Ot = wv.tile([128, 512], F32, tag="O")
            nc.sync.dma_start(out=E, in_=x[c, 2 * y0:2 * y0 + 256:2, :])
            nc.sync.dma_start(out=Ot, in_=x[c, 2 * y0 + 1:2 * y0 + 256:2, :])
            Ev = E.rearrange("p (a b) -> p a b", b=2)
            Ov = Ot.rearrange("p (a b) -> p a b", b=2)
            p_ = wv.tile([128, 256], F32, tag="p_")
            m_ = wv.tile([128, 256], F32, tag="m_")
            q_ = wv.tile([128, 256], F32, tag="q_")
            n_ = wv.tile([128, 256], F32, tag="n_")
            nc.vector.tensor_tensor(out=p_, in0=Ev[:, :, 0], in1=Ev[:, :, 1], op=OP.add)
            nc.vector.tensor_tensor(out=m_, in0=Ev[:, :, 0], in1=Ev[:, :, 1], op=OP.subtract)
            nc.vector.tensor_tensor(out=q_, in0=Ov[:, :, 0], in1=Ov[:, :, 1], op=OP.add)
            nc.vector.tensor_tensor(out=n_, in0=Ov[:, :, 0], in1=Ov[:, :, 1], op=OP.subtract)
            # subband/pool tiles carry zero cols at 0 and 257 so the three
            # dx-shifted DRAM writes are single full-width row DMAs
            sb = wv.tile([128, 4, 258], F32, tag="sb")
            nc.vector.memset(sb[:, :, 0], 0.0)
            nc.vector.memset(sb[:, :, 257], 0.0)
            nc.vector.tensor_tensor(out=sb[:, 0, 1:257], in0=p_, in1=q_, op=OP.add)
            nc.vector.tensor_tensor(out=sb[:, 1, 1:257], in0=p_, in1=q_, op=OP.subtract)
            nc.vector.tensor_tensor(out=sb[:, 2, 1:257], in0=m_, in1=n_, op=OP.add)
            nc.vector.tensor_tensor(out=sb[:, 3, 1:257], in0=m_, in1=n_, op=OP.subtract)
            pl = wv.tile([128, 258], F32, tag="pl")
            nc.vector.memset(pl[:, 0:1], 0.0)
            nc.vector.memset(pl[:, 257:258], 0.0)
            nc.vector.tensor_tensor(out=p_, in0=Ev[:, :, 0], in1=Ev[:, :, 1], op=OP.max)
            nc.vector.tensor_tensor(out=q_, in0=Ov[:, :, 0], in1=Ov[:, :, 1], op=OP.max)
            nc.vector.tensor_tensor(out=pl[:, 1:257], in0=p_, in1=q_, op=OP.max)
            # in1d45[(dx,c'), 1+r, x] = in1[c', r, x-1+dx]; src cols [dx, dx+256)
            for ch_, src in [(c, pl)] + [(3 + 4 * c + s, sb[:, s, :]) for s in range(4)]:
                for dx in range(3):
                    nc.sync.dma_start(
                        out=in1d[15 * dx + ch_, 1 + y0:1 + y0 + 128, :],
                        in_=src[:, dx:dx + 256])
            nc.sync.dma_start(out=ll_d[1][c, y0:y0 + 128, :], in_=sb[:, 0, 1:257])

        # levels 2..5 -> wav_d planes (+ ll chain)
        for lvl in (2, 3, 4, 5):
            if lvl not in levels:
                continue
            src = ll_d[lvl - 1]
            Sin = src.shape[1]
            Sout = Sin // 2
            for c in range(3):
                ng = (Sout + 127) // 128
                for gi in range(ng):
                    y0 = gi * 128
                    nrr = min(128, Sout - y0)
                    E = wv.tile([128, 512], F32, tag="E")
                    Ot = wv.tile([128, 512], F32, tag="O")
                    nc.sync.dma_start(out=E[:nrr, :Sin], in_=src[c, 2 * y0:2 * (y0 + nrr):2, :])
                    nc.sync.dma_start(out=Ot[:nrr, :Sin], in_=src[c, 2 * y0 + 1:2 * (y0 + nrr):2, :])
                    Ev = E[:nrr, :Sin].rearrange("p (a b) -> p a b", b=2)
                    Ov = Ot[:nrr, :Sin].rearrange("p (a b) -> p a b", b=2)
                    p_ = wv.tile([128, 256], F32, tag="p_")
                    m_ = wv.tile([128, 256], F32, tag="m_")
                    q_ = wv.tile([128, 256], F32, tag="q_")
                    n_ = wv.tile([128, 256], F32, tag="n_")
                    nc.vector.tensor_tensor(out=p_[:nrr, :Sout], in0=Ev[:, :, 0], in1=Ev[:, :, 1], op=OP.add)
                    nc.vector.tensor_tensor(out=m_[:nrr, :Sout], in0=Ev[:, :, 0], in1=Ev[:, :, 1], op=OP.subtract)
                    nc.vector.tensor_tensor(out=q_[:nrr, :Sout], in0=Ov[:, :, 0], in1=Ov[:, :, 1], op=OP.add)
                    nc.vector.tensor_tensor(out=n_[:nrr, :Sout], in0=Ov[:, :, 0], in1=Ov[:, :, 1], op=OP.subtract)
                    sb = wv.tile([128, 4, 256], F32, tag="sb")
                    nc.vector.tensor_tensor(out=sb[:nrr, 0, :Sout], in0=p_[:nrr, :Sout], in1=q_[:nrr, :Sout], op=OP.add)
                    nc.vector.tensor_tensor(out=sb[:nrr, 1, :Sout], in0=p_[:nrr, :Sout], in1=q_[:nrr, :Sout], op=OP.subtract)
                    nc.vector.tensor_tensor(out=sb[:nrr, 2, :Sout], in0=m_[:nrr, :Sout], in1=n_[:nrr, :Sout], op=OP.add)
                    nc.vector.tensor_tensor(out=sb[:nrr, 3, :Sout], in0=m_[:nrr, :Sout], in1=n_[:nrr, :Sout], op=OP.subtract)
                    for s in range(4):
                        nc.sync.dma_start(out=wav_d[lvl][4 * c + s, y0:y0 + nrr, :],
                                          in_=sb[:nrr, s, :Sout])
                    if lvl < 5:
                        nc.sync.dma_start(out=ll_d[lvl][c, y0:y0 + nrr, :],
                                          in_=sb[:nrr, 0, :Sout])


def _emit(ctx, tc, nc, x, wt, eout, in1d, ll_d, wav_d, yscD1, y2D1, yscD2,
          cc, replica):
    _NEXT_IN.clear()
    small = ctx.enter_context(tc.tile_pool(name="small", bufs=1))

    _phase_wavelets(tc, nc, x, in1d, ll_d, wav_d, levels=(1,))

    # =====================================================================
    # Block 1 (strip-tiled; y1 SBUF bf16; ysc & y2 staged in DRAM bf16)
    # =====================================================================

    with tc.tile_pool(name="b1w", bufs=1) as b1w:
        w1l = b1w.tile([45, 3, 128], F32R, tag="w1l")
        nc.sync.dma_start(out=w1l, in_=wt["w1_1"][:, :, :].rearrange("a b c -> b a c").bitcast(F32R))
        w2l = b1w.tile([64, 9, 64], F32R, tag="w2l")
        nc.sync.dma_start(out=w2l, in_=wt["w2_1"][:, :, :].rearrange("a b c -> b a c").bitcast(F32R))

        _phase_wavelets(tc, nc, x, in1d, ll_d, wav_d, levels=(2, 3, 4, 5))

        with tc.tile_pool(name="pY1", bufs=1) as pY1:
            Y1 = pY1.tile([128, 34816], BF16, tag="Y1")  # part 64h+c; free (r%128)*256+x

            # ---------------- pass A: conv1 + shortcut ----------------
            with tc.tile_pool(name="b1a", bufs=2) as b1a, \
                 tc.tile_pool(name="b1ps", bufs=4, space="PSUM") as b1ps, \
                 tc.tile_pool(name="b1st", bufs=2) as b1st:
                seg1 = Seg(small, 128, 128, "seg1")
                for s in range(8):
                    # T45[(dx,c), yy, x] = in1[c, 32s-1+yy, x-1+dx] (pre-shifted DRAM)
                    T45 = b1a.tile([45, 34, 256], F32R, tag="T45")
                    nc.sync.dma_start(out=T45[:, :, :],
                                      in_=in1d[:, 32 * s:32 * s + 34, :].bitcast(F32R))
                    yscS = b1st.tile([64, 32, 256], BF16, tag="yscS")
                    h = s // 4
                    for i in range(16):
                        yo = 2 * i
                        ps = b1ps.tile([128, 2, 256], F32, tag="ps1")
                        for dy in range(3):
                            nc.tensor.matmul(ps, r32(w1l[:, dy, :]),
                                             r32(T45[:, yo + dy:yo + dy + 2, :]),
                                             start=(dy == 0), stop=(dy == 2))
                        psf = ps.rearrange("p a b -> p (a b)")
                        seg1.add(nc, psf)
                        rr = (32 * s + yo) % 128
                        nc.scalar.copy(Y1[64 * h:64 * h + 64, rr * 256:(rr + 2) * 256],
                                       psf[0:64, :])
                        nc.scalar.copy(yscS[:, yo:yo + 2, :], ps[64:128, :, :])
                    nc.sync.dma_start(out=yscD1[:, 32 * s:32 * s + 32, :], in_=yscS)
                mv1 = seg1.finish(nc, small, "seg1")
                pk1 = _pack_stats(nc, small, mv1, 128, "pk1")
            g1 = _allreduce(nc, small, [pk1], cc["ar1_1"], replica, "ar11")[0]
            gb1a = _load_gb(nc, small, wt["gb_1"], 0, 128, "gb1a")
            ac1 = _unpack_stats(nc, small, g1, gb1a, 128, "ac1")
            acs_d = small.tile([128, 2], F32, tag="acs_d")
            nc.scalar.copy(acs_d[0:64, :], ac1[64:128, :])
            nc.scalar.copy(acs_d[64:128, :], ac1[64:128, :])

            # ---------------- pass B: t1 = relu(bn(y1)); conv2 ----------------
            with tc.tile_pool(name="b1b", bufs=2) as b1b, \
                 tc.tile_pool(name="b1ps2", bufs=4, space="PSUM") as b1ps2, \
                 tc.tile_pool(name="b1st2", bufs=2) as b1st2:
                seg2 = Seg(small, 64, 128, "seg2")
                for s in range(8):
                    t1 = b1b.tile([64, 34, 258], F32R, tag="t1")
                    nc.vector.memset(t1.bitcast(F32)[:, :, 0], 0.0)
                    nc.vector.memset(t1.bitcast(F32)[:, :, 257], 0.0)
                    r0, r1 = 32 * s - 1, 32 * s + 33
                    if r0 < 0:
                        nc.vector.memset(t1.bitcast(F32)[:, 0, :], 0.0)
                        r0 = 0
                    if r1 > 256:
                        nc.vector.memset(t1.bitcast(F32)[:, 33, :], 0.0)
                        r1 = 256
                    spans = []
                    if r0 < 128:
                        spans.append((0, r0, min(r1, 128)))
                    if r1 > 128:
                        spans.append((1, max(r0, 128), r1))
                    for h, a, bnd in spans:
                        Yv = Y1[64 * h:64 * h + 64,
                                (a % 128) * 256:((a % 128) + (bnd - a)) * 256]
                        yy = a - (32 * s - 1)
                        nc.scalar.activation(
                            out=t1[:, yy:yy + (bnd - a), 1:257],
                            in_=Yv.rearrange("p (r c) -> p r c", c=256),
                            func=ACTF.Relu,
                            bias=ac1[0:64, 1:2], scale=ac1[0:64, 0:1])
                    y2S = b1st2.tile([64, 32, 256], BF16, tag="y2S")
                    for i in range(16):
                        yo = 2 * i
                        ps = b1ps2.tile([64, 2, 256], F32, tag="ps2")
                        first = True
                        for dy in range(3):
                            for dx in range(3):
                                nc.tensor.matmul(
                                    ps, r32(w2l[:, dy * 3 + dx, :]),
                                    r32(t1[:, yo + dy:yo + dy + 2, dx:dx + 256]),
                                    start=first, stop=(dy == 2 and dx == 2))
                                first = False
                        psf = ps.rearrange("p a b -> p (a b)")
                        seg2.add(nc, psf)
                        nc.scalar.copy(y2S[:, yo:yo + 2, :], ps)
                    nc.sync.dma_start(out=y2D1[:, 32 * s:32 * s + 32, :], in_=y2S)
                mv2 = seg2.finish(nc, small, "seg2")
                pk2 = _pack_stats(nc, small, mv2, 64, "pk2")
        # Y1 pool closed here
        g2 = _allreduce(nc, small, [pk2], cc["ar2_1"], replica, "ar21")[0]
        gb1b = _load_gb(nc, small, wt["gb_1"], 128, 64, "gb1b")
        ac2 = _unpack_stats(nc, small, g2, gb1b, 64, "ac2")
        ac2_d = small.tile([128, 2], F32, tag="ac2_d")
        nc.scalar.copy(ac2_d[0:64, :], ac2)
        nc.scalar.copy(ac2_d[64:128, :], ac2)
        ccs1 = small.tile([128, 1], F32, tag="ccs1")
        nc.vector.tensor_tensor(out=ccs1, in0=acs_d[:, 1:2], in1=ac2_d[:, 1:2], op=OP.add)

    # in2sb spans blk1 pass C .. blk2 pass A
    p_in2 = tc.tile_pool(name="p_in2", bufs=1, side="right")
    in2p = p_in2.__enter__()
    in2sb = in2p.tile([76, 130, 130], F32R, tag="in2sb")
    _pad_memset(nc, in2sb)
    # wav2 channels
    nc.sync.dma_start(out=in2sb[64:76, 1:129, 1:129], in_=wav_d[2][:, :, :].bitcast(F32R))

    # ---------------- blk1 pass C ----------------
    with tc.tile_pool(name="b1c", bufs=2) as b1c:
        for j in range(16):
            # chunk covers rows [8j,8j+8) of each half
            y2c = b1c.tile([128, 8, 256], BF16, tag="y2c")
            ysc = b1c.tile([128, 8, 256], BF16, tag="ysc")
            for h in range(2):
                rb = 128 * h + 8 * j
                nc.sync.dma_start(out=y2c[64 * h:64 * h + 64, :, :],
                                  in_=y2D1[:, rb:rb + 8, :])
                nc.sync.dma_start(out=ysc[64 * h:64 * h + 64, :, :],
                                  in_=yscD1[:, rb:rb + 8, :])
            tmp = b1c.tile([128, 8, 256], F32, tag="tmpc")
            nc.scalar.activation(out=tmp, in_=ysc, func=ACTF.Identity,
                                 bias=ccs1[:, 0:1], scale=acs_d[:, 0:1])
            nc.vector.scalar_tensor_tensor(
                out=tmp, in0=y2c, scalar=ac2_d[:, 0:1], in1=tmp,
                op0=OP.mult, op1=OP.add)
            nc.scalar.activation(out=tmp, in_=tmp, func=ACTF.Relu)
            for h in range(2):
                rb = 128 * h + 8 * j
                nc.sync.dma_start(out=eout[1][:, rb:rb + 8, :],
                                  in_=tmp[64 * h:64 * h + 64, :, :])
            m1 = b1c.tile([128, 8, 128], F32, tag="m1")
            tv = tmp.rearrange("p r (c d) -> p r c d", d=2)
            nc.vector.tensor_tensor(out=m1, in0=tv[:, :, :, 0], in1=tv[:, :, :, 1], op=OP.max)
            m2 = b1c.tile([128, 4, 128], F32, tag="m2")
            m1v = m1.rearrange("p (r d) c -> p r d c", d=2)
            nc.vector.tensor_tensor(out=m2, in0=m1v[:, :, 0, :], in1=m1v[:, :, 1, :], op=OP.max)
            for h in range(2):
                rb = 64 * h + 4 * j
                nc.sync.dma_start(out=in2sb[0:64, 1 + rb:1 + rb + 4, 1:129],
                                  in_=m2[64 * h:64 * h + 64, :, :].bitcast(F32R))

    # =====================================================================
    # Block 2
    # =====================================================================
    _emit_block2(tc, nc, wt, eout, in2sb, p_in2, yscD2, wav_d, cc, replica, small)

    # =====================================================================
    # Blocks 3..5
    # =====================================================================
    # in3a was created by _emit_block2 pass C (returned via small registry)
    _emit_blockk(tc, nc, 3, wt, eout, cc, replica, small, wav_d,
                 stream_w1=False, stream_w2=False)
    _emit_blockk(tc, nc, 4, wt, eout, cc, replica, small, wav_d,
                 stream_w1=True, stream_w2=True)
    _emit_blockk(tc, nc, 5, wt, eout, cc, replica, small, wav_d,
                 stream_w1=True, stream_w2=True)


_NEXT_IN = {}  # k -> list of [128, Spad, Spad] pool-input tiles (built by k-1)


def _open_next_in(tc, nc, k):
    """Create block k's pool-channel input tiles (padded, borders zeroed)."""
    spec = BLOCKS[k]
    S, Ktp = spec["S"], spec["Cp"] // 128 if k >= 3 else 1
    Spad = S + 2
    pool_cm = tc.tile_pool(name=f"p_in{k}", bufs=1, side="right")
    p = pool_cm.__enter__()
    nt = max(1, spec["Cp"] // 128)
    npart = 128 if spec["Cp"] >= 128 else spec["Cp"]
    tiles = []
    for i in range(nt):
        t = p.tile([npart, Spad, Spad], F32R, tag=f"in{k}_{i}", name=f"in{k}_{i}")
        tiles.append(t)
    for t in tiles:
        _pad_memset(nc, t)
    _NEXT_IN[k] = (tiles, pool_cm, p)
    return tiles


def _emit_block2(tc, nc, wt, eout, in2sb, p_in2_cm, yscD2, wav_d, cc, replica, small):
    S, Spad, nr = 128, 130, 4
    ngr = S // nr

    with tc.tile_pool(name="b2y", bufs=1) as b2y:
        y1b = b2y.tile([128, S * S], F32, tag="y1b2")
        with tc.tile_pool(name="b2w", bufs=1) as b2w:
            w1l = b2w.tile([76, 10, 128], F32R, tag="w1l2")
            nc.sync.dma_start(out=w1l, in_=wt["w1_2"][:, :, :].rearrange("a b c -> b a c").bitcast(F32R))
            w2l = b2w.tile([128, 9, 128], F32R, tag="w2l2")
            nc.sync.dma_start(out=w2l, in_=wt["w2_2"][:, :, :].rearrange("a b c -> b a c").bitcast(F32R))

            # ---- pass A: conv1 + sc ----
            with tc.tile_pool(name="b2ps", bufs=4, space="PSUM") as psp, \
                 tc.tile_pool(name="b2st", bufs=2) as stp:
                seg1 = Seg(small, 128, ngr, "b2seg1")
                segs = Seg(small, 128, ngr, "b2segs")
                for g in range(ngr):
                    y0 = g * nr
                    ps = psp.tile([128, nr, S], F32, tag="psA")
                    first = True
                    for dy in range(3):
                        for dx in range(3):
                            nc.tensor.matmul(
                                ps, r32(w1l[:, dy * 3 + dx, :]),
                                r32(in2sb[:, y0 + dy:y0 + dy + nr, dx:dx + S]),
                                start=first, stop=(dy == 2 and dx == 2))
                            first = False
                    pss = psp.tile([128, nr, S], F32, tag="psS")
                    nc.tensor.matmul(pss, r32(w1l[:, 9, :]),
                                     r32(in2sb[:, 1 + y0:1 + y0 + nr, 1:1 + S]),
                                     start=True, stop=True)
                    seg1.add(nc, ps.rearrange("p a b -> p (a b)"))
                    segs.add(nc, pss.rearrange("p a b -> p (a b)"))
                    nc.scalar.copy(y1b[:, y0 * S:(y0 + nr) * S],
                                   ps.rearrange("p a b -> p (a b)"))
                    yscS = stp.tile([128, nr, S], BF16, tag="yscS2")
                    nc.scalar.copy(yscS, pss)
                    nc.sync.dma_start(out=yscD2[:, y0:y0 + nr, :], in_=yscS)
                mv1 = seg1.finish(nc, small, "b2seg1")
                mvs = segs.finish(nc, small, "b2segs")
                pk1 = _pack_stats(nc, small, mv1, 128, "b2pk1")
                pks = _pack_stats(nc, small, mvs, 128, "b2pks")
            # in2sb dead from here
            p_in2_cm.__exit__(None, None, None)
            gars = _allreduce(nc, small, [pk1, pks], cc["ar1_2"], replica, "b2ar1")
            ac1 = _unpack_stats(nc, small, gars[0], _load_gb(nc, small, wt["gb_2"], 0, 128, "gb2_1"), 128, "b2ac1")
            acs = _unpack_stats(nc, small, gars[1], _load_gb(nc, small, wt["gb_2"], 128, 128, "gb2_s"), 128, "b2acs")

            # ---- pass B: t1 strips + conv2 ----
            with tc.tile_pool(name="b2y2", bufs=1) as b2y2:
                y2b = b2y2.tile([128, S * S], BF16, tag="y2b2")
                with tc.tile_pool(name="b2b", bufs=3) as b2b, \
                     tc.tile_pool(name="b2ps2", bufs=4, space="PSUM") as psp2:
                    seg2 = Seg(small, 128, ngr, "b2seg2")
                    y1v = y1b.rearrange("p (r c) -> p r c", c=S)
                    for g in range(ngr):
                        y0 = g * nr
                        t1 = b2b.tile([128, nr + 2, Spad], F32R, tag="t1s2")
                        nc.vector.memset(t1.bitcast(F32)[:, :, 0], 0.0)
                        nc.vector.memset(t1.bitcast(F32)[:, :, Spad - 1], 0.0)
                        r0, r1 = y0 - 1, y0 + nr + 1
                        if r0 < 0:
                            nc.vector.memset(t1.bitcast(F32)[:, 0, :], 0.0)
                            r0 = 0
                        if r1 > S:
                            nc.vector.memset(t1.bitcast(F32)[:, nr + 1, :], 0.0)
                            r1 = S
                        nc.scalar.activation(
                            out=t1[:, r0 - (y0 - 1):r1 - (y0 - 1), 1:1 + S],
                            in_=y1v[:, r0:r1, :], func=ACTF.Relu,
                            bias=ac1[:, 1:2], scale=ac1[:, 0:1])
                        ps = psp2.tile([128, nr, S], F32, tag="psB")
                        first = True
                        for dy in range(3):
                            for dx in range(3):
                                nc.tensor.matmul(
                                    ps, r32(w2l[:, dy * 3 + dx, :]),
                                    r32(t1[:, dy:dy + nr, dx:dx + S]),
                                    start=first, stop=(dy == 2 and dx == 2))
                                first = False
                        seg2.add(nc, ps.rearrange("p a b -> p (a b)"))
                        nc.scalar.copy(
                            y2b.rearrange("p (r c) -> p r c", c=S)[:, y0:y0 + nr, :], ps)
                    mv2 = seg2.finish(nc, small, "b2seg2")
                    pk2 = _pack_stats(nc, small, mv2, 128, "b2pk2")
                # y1b dead
                g2 = _allreduce(nc, small, [pk2], cc["ar2_2"], replica, "b2ar2")[0]
                ac2 = _unpack_stats(nc, small, g2, _load_gb(nc, small, wt["gb_2"], 256, 128, "gb2_2"), 128, "b2ac2")
                ccs = small.tile([128, 1], F32, tag="b2ccs")
                nc.vector.tensor_tensor(out=ccs, in0=acs[:, 1:2], in1=ac2[:, 1:2], op=OP.add)

                # ---- pass C ----
                in3 = _open_next_in(tc, nc, 3)
                with tc.tile_pool(name="b2c", bufs=1) as bc:
                    for j in range(16):
                        rb = 8 * j
                        ysc = bc.tile([128, 8, S], BF16, tag="yscC2")
                        nc.sync.dma_start(out=ysc, in_=yscD2[:, rb:rb + 8, :])
                        tmp = bc.tile([128, 8, S], F32, tag="tmpC2")
                        nc.scalar.activation(out=tmp, in_=ysc, func=ACTF.Identity,
                                             bias=ccs[:, 0:1], scale=acs[:, 0:1])
                        y2v = y2b.rearrange("p (r c) -> p r c", c=S)[:, rb:rb + 8, :]
                        nc.vector.scalar_tensor_tensor(
                            out=tmp, in0=y2v, scalar=ac2[:, 0:1], in1=tmp,
                            op0=OP.mult, op1=OP.add)
                        nc.scalar.activation(out=tmp, in_=tmp, func=ACTF.Relu)
                        nc.sync.dma_start(out=eout[2][:, rb:rb + 8, :], in_=tmp)
                        m1 = bc.tile([128, 8, 64], F32, tag="m1C2")
                        tv = tmp.rearrange("p r (c d) -> p r c d", d=2)
                        nc.vector.tensor_tensor(out=m1, in0=tv[:, :, :, 0],
                                                in1=tv[:, :, :, 1], op=OP.max)
                        m2 = bc.tile([128, 4, 64], F32, tag="m2C2")
                        m1v = m1.rearrange("p (r d) c -> p r d c", d=2)
                        nc.vector.tensor_tensor(out=m2, in0=m1v[:, :, 0, :],
                                                in1=m1v[:, :, 1, :], op=OP.max)
                        nc.sync.dma_start(out=in3[0][:, 1 + 4 * j:1 + 4 * j + 4, 1:65],
                                          in_=m2.bitcast(F32R))


def _emit_blockk(tc, nc, k, wt, eout, cc, replica, small, wav_d,
                 stream_w1=False, stream_w2=False):
    spec = BLOCKS[k]
    Cp, Cout, S, nr = spec["Cp"], spec["Cout"], spec["S"], spec["nr"]
    Spad = S + 2
    Mt, Ktp = Cout // 128, Cp // 128
    ngr = S // nr
    inP, inP_cm, inP_pool = _NEXT_IN[k]

    with tc.tile_pool(name=f"bk{k}w", bufs=1) as bkw:
        bkT_cm = tc.tile_pool(name=f"bk{k}T", bufs=1, side="right")
        bkT = bkT_cm.__enter__()
        # wavelet im2col from DRAM (padded implicitly via shifts + zero pad)
        T108 = bkT.tile([108, S, S], F32R, tag=f"T108_{k}")
        wpad = bkT.tile([12, Spad, Spad], F32R, tag=f"wpad{k}")
        _pad_memset(nc, wpad)
        nc.sync.dma_start(out=wpad[:, 1:1 + S, 1:1 + S], in_=wav_d[k][:, :, :].bitcast(F32R))
        for ti, t in enumerate(T_ORDER):
            dy, dx = t // 3, t % 3
            nc.sync.dma_start(out=T108[12 * ti:12 * ti + 12, :, :],
                              in_=wpad[:, dy:dy + S, dx:dx + S])
        w1wl = bkw.tile([108, Mt, 128], F32R, tag=f"w1wl{k}")
        nc.sync.dma_start(out=w1wl, in_=wt[f"w1w_{k}"][:, :, :].rearrange("a b c -> b a c").bitcast(F32R))
        wswl = bkw.tile([12, Mt, 128], F32R, tag=f"wswl{k}")
        nc.sync.dma_start(out=wswl, in_=wt[f"wsw_{k}"][:, :, :].rearrange("a b c -> b a c").bitcast(F32R))
        wspl = bkw.tile([128, Mt, Ktp, 128], F32R, tag=f"wspl{k}")
        nc.sync.dma_start(out=wspl, in_=wt[f"ws_{k}"][:, :, :, :].rearrange("a b c d -> c a b d").bitcast(F32R))
        if not stream_w1:
            w1pl = bkw.tile([128, Mt, Ktp, 9, 128], F32R, tag=f"w1pl{k}")
            nc.sync.dma_start(out=w1pl,
                              in_=wt[f"w1_{k}"][:, :, :, :, :].rearrange("a b c d e -> d a b c e").bitcast(F32R))

        with tc.tile_pool(name=f"bk{k}ys", bufs=1) as bkys:
            y1b = [inP_pool.tile([128, S * S], F32, tag=f"y1b{k}_{m}", name=f"y1b{k}_{m}") for m in range(Mt)]
            yscb = [bkys.tile([128, S * S], F32, tag=f"yscb{k}_{m}", name=f"yscb{k}_{m}") for m in range(Mt)]

            # ---- pass A ----
            with tc.tile_pool(name=f"b{k}ps", bufs=4, space="PSUM") as psp, \
                 tc.tile_pool(name=f"b{k}wst", bufs=2) as wstr:
                seg1 = [Seg(small, 128, ngr, f"b{k}seg1_{m}") for m in range(Mt)]
                segs = [Seg(small, 128, ngr, f"b{k}segs_{m}") for m in range(Mt)]
                for m in range(Mt):
                    w1m = {}
                    if stream_w1:
                        for kt in range(Ktp):
                            w1kt = wstr.tile([128, 9, 128], F32R, tag=f"w1m{kt % 2}",
                                             name=f"w1m_{m}_{kt}")
                            nc.sync.dma_start(out=w1kt,
                                              in_=wt[f"w1_{k}"][m, kt].rearrange("d e f -> e d f").bitcast(F32R))
                            w1m[kt] = w1kt
                    for g in range(ngr):
                        y0 = g * nr
                        ps = psp.tile([128, nr, S], F32, tag="psA")
                        first = True
                        for kt in range(Ktp):
                            for dy in range(3):
                                for dx in range(3):
                                    lw = (w1m[kt][:, dy * 3 + dx, :] if stream_w1
                                          else w1pl[:, m, kt, dy * 3 + dx, :])
                                    nc.tensor.matmul(
                                        ps, r32(lw),
                                        r32(inP[kt][:, y0 + dy:y0 + dy + nr, dx:dx + S]),
                                        start=first, stop=False)
                                    first = False
                        nc.tensor.matmul(ps, r32(w1wl[:, m, :]),
                                         r32(T108[:, y0:y0 + nr, :]),
                                         start=False, stop=True)
                        pss = psp.tile([128, nr, S], F32, tag="psS")
                        for kt in range(Ktp):
                            nc.tensor.matmul(pss, r32(wspl[:, m, kt, :]),
                                             r32(inP[kt][:, 1 + y0:1 + y0 + nr, 1:1 + S]),
                                             start=(kt == 0), stop=False)
                        nc.tensor.matmul(pss, r32(wswl[:, m, :]),
                                         r32(T108[0:12, y0:y0 + nr, :]),
                                         start=False, stop=True)
                        seg1[m].add(nc, ps.rearrange("p a b -> p (a b)"))
                        segs[m].add(nc, pss.rearrange("p a b -> p (a b)"))
                        nc.scalar.copy(y1b[m][:, y0 * S:(y0 + nr) * S],
                                       ps.rearrange("p a b -> p (a b)"))
                        nc.scalar.copy(yscb[m][:, y0 * S:(y0 + nr) * S],
                                       pss.rearrange("p a b -> p (a b)"))
                pks = [_pack_stats(nc, small, seg1[m].finish(nc, small, f"b{k}seg1_{m}"),
                                   128, f"b{k}p1{m}") for m in range(Mt)] + \
                      [_pack_stats(nc, small, segs[m].finish(nc, small, f"b{k}segs_{m}"),
                                   128, f"b{k}ps{m}") for m in range(Mt)]
            bkT_cm.__exit__(None, None, None)  # T108/wpad dead after pass A
            gl = _allreduce(nc, small, pks, cc[f"ar1_{k}"], replica, f"b{k}ar1")
            ac1 = [_unpack_stats(nc, small, gl[m],
                                 _load_gb(nc, small, wt[f"gb_{k}"], m * 128, 128, f"gbl{k}1{m}"),
                                 128, f"b{k}ac1{m}") for m in range(Mt)]
            acs = [_unpack_stats(nc, small, gl[Mt + m],
                                 _load_gb(nc, small, wt[f"gb_{k}"], Cout + m * 128, 128, f"gbl{k}s{m}"),
                                 128, f"b{k}acs{m}") for m in range(Mt)]

            # ---- pass B ----
            with tc.tile_pool(name=f"bk{k}y2", bufs=1) as bky2:
                y2b = [bky2.tile([128, S * S], F32, tag=f"y2b{k}_{m}", name=f"y2b{k}_{m}") for m in range(Mt)]
                with tc.tile_pool(name=f"b{k}t1s", bufs=2) as bt1s, \
                     tc.tile_pool(name=f"b{k}ps2", bufs=4, space="PSUM") as psp2, \
                     tc.tile_pool(name=f"b{k}wst2", bufs=2) as wstr2:
                    seg2 = [Seg(small, 128, ngr, f"b{k}seg2_{m}") for m in range(Mt)]
                    if not stream_w2:
                        w2lf = bkw.tile([128, Mt, Mt, 9, 128], F32R, tag=f"w2l{k}")
                        nc.sync.dma_start(out=w2lf,
                                          in_=wt[f"w2_{k}"][:, :, :, :, :].rearrange("a b c d e -> d a b c e").bitcast(F32R))
                    for m in range(Mt):
                        w2m = {}
                        if stream_w2:
                            for kt in range(Mt):
                                w2kt = wstr2.tile([128, 9, 128], F32R, tag=f"w2m{kt % 2}",
                                                  name=f"w2m_{m}_{kt}")
                                nc.sync.dma_start(out=w2kt,
                                                  in_=wt[f"w2_{k}"][m, kt].rearrange("d e f -> e d f").bitcast(F32R))
                                w2m[kt] = w2kt
                        for g in range(ngr):
                            y0 = g * nr
                            t1s = []
                            for kt in range(Mt):
                                t1k = bt1s.tile([128, nr + 2, Spad], F32R,
                                                tag=f"t1s{kt}", name=f"t1s{kt}")
                                nc.vector.memset(t1k.bitcast(F32)[:, :, 0], 0.0)
                                nc.vector.memset(t1k.bitcast(F32)[:, :, Spad - 1], 0.0)
                                r0, r1 = y0 - 1, y0 + nr + 1
                                if r0 < 0:
                                    nc.vector.memset(t1k.bitcast(F32)[:, 0, :], 0.0)
                                    r0 = 0
                                if r1 > S:
                                    nc.vector.memset(t1k.bitcast(F32)[:, nr + 1, :], 0.0)
                                    r1 = S
                                nc.scalar.activation(
                                    out=t1k[:, r0 - (y0 - 1):r1 - (y0 - 1), 1:1 + S],
                                    in_=y1b[kt].rearrange("p (r c) -> p r c", c=S)[:, r0:r1, :],
                                    func=ACTF.Relu, bias=ac1[kt][:, 1:2], scale=ac1[kt][:, 0:1])
                                t1s.append(t1k)
                            ps = psp2.tile([128, nr, S], F32, tag="psB")
                            first = True
                            for kt in range(Mt):
                                for dy in range(3):
                                    for dx in range(3):
                                        lw = (w2m[kt][:, dy * 3 + dx, :] if stream_w2
                                              else w2lf[:, m, kt, dy * 3 + dx, :])
                                        nc.tensor.matmul(
                                            ps, r32(lw),
                                            r32(t1s[kt][:, dy:dy + nr, dx:dx + S]),
                                            start=first,
                                            stop=(kt == Mt - 1 and dy == 2 and dx == 2))
                                        first = False
                            seg2[m].add(nc, ps.rearrange("p a b -> p (a b)"))
                            nc.scalar.copy(y2b[m][:, y0 * S:(y0 + nr) * S],
                                           ps.rearrange("p a b -> p (a b)"))
                    pk2 = [_pack_stats(nc, small, seg2[m].finish(nc, small, f"b{k}seg2_{m}"),
                                       128, f"b{k}p2{m}") for m in range(Mt)]
                inP_cm.__exit__(None, None, None)  # in-tiles + y1b dead
                gl2 = _allreduce(nc, small, pk2, cc[f"ar2_{k}"], replica, f"b{k}ar2")
                ac2 = [_unpack_stats(nc, small, gl2[m],
                                     _load_gb(nc, small, wt[f"gb_{k}"], 2 * Cout + m * 128, 128, f"gbl{k}2{m}"),
                                     128, f"b{k}ac2{m}") for m in range(Mt)]

                # ---- pass C ----
                outP = _open_next_in(tc, nc, k + 1) if k < 5 else None
                ncch = S // 16 if S >= 32 else 1   # row chunks
                rch = S // ncch
                with tc.tile_pool(name=f"b{k}c", bufs=2) as bc:
                    ccs_l = []
                    for m in range(Mt):
                        ccs = small.tile([128, 1], F32, tag=f"b{k}ccs{m}", name=f"b{k}ccs{m}")
                        nc.vector.tensor_tensor(out=ccs, in0=acs[m][:, 1:2],
                                                in1=ac2[m][:, 1:2], op=OP.add)
                        ccs_l.append(ccs)
                    for ch in range(ncch):
                        for m in range(Mt):
                            ccs = ccs_l[m]
                            rb = ch * rch
                            tmp = bc.tile([128, rch, S], F32, tag="tmpC")
                            nc.scalar.activation(
                                out=tmp,
                                in_=yscb[m].rearrange("p (r c) -> p r c", c=S)[:, rb:rb + rch, :],
                                func=ACTF.Identity, bias=ccs[:, 0:1], scale=acs[m][:, 0:1])
                            nc.vector.scalar_tensor_tensor(
                                out=tmp,
                                in0=y2b[m].rearrange("p (r c) -> p r c", c=S)[:, rb:rb + rch, :],
                                scalar=ac2[m][:, 0:1], in1=tmp, op0=OP.mult, op1=OP.add)
                            nc.scalar.activation(out=tmp, in_=tmp, func=ACTF.Relu)
                            nc.sync.dma_start(out=eout[k][m * 128:(m + 1) * 128, rb:rb + rch, :], in_=tmp)
                            if k < 5:
                                m1 = bc.tile([128, rch, S // 2], F32, tag="m1C")
                                tv = tmp.rearrange("p r (c d) -> p r c d", d=2)
                                nc.vector.tensor_tensor(out=m1, in0=tv[:, :, :, 0],
                                                        in1=tv[:, :, :, 1], op=OP.max)
                                m2 = bc.tile([128, rch // 2, S // 2], F32, tag="m2C")
                                m1v = m1.rearrange("p (r d) c -> p r d c", d=2)
                                nc.vector.tensor_tensor(out=m2, in0=m1v[:, :, 0, :],
                                                        in1=m1v[:, :, 1, :], op=OP.max)
                                nc.sync.dma_start(
                                    out=outP[m][:, 1 + rb // 2:1 + rb // 2 + rch // 2, 1:1 + S // 2],
                                    in_=m2.bitcast(F32R))



# ---------------------------------------------------------------------------
# entry point
# ---------------------------------------------------------------------------

_NC_CACHE = {}


def _get_nc():
    if "nc" not in _NC_CACHE:
        _NC_CACHE["nc"] = _build_nc()
    return _NC_CACHE["nc"]


def kernel(x_img, params):
    x_img = np.asarray(x_img, dtype=np.float32)
    P = _prep_weights(params)
    nc = _get_nc()
    in_maps = []
    for i in range(N_CORES):
        m = {"x": np.ascontiguousarray(x_img[i])}
        m.update(P)
        in_maps.append(m)
    res = run_bass_kernel_spmd(nc, in_maps, core_ids=list(range(N_CORES)))
    outs = []
    for k in range(1, 6):
        ek = np.stack([res.results[i][f"e{k}"] for i in range(N_CORES)], axis=0)
        outs.append(ek)
    return (x_img, *outs)


# revision 35
# speedup vs baseline: 1.0520x; 1.0340x over previous
"""Trainium2 Bass kernel for nn_EncoderWav (wavelet CNN encoder).

Strategy: pure data parallelism — 8 images, one per NeuronCore. Sync-BN
batch statistics are combined with tiny AllReduce collectives (2 per
residual block). Convolutions run as fp32r matmuls on the tensor engine
with taps accumulated in PSUM; the first block folds (dx, cin) into the
contraction dim to use the 128-wide PE array despite cin=15.
"""

import sys
from contextlib import ExitStack

sys.path.insert(0, "/opt/trn_rl_repo")

import numpy as np  # noqa: E402

import concourse.bass as bass  # noqa: E402
import concourse.bacc as bacc  # noqa: E402
import concourse.tile as tile  # noqa: E402
import concourse.mybir as mybir  # noqa: E402
from concourse.bass_utils import run_bass_kernel_spmd  # noqa: E402

F32 = mybir.dt.float32
F32R = mybir.dt.float32r
BF16 = mybir.dt.bfloat16
OP = mybir.AluOpType
ACTF = mybir.ActivationFunctionType
BN_EPS = 1e-5
N_CORES = 8

# block specs (k>=2): (Cp pool chans, Cout, S spatial, rows-per-matmul)
BLOCKS = {
    2: dict(Cp=64, Cout=128, S=128, nr=4),
    3: dict(Cp=128, Cout=256, S=64, nr=8),
    4: dict(Cp=256, Cout=512, S=32, nr=16),
    5: dict(Cp=512, Cout=1024, S=16, nr=16),
}
T_ORDER = [4, 0, 1, 2, 3, 5, 6, 7, 8]  # tap (1,1) first so sc rhs has base partition 0
OUT_SHAPES = {
    1: (64, 256, 256), 2: (128, 128, 128), 3: (256, 64, 64),
    4: (512, 32, 32), 5: (1024, 16, 16),
}


def r32(ap):
    return ap.bitcast(F32R)


# ---------------------------------------------------------------------------
# host-side weight packing
# ---------------------------------------------------------------------------

def _prep_weights(params):
    """Transform conv weights into lhsT tensors for the kernel.

    Wavelet inputs are computed unnormalized on device (plain subband sums,
    no 0.5 factors); the 2^-k scale of level-k wavelets is folded into the
    conv weights that consume them. Conv biases are dropped entirely:
    train-mode BN directly follows every conv, and BN(y + b) == BN(y).
    """
    P = {}
    f32 = lambda a: np.ascontiguousarray(np.asarray(a, dtype=np.float32))

    # ---- block 1 ----
    b = params["blk1"]
    w1 = f32(b["w1"]).copy()              # [64, 15, 3, 3]
    ws = f32(b["ws"])[:, :, 0, 0].copy()  # [64, 15]
    w1[:, 3:, :, :] *= 0.5                # wav1 channels carry 2x scale
    ws[:, 3:] *= 0.5
    w1_1 = np.zeros((3, 45, 128), np.float32)
    for dy in range(3):
        for dx in range(3):
            w1_1[dy, dx * 15:dx * 15 + 15, 0:64] = w1[:, :, dy, dx].T
    w1_1[1, 15:30, 64:128] = ws.T          # shortcut rides tap (dy=1, dx=1)
    P["w1_1"] = w1_1
    w2 = f32(b["w2"])                      # [64, 64, 3, 3]
    w2_1 = np.zeros((3, 128, 64), np.float32)   # pairs: taps (dy,0)+(dy,1)
    w2s_1 = np.zeros((3, 64, 64), np.float32)   # singles: taps (dy,2)
    for dy in range(3):
        w2_1[dy, 0:64] = w2[:, :, dy, 0].T
        w2_1[dy, 64:128] = w2[:, :, dy, 1].T
        w2s_1[dy] = w2[:, :, dy, 2].T
    P["w2_1"] = w2_1
    P["w2s_1"] = w2s_1
    gb = np.zeros((192, 2), np.float32)
    gb[0:64, 0], gb[0:64, 1] = f32(b["g1"]), f32(b["be1"])
    gb[64:128, 0], gb[64:128, 1] = f32(b["gs"]), f32(b["bes"])
    gb[128:192, 0], gb[128:192, 1] = f32(b["g2"]), f32(b["be2"])
    P["gb_1"] = gb

    # ---- block 2 ----
    b = params["blk2"]
    w1 = f32(b["w1"]).copy()               # [128, 76, 3, 3]
    ws = f32(b["ws"])[:, :, 0, 0].copy()
    w1[:, 64:, :, :] *= 0.25               # wav2 carries 4x
    ws[:, 64:] *= 0.25
    w1_2 = np.zeros((10, 76, 128), np.float32)
    for dy in range(3):
        for dx in range(3):
            w1_2[dy * 3 + dx] = w1[:, :, dy, dx].T
    w1_2[9] = ws.T
    P["w1_2"] = w1_2
    w2 = f32(b["w2"])
    w2_2 = np.zeros((9, 128, 128), np.float32)
    for dy in range(3):
        for dx in range(3):
            w2_2[dy * 3 + dx] = w2[:, :, dy, dx].T
    P["w2_2"] = w2_2
    gb = np.zeros((384, 2), np.float32)
    gb[0:128, 0], gb[0:128, 1] = f32(b["g1"]), f32(b["be1"])
    gb[128:256, 0], gb[128:256, 1] = f32(b["gs"]), f32(b["bes"])
    gb[256:384, 0], gb[256:384, 1] = f32(b["g2"]), f32(b["be2"])
    P["gb_2"] = gb

    # ---- blocks 3..5 ----
    for k in (3, 4, 5):
        spec = BLOCKS[k]
        Cp, Cout = spec["Cp"], spec["Cout"]
        Mt, Ktp = Cout // 128, Cp // 128
        b = params[f"blk{k}"]
        w1 = f32(b["w1"]).copy()
        ws = f32(b["ws"])[:, :, 0, 0].copy()
        wavscale = 0.5 ** k
        w1[:, Cp:, :, :] *= wavscale
        ws[:, Cp:] *= wavscale
        w1p = np.zeros((Mt, Ktp, 9, 128, 128), np.float32)
        w1w = np.zeros((Mt, 108, 128), np.float32)
        wsp = np.zeros((Mt, Ktp, 128, 128), np.float32)
        wsw = np.zeros((Mt, 12, 128), np.float32)
        for m in range(Mt):
            wm = w1[m * 128:(m + 1) * 128]
            for kt in range(Ktp):
                for dy in range(3):
                    for dx in range(3):
                        w1p[m, kt, dy * 3 + dx] = wm[:, kt * 128:(kt + 1) * 128, dy, dx].T
                wsp[m, kt] = ws[m * 128:(m + 1) * 128, kt * 128:(kt + 1) * 128].T
            for ti, t in enumerate(T_ORDER):
                dy, dx = t // 3, t % 3
                w1w[m, ti * 12:(ti + 1) * 12, :] = wm[:, Cp:, dy, dx].T
            wsw[m] = ws[m * 128:(m + 1) * 128, Cp:].T
        P[f"w1_{k}"] = w1p
        P[f"w1w_{k}"] = w1w
        P[f"ws_{k}"] = wsp
        P[f"wsw_{k}"] = wsw
        w2 = f32(b["w2"])
        w2p = np.zeros((Mt, Mt, 9, 128, 128), np.float32)
        for m in range(Mt):
            for kt in range(Mt):
                for dy in range(3):
                    for dx in range(3):
                        w2p[m, kt, dy * 3 + dx] = \
                            w2[m * 128:(m + 1) * 128, kt * 128:(kt + 1) * 128, dy, dx].T
        P[f"w2_{k}"] = w2p
        gb = np.zeros((3 * Cout, 2), np.float32)
        gb[0:Cout, 0], gb[0:Cout, 1] = f32(b["g1"]), f32(b["be1"])
        gb[Cout:2 * Cout, 0], gb[Cout:2 * Cout, 1] = f32(b["gs"]), f32(b["bes"])
        gb[2 * Cout:, 0], gb[2 * Cout:, 1] = f32(b["g2"]), f32(b["be2"])
        P[f"gb_{k}"] = gb
    return P


# ---------------------------------------------------------------------------
# device-side helpers
# ---------------------------------------------------------------------------

class Seg:
    """Accumulates bn_stats chunks for one [P, npix] conv-output segment."""

    def __init__(self, pool, P, nchunks, tag):
        self.P = P
        self.buf = pool.tile([P, max(nchunks, 1), 6], F32, tag=tag)
        self.n = 0

    def add(self, nc, src_flat):
        nc.vector.bn_stats(out=self.buf[:, self.n, :], in_=src_flat)
        self.n += 1

    def finish(self, nc, pool, tag):
        mv = pool.tile([self.P, 2], F32, tag=tag + "_mv")
        nc.vector.bn_aggr(out=mv, in_=self.buf[:, :self.n, :])
        return mv


def _pack_stats(nc, pool, mv, P, tag):
    """[P,2] (mean,var) -> [P,2] (mean/8, (var+mean^2)/8)."""
    pk = pool.tile([P, 2], F32, tag=tag + "_pk")
    tmp = pool.tile([P, 1], F32, tag=tag + "_tmp")
    nc.vector.tensor_tensor(out=tmp, in0=mv[:, 0:1], in1=mv[:, 0:1], op=OP.mult)
    nc.vector.tensor_tensor(out=pk[:, 1:2], in0=mv[:, 1:2], in1=tmp, op=OP.add)
    nc.scalar.mul(pk[:, 1:2], pk[:, 1:2], 1.0 / N_CORES)
    nc.scalar.mul(pk[:, 0:1], mv[:, 0:1], 1.0 / N_CORES)
    return pk


def _unpack_stats(nc, pool, g, gamma_beta, P, tag):
    """g [P,2] = (mean, E[x^2]) -> ac [P,2] = (a, c):
    a = gamma * rsqrt(var + eps), c = beta - mean * a."""
    ac = pool.tile([P, 2], F32, tag=tag + "_ac")
    tmp = pool.tile([P, 1], F32, tag=tag + "_t1")
    var = pool.tile([P, 1], F32, tag=tag + "_t2")
    nc.vector.tensor_tensor(out=tmp, in0=g[:, 0:1], in1=g[:, 0:1], op=OP.mult)
    nc.vector.tensor_tensor(out=var, in0=g[:, 1:2], in1=tmp, op=OP.subtract)
    eps = pool.tile([P, 1], F32, tag=tag + "_eps")
    nc.vector.memset(eps, BN_EPS)
    nc.scalar.activation(out=var, in_=var, func=ACTF.Sqrt, bias=eps, scale=1.0)
    nc.vector.reciprocal(out=var, in_=var)
    nc.vector.tensor_tensor(out=ac[:, 0:1], in0=var, in1=gamma_beta[:, 0:1], op=OP.mult)
    nc.vector.tensor_tensor(out=tmp, in0=g[:, 0:1], in1=ac[:, 0:1], op=OP.mult)
    nc.vector.tensor_tensor(out=ac[:, 1:2], in0=gamma_beta[:, 1:2], in1=tmp, op=OP.subtract)
    return ac


def _allreduce(nc, pool, segs_pk, cc_pair, replica, tag):
    """Pack per-segment [P,2] tiles into cci, AllReduce, read back tiles of
    (global mean, global E[x^2])."""
    cci, cco = cc_pair
    off = 0
    for pk in segs_pk:
        Pp = pk.shape[0]
        nc.sync.dma_start(out=cci[0, off:off + 2 * Pp], in_=pk)
        off += 2 * Pp
    nc.gpsimd.collective_compute(
        "AllReduce", OP.add, ins=[cci[:, :]], outs=[cco[:, :]],
        replica_groups=[replica])
    outs = []
    off = 0
    for i, pk in enumerate(segs_pk):
        Pp = pk.shape[0]
        g = pool.tile([Pp, 2], F32, tag=f"{tag}_g{i}")
        nc.sync.dma_start(out=g, in_=cco[0, off:off + 2 * Pp])
        off += 2 * Pp
        outs.append(g)
    return outs


def _load_gb(nc, pool, dram, row0, P, tag):
    t = pool.tile([P, 2], F32, tag=tag)
    nc.sync.dma_start(out=t, in_=dram[row0:row0 + P, :])
    return t


def _pad_memset(nc, t):
    S2 = t.shape[1]
    tb = t.bitcast(F32) if t.dtype == F32R else t
    nc.vector.memset(tb[:, 0, :], 0.0)
    nc.vector.memset(tb[:, S2 - 1, :], 0.0)
    nc.vector.memset(tb[:, :, 0], 0.0)
    nc.vector.memset(tb[:, :, S2 - 1], 0.0)


# ---------------------------------------------------------------------------
# kernel body
# ---------------------------------------------------------------------------

def _build_nc(num_devices=N_CORES, replica=None):
    if replica is None:
        replica = list(range(num_devices))
    nc = bacc.Bacc("TRN2", target_bir_lowering=False, debug=False,
                   num_devices=num_devices)

    x = nc.dram_tensor("x", [3, 512, 512], F32, kind="ExternalInput")
    wt = {}
    wt["w1_1"] = nc.dram_tensor("w1_1", [3, 45, 128], F32, kind="ExternalInput")
    wt["w2_1"] = nc.dram_tensor("w2_1", [3, 128, 64], F32, kind="ExternalInput")
    wt["w2s_1"] = nc.dram_tensor("w2s_1", [3, 64, 64], F32, kind="ExternalInput")
    wt["gb_1"] = nc.dram_tensor("gb_1", [192, 2], F32, kind="ExternalInput")
    wt["w1_2"] = nc.dram_tensor("w1_2", [10, 76, 128], F32, kind="ExternalInput")
    wt["w2_2"] = nc.dram_tensor("w2_2", [9, 128, 128], F32, kind="ExternalInput")
    wt["gb_2"] = nc.dram_tensor("gb_2", [384, 2], F32, kind="ExternalInput")
    for k in (3, 4, 5):
        Cp, Cout = BLOCKS[k]["Cp"], BLOCKS[k]["Cout"]
        Mt, Ktp = Cout // 128, Cp // 128
        wt[f"w1_{k}"] = nc.dram_tensor(f"w1_{k}", [Mt, Ktp, 9, 128, 128], F32, kind="ExternalInput")
        wt[f"w1w_{k}"] = nc.dram_tensor(f"w1w_{k}", [Mt, 108, 128], F32, kind="ExternalInput")
        wt[f"ws_{k}"] = nc.dram_tensor(f"ws_{k}", [Mt, Ktp, 128, 128], F32, kind="ExternalInput")
        wt[f"wsw_{k}"] = nc.dram_tensor(f"wsw_{k}", [Mt, 12, 128], F32, kind="ExternalInput")
        wt[f"w2_{k}"] = nc.dram_tensor(f"w2_{k}", [Mt, Mt, 9, 128, 128], F32, kind="ExternalInput")
        wt[f"gb_{k}"] = nc.dram_tensor(f"gb_{k}", [3 * Cout, 2], F32, kind="ExternalInput")

    eout = {k: nc.dram_tensor(f"e{k}", list(OUT_SHAPES[k]), F32, kind="ExternalOutput")
            for k in range(1, 6)}

    in1d = nc.dram_tensor("in1d", [45, 258, 256], F32)  # (dx,c), 1+256+1 rows, shifted cols
    ll_d = {1: nc.dram_tensor("ll1", [3, 256, 256], F32),
            2: nc.dram_tensor("ll2", [3, 128, 128], F32),
            3: nc.dram_tensor("ll3", [3, 64, 64], F32),
            4: nc.dram_tensor("ll4", [3, 32, 32], F32)}
    wav_d = {2: nc.dram_tensor("wav2", [12, 128, 128], F32),
             3: nc.dram_tensor("wav3", [12, 64, 64], F32),
             4: nc.dram_tensor("wav4", [12, 32, 32], F32),
             5: nc.dram_tensor("wav5", [12, 16, 16], F32)}
    yscD1 = nc.dram_tensor("yscD1", [64, 256, 256], BF16)
    y2D1 = nc.dram_tensor("y2D1", [64, 256, 256], BF16)
    yscD2 = nc.dram_tensor("yscD2", [128, 128, 128], BF16)
    cc = {}
    for tag, n in (("ar1_1", 256), ("ar2_1", 128), ("ar1_2", 512), ("ar2_2", 256),
                   ("ar1_3", 1024), ("ar2_3", 512), ("ar1_4", 2048), ("ar2_4", 1024),
                   ("ar1_5", 4096), ("ar2_5", 2048)):
        cc[tag] = (nc.dram_tensor(f"cci_{tag}", [1, n], F32),
                   nc.dram_tensor(f"cco_{tag}", [1, n], F32, addr_space="Shared"))

    with tile.TileContext(nc, pool_alloc_mode="queue") as tc:
        with ExitStack() as ctx:
            _emit(ctx, tc, nc, x, wt, eout, in1d, ll_d, wav_d,
                  yscD1, y2D1, yscD2, cc, replica)
    nc.compile()
    return nc


def _phase_wavelets(tc, nc, x, in1d, ll_d, wav_d, levels=(1, 2, 3, 4, 5)):
    with tc.tile_pool(name="wv", bufs=3) as wv:
        # zero pad rows (r=0, r=257) of the pre-shifted in1d45
        zr = wv.tile([128, 8192], F32, tag="zr", bufs=1)
        nc.vector.memset(zr, 0.0)
        nc.sync.dma_start(out=in1d[:, 0, :], in_=zr[0:45, 0:256])
        nc.sync.dma_start(out=in1d[:, 257, :], in_=zr[0:45, 0:256])
        # level 1: x [3,512,512] -> in1d channels (pool + wav1) + ll1
        for g in (range(6) if 1 in levels else ()):
            c, half = g // 2, g % 2
            y0 = 128 * half
            E = wv.tile([128, 512], F32, tag="E")
            Ot = wv.tile([128, 512], F32, tag="O")
            nc.sync.dma_start(out=E, in_=x[c, 2 * y0:2 * y0 + 256:2, :])
            nc.sync.dma_start(out=Ot, in_=x[c, 2 * y0 + 1:2 * y0 + 256:2, :])
            Ev = E.rearrange("p (a b) -> p a b", b=2)
            Ov = Ot.rearrange("p (a b) -> p a b", b=2)
            p_ = wv.tile([128, 256], F32, tag="p_")
            m_ = wv.tile([128, 256], F32, tag="m_")
            q_ = wv.tile([128, 256], F32, tag="q_")
            n_ = wv.tile([128, 256], F32, tag="n_")
            nc.vector.tensor_tensor(out=p_, in0=Ev[:, :, 0], in1=Ev[:, :, 1], op=OP.add)
            nc.vector.tensor_tensor(out=m_, in0=Ev[:, :, 0], in1=Ev[:, :, 1], op=OP.subtract)
            nc.vector.tensor_tensor(out=q_, in0=Ov[:, :, 0], in1=Ov[:, :, 1], op=OP.add)
            nc.vector.tensor_tensor(out=n_, in0=Ov[:, :, 0], in1=Ov[:, :, 1], op=OP.subtract)
            # subband/pool tiles carry zero cols at 0 and 257 so the three
            # dx-shifted DRAM writes are single full-width row DMAs
            sb = wv.tile([128, 4, 258], F32, tag="sb")
            nc.vector.memset(sb[:, :, 0], 0.0)
            nc.vector.memset(sb[:, :, 257], 0.0)
            nc.vector.tensor_tensor(out=sb[:, 0, 1:257], in0=p_, in1=q_, op=OP.add)
            nc.vector.tensor_tensor(out=sb[:, 1, 1:257], in0=p_, in1=q_, op=OP.subtract)
            nc.vector.tensor_tensor(out=sb[:, 2, 1:257], in0=m_, in1=n_, op=OP.add)
            nc.vector.tensor_tensor(out=sb[:, 3, 1:257], in0=m_, in1=n_, op=OP.subtract)
            pl = wv.tile([128, 258], F32, tag="pl")
            nc.vector.memset(pl[:, 0:1], 0.0)
            nc.vector.memset(pl[:, 257:258], 0.0)
            nc.vector.tensor_tensor(out=p_, in0=Ev[:, :, 0], in1=Ev[:, :, 1], op=OP.max)
            nc.vector.tensor_tensor(out=q_, in0=Ov[:, :, 0], in1=Ov[:, :, 1], op=OP.max)
            nc.vector.tensor_tensor(out=pl[:, 1:257], in0=p_, in1=q_, op=OP.max)
            # in1d45[(dx,c'), 1+r, x] = in1[c', r, x-1+dx]; src cols [dx, dx+256)
            for ch_, src in [(c, pl)] + [(3 + 4 * c + s, sb[:, s, :]) for s in range(4)]:
                for dx in range(3):
                    nc.sync.dma_start(
                        out=in1d[15 * dx + ch_, 1 + y0:1 + y0 + 128, :],
                        in_=src[:, dx:dx + 256])
            nc.sync.dma_start(out=ll_d[1][c, y0:y0 + 128, :], in_=sb[:, 0, 1:257])

        # levels 2..5 -> wav_d planes (+ ll chain)
        for lvl in (2, 3, 4, 5):
            if lvl not in levels:
                continue
            src = ll_d[lvl - 1]
            Sin = src.shape[1]
            Sout = Sin // 2
            for c in range(3):
                ng = (Sout + 127) // 128
                for gi in range(ng):
                    y0 = gi * 128
                    nrr = min(128, Sout - y0)
                    E = wv.tile([128, 512], F32, tag="E")
                    Ot = wv.tile([128, 512], F32, tag="O")
                    nc.sync.dma_start(out=E[:nrr, :Sin], in_=src[c, 2 * y0:2 * (y0 + nrr):2, :])
                    nc.sync.dma_start(out=Ot[:nrr, :Sin], in_=src[c, 2 * y0 + 1:2 * (y0 + nrr):2, :])
                    Ev = E[:nrr, :Sin].rearrange("p (a b) -> p a b", b=2)
                    Ov = Ot[:nrr, :Sin].rearrange("p (a b) -> p a b", b=2)
                    p_ = wv.tile([128, 256], F32, tag="p_")
                    m_ = wv.tile([128, 256], F32, tag="m_")
                    q_ = wv.tile([128, 256], F32, tag="q_")
                    n_ = wv.tile([128, 256], F32, tag="n_")
                    nc.vector.tensor_tensor(out=p_[:nrr, :Sout], in0=Ev[:, :, 0], in1=Ev[:, :, 1], op=OP.add)
                    nc.vector.tensor_tensor(out=m_[:nrr, :Sout], in0=Ev[:, :, 0], in1=Ev[:, :, 1], op=OP.subtract)
                    nc.vector.tensor_tensor(out=q_[:nrr, :Sout], in0=Ov[:, :, 0], in1=Ov[:, :, 1], op=OP.add)
                    nc.vector.tensor_tensor(out=n_[:nrr, :Sout], in0=Ov[:, :, 0], in1=Ov[:, :, 1], op=OP.subtract)
                    sb = wv.tile([128, 4, 256], F32, tag="sb")
                    nc.vector.tensor_tensor(out=sb[:nrr, 0, :Sout], in0=p_[:nrr, :Sout], in1=q_[:nrr, :Sout], op=OP.add)
                    nc.vector.tensor_tensor(out=sb[:nrr, 1, :Sout], in0=p_[:nrr, :Sout], in1=q_[:nrr, :Sout], op=OP.subtract)
                    nc.vector.tensor_tensor(out=sb[:nrr, 2, :Sout], in0=m_[:nrr, :Sout], in1=n_[:nrr, :Sout], op=OP.add)
                    nc.vector.tensor_tensor(out=sb[:nrr, 3, :Sout], in0=m_[:nrr, :Sout], in1=n_[:nrr, :Sout], op=OP.subtract)
                    for s in range(4):
                        nc.sync.dma_start(out=wav_d[lvl][4 * c + s, y0:y0 + nrr, :],
                                          in_=sb[:nrr, s, :Sout])
                    if lvl < 5:
                        nc.sync.dma_start(out=ll_d[lvl][c, y0:y0 + nrr, :],
                                          in_=sb[:nrr, 0, :Sout])


def _emit(ctx, tc, nc, x, wt, eout, in1d, ll_d, wav_d, yscD1, y2D1, yscD2,
          cc, replica):
    _NEXT_IN.clear()
    small = ctx.enter_context(tc.tile_pool(name="small", bufs=1))

    _phase_wavelets(tc, nc, x, in1d, ll_d, wav_d, levels=(1,))

    # =====================================================================
    # Block 1 (strip-tiled; y1 SBUF bf16; ysc & y2 staged in DRAM bf16)
    # =====================================================================

    with tc.tile_pool(name="b1w", bufs=1) as b1w:
        w1l = b1w.tile([45, 3, 128], F32R, tag="w1l")
        nc.sync.dma_start(out=w1l, in_=wt["w1_1"][:, :, :].rearrange("a b c -> b a c").bitcast(F32R))
        w2l = b1w.tile([128, 3, 64], F32R, tag="w2l")
        nc.sync.dma_start(out=w2l, in_=wt["w2_1"][:, :, :].rearrange("a b c -> b a c").bitcast(F32R))
        w2sl = b1w.tile([64, 3, 64], F32R, tag="w2sl")
        nc.sync.dma_start(out=w2sl, in_=wt["w2s_1"][:, :, :].rearrange("a b c -> b a c").bitcast(F32R))

        _phase_wavelets(tc, nc, x, in1d, ll_d, wav_d, levels=(2, 3, 4, 5))

        with tc.tile_pool(name="pY1", bufs=1) as pY1:
            Y1 = pY1.tile([128, 34816], BF16, tag="Y1")  # part 64h+c; free (r%128)*256+x

            # ---------------- pass A: conv1 + shortcut ----------------
            with tc.tile_pool(name="b1a", bufs=2) as b1a, \
                 tc.tile_pool(name="b1ps", bufs=4, space="PSUM") as b1ps, \
                 tc.tile_pool(name="b1st", bufs=2) as b1st:
                seg1 = Seg(small, 128, 128, "seg1")
                for s in range(8):
                    # T45[(dx,c), yy, x] = in1[c, 32s-1+yy, x-1+dx] (pre-shifted DRAM)
                    T45 = b1a.tile([45, 34, 256], F32R, tag="T45")
                    nc.sync.dma_start(out=T45[:, :, :],
                                      in_=in1d[:, 32 * s:32 * s + 34, :].bitcast(F32R))
                    yscS = b1st.tile([64, 32, 256], BF16, tag="yscS")
                    h = s // 4
                    for i in range(16):
                        yo = 2 * i
                        ps = b1ps.tile([128, 2, 256], F32, tag="ps1")
                        for dy in range(3):
                            nc.tensor.matmul(ps, r32(w1l[:, dy, :]),
                                             r32(T45[:, yo + dy:yo + dy + 2, :]),
                                             start=(dy == 0), stop=(dy == 2))
                        psf = ps.rearrange("p a b -> p (a b)")
                        seg1.add(nc, psf)
                        rr = (32 * s + yo) % 128
                        nc.scalar.copy(Y1[64 * h:64 * h + 64, rr * 256:(rr + 2) * 256],
                                       psf[0:64, :])
                        nc.scalar.copy(yscS[:, yo:yo + 2, :], ps[64:128, :, :])
                    nc.sync.dma_start(out=yscD1[:, 32 * s:32 * s + 32, :], in_=yscS)
                mv1 = seg1.finish(nc, small, "seg1")
                pk1 = _pack_stats(nc, small, mv1, 128, "pk1")
            g1 = _allreduce(nc, small, [pk1], cc["ar1_1"], replica, "ar11")[0]
            gb1a = _load_gb(nc, small, wt["gb_1"], 0, 128, "gb1a")
            ac1 = _unpack_stats(nc, small, g1, gb1a, 128, "ac1")
            acs_d = small.tile([128, 2], F32, tag="acs_d")
            nc.scalar.copy(acs_d[0:64, :], ac1[64:128, :])
            nc.scalar.copy(acs_d[64:128, :], ac1[64:128, :])

            # ---------------- pass B: t1 = relu(bn(y1)); conv2 ----------------
            with tc.tile_pool(name="b1b", bufs=2) as b1b, \
                 tc.tile_pool(name="b1ps2", bufs=4, space="PSUM") as b1ps2, \
                 tc.tile_pool(name="b1st2", bufs=2) as b1st2:
                seg2 = Seg(small, 64, 128, "seg2")
                for s in range(8):
                    # partitions 0:64 = t1; 64:128 = t1 shifted left one col
                    t1 = b1b.tile([128, 34, 258], F32R, tag="t1")
                    nc.vector.memset(t1.bitcast(F32)[:, :, 0], 0.0)
                    nc.vector.memset(t1.bitcast(F32)[:, :, 257], 0.0)
                    r0, r1 = 32 * s - 1, 32 * s + 33
                    if r0 < 0:
                        nc.vector.memset(t1.bitcast(F32)[:, 0, :], 0.0)
                        r0 = 0
                    if r1 > 256:
                        nc.vector.memset(t1.bitcast(F32)[:, 33, :], 0.0)
                        r1 = 256
                    spans = []
                    if r0 < 128:
                        spans.append((0, r0, min(r1, 128)))
                    if r1 > 128:
                        spans.append((1, max(r0, 128), r1))
                    for h, a, bnd in spans:
                        Yv = Y1[64 * h:64 * h + 64,
                                (a % 128) * 256:((a % 128) + (bnd - a)) * 256]
                        yy = a - (32 * s - 1)
                        nc.scalar.activation(
                            out=t1[0:64, yy:yy + (bnd - a), 1:257],
                            in_=Yv.rearrange("p (r c) -> p r c", c=256),
                            func=ACTF.Relu,
                            bias=ac1[0:64, 1:2], scale=ac1[0:64, 0:1])
                    # duplicate t1 into partitions 64:128 shifted left one col
                    nc.sync.dma_start(out=t1[64:128, :, 0:257],
                                      in_=t1[0:64, :, 1:258])
                    y2S = b1st2.tile([64, 32, 256], BF16, tag="y2S")
                    for i in range(16):
                        yo = 2 * i
                        ps = b1ps2.tile([64, 2, 256], F32, tag="ps2")
                        for dy in range(3):
                            nc.tensor.matmul(
                                ps, r32(w2l[:, dy, :]),
                                r32(t1[:, yo + dy:yo + dy + 2, 0:256]),
                                start=(dy == 0), stop=False)
                        for dy in range(3):
                            nc.tensor.matmul(
                                ps, r32(w2sl[:, dy, :]),
                                r32(t1[0:64, yo + dy:yo + dy + 2, 2:258]),
                                start=False, stop=(dy == 2))
                        psf = ps.rearrange("p a b -> p (a b)")
                        seg2.add(nc, psf)
                        nc.scalar.copy(y2S[:, yo:yo + 2, :], ps)
                    nc.sync.dma_start(out=y2D1[:, 32 * s:32 * s + 32, :], in_=y2S)
                mv2 = seg2.finish(nc, small, "seg2")
                pk2 = _pack_stats(nc, small, mv2, 64, "pk2")
        # Y1 pool closed here
        g2 = _allreduce(nc, small, [pk2], cc["ar2_1"], replica, "ar21")[0]
        gb1b = _load_gb(nc, small, wt["gb_1"], 128, 64, "gb1b")
        ac2 = _unpack_stats(nc, small, g2, gb1b, 64, "ac2")
        ac2_d = small.tile([128, 2], F32, tag="ac2_d")
        nc.scalar.copy(ac2_d[0:64, :], ac2)
        nc.scalar.copy(ac2_d[64:128, :], ac2)
        ccs1 = small.tile([128, 1], F32, tag="ccs1")
        nc.vector.tensor_tensor(out=ccs1, in0=acs_d[:, 1:2], in1=ac2_d[:, 1:2], op=OP.add)

    # in2sb spans blk1 pass C .. blk2 pass A
    p_in2 = tc.tile_pool(name="p_in2", bufs=1, side="right")
    in2p = p_in2.__enter__()
    in2sb = in2p.tile([76, 130, 130], F32R, tag="in2sb")
    _pad_memset(nc, in2sb)
    # wav2 channels
    nc.sync.dma_start(out=in2sb[64:76, 1:129, 1:129], in_=wav_d[2][:, :, :].bitcast(F32R))

    # ---------------- blk1 pass C ----------------
    with tc.tile_pool(name="b1c", bufs=2) as b1c:
        for j in range(16):
            # chunk covers rows [8j,8j+8) of each half
            y2c = b1c.tile([128, 8, 256], BF16, tag="y2c")
            ysc = b1c.tile([128, 8, 256], BF16, tag="ysc")
            for h in range(2):
                rb = 128 * h + 8 * j
                nc.sync.dma_start(out=y2c[64 * h:64 * h + 64, :, :],
                                  in_=y2D1[:, rb:rb + 8, :])
                nc.sync.dma_start(out=ysc[64 * h:64 * h + 64, :, :],
                                  in_=yscD1[:, rb:rb + 8, :])
            tmp = b1c.tile([128, 8, 256], F32, tag="tmpc")
            nc.scalar.activation(out=tmp, in_=ysc, func=ACTF.Identity,
                                 bias=ccs1[:, 0:1], scale=acs_d[:, 0:1])
            nc.vector.scalar_tensor_tensor(
                out=tmp, in0=y2c, scalar=ac2_d[:, 0:1], in1=tmp,
                op0=OP.mult, op1=OP.add)
            nc.scalar.activation(out=tmp, in_=tmp, func=ACTF.Relu)
            for h in range(2):
                rb = 128 * h + 8 * j
                nc.sync.dma_start(out=eout[1][:, rb:rb + 8, :],
                                  in_=tmp[64 * h:64 * h + 64, :, :])
            m1 = b1c.tile([128, 8, 128], F32, tag="m1")
            tv = tmp.rearrange("p r (c d) -> p r c d", d=2)
            nc.vector.tensor_tensor(out=m1, in0=tv[:, :, :, 0], in1=tv[:, :, :, 1], op=OP.max)
            m2 = b1c.tile([128, 4, 128], F32, tag="m2")
            m1v = m1.rearrange("p (r d) c -> p r d c", d=2)
            nc.vector.tensor_tensor(out=m2, in0=m1v[:, :, 0, :], in1=m1v[:, :, 1, :], op=OP.max)
            for h in range(2):
                rb = 64 * h + 4 * j
                nc.sync.dma_start(out=in2sb[0:64, 1 + rb:1 + rb + 4, 1:129],
                                  in_=m2[64 * h:64 * h + 64, :, :].bitcast(F32R))

    # =====================================================================
    # Block 2
    # =====================================================================
    _emit_block2(tc, nc, wt, eout, in2sb, p_in2, yscD2, wav_d, cc, replica, small)

    # =====================================================================
    # Blocks 3..5
    # =====================================================================
    # in3a was created by _emit_block2 pass C (returned via small registry)
    _emit_blockk(tc, nc, 3, wt, eout, cc, replica, small, wav_d,
                 stream_w1=False, stream_w2=False)
    _emit_blockk(tc, nc, 4, wt, eout, cc, replica, small, wav_d,
                 stream_w1=True, stream_w2=True)
    _emit_blockk(tc, nc, 5, wt, eout, cc, replica, small, wav_d,
                 stream_w1=True, stream_w2=True)


_NEXT_IN = {}  # k -> list of [128, Spad, Spad] pool-input tiles (built by k-1)


def _open_next_in(tc, nc, k):
    """Create block k's pool-channel input tiles (padded, borders zeroed)."""
    spec = BLOCKS[k]
    S, Ktp = spec["S"], spec["Cp"] // 128 if k >= 3 else 1
    Spad = S + 2
    pool_cm = tc.tile_pool(name=f"p_in{k}", bufs=1, side="right")
    p = pool_cm.__enter__()
    nt = max(1, spec["Cp"] // 128)
    npart = 128 if spec["Cp"] >= 128 else spec["Cp"]
    tiles = []
    for i in range(nt):
        t = p.tile([npart, Spad, Spad], F32R, tag=f"in{k}_{i}", name=f"in{k}_{i}")
        tiles.append(t)
    for t in tiles:
        _pad_memset(nc, t)
    _NEXT_IN[k] = (tiles, pool_cm, p)
    return tiles


def _emit_block2(tc, nc, wt, eout, in2sb, p_in2_cm, yscD2, wav_d, cc, replica, small):
    S, Spad, nr = 128, 130, 4
    ngr = S // nr

    with tc.tile_pool(name="b2y", bufs=1) as b2y:
        y1b = b2y.tile([128, S * S], F32, tag="y1b2")
        with tc.tile_pool(name="b2w", bufs=1) as b2w:
            w1l = b2w.tile([76, 10, 128], F32R, tag="w1l2")
            nc.sync.dma_start(out=w1l, in_=wt["w1_2"][:, :, :].rearrange("a b c -> b a c").bitcast(F32R))
            w2l = b2w.tile([128, 9, 128], F32R, tag="w2l2")
            nc.sync.dma_start(out=w2l, in_=wt["w2_2"][:, :, :].rearrange("a b c -> b a c").bitcast(F32R))

            # ---- pass A: conv1 + sc ----
            with tc.tile_pool(name="b2ps", bufs=4, space="PSUM") as psp, \
                 tc.tile_pool(name="b2st", bufs=2) as stp:
                seg1 = Seg(small, 128, ngr, "b2seg1")
                segs = Seg(small, 128, ngr, "b2segs")
                for g in range(ngr):
                    y0 = g * nr
                    ps = psp.tile([128, nr, S], F32, tag="psA")
                    first = True
                    for dy in range(3):
                        for dx in range(3):
                            nc.tensor.matmul(
                                ps, r32(w1l[:, dy * 3 + dx, :]),
                                r32(in2sb[:, y0 + dy:y0 + dy + nr, dx:dx + S]),
                                start=first, stop=(dy == 2 and dx == 2))
                            first = False
                    pss = psp.tile([128, nr, S], F32, tag="psS")
                    nc.tensor.matmul(pss, r32(w1l[:, 9, :]),
                                     r32(in2sb[:, 1 + y0:1 + y0 + nr, 1:1 + S]),
                                     start=True, stop=True)
                    seg1.add(nc, ps.rearrange("p a b -> p (a b)"))
                    segs.add(nc, pss.rearrange("p a b -> p (a b)"))
                    nc.scalar.copy(y1b[:, y0 * S:(y0 + nr) * S],
                                   ps.rearrange("p a b -> p (a b)"))
                    yscS = stp.tile([128, nr, S], BF16, tag="yscS2")
                    nc.scalar.copy(yscS, pss)
                    nc.sync.dma_start(out=yscD2[:, y0:y0 + nr, :], in_=yscS)
                mv1 = seg1.finish(nc, small, "b2seg1")
                mvs = segs.finish(nc, small, "b2segs")
                pk1 = _pack_stats(nc, small, mv1, 128, "b2pk1")
                pks = _pack_stats(nc, small, mvs, 128, "b2pks")
            # in2sb dead from here
            p_in2_cm.__exit__(None, None, None)
            gars = _allreduce(nc, small, [pk1, pks], cc["ar1_2"], replica, "b2ar1")
            ac1 = _unpack_stats(nc, small, gars[0], _load_gb(nc, small, wt["gb_2"], 0, 128, "gb2_1"), 128, "b2ac1")
            acs = _unpack_stats(nc, small, gars[1], _load_gb(nc, small, wt["gb_2"], 128, 128, "gb2_s"), 128, "b2acs")

            # ---- pass B: t1 strips + conv2 ----
            with tc.tile_pool(name="b2y2", bufs=1) as b2y2:
                y2b = b2y2.tile([128, S * S], BF16, tag="y2b2")
                with tc.tile_pool(name="b2b", bufs=3) as b2b, \
                     tc.tile_pool(name="b2ps2", bufs=4, space="PSUM") as psp2:
                    seg2 = Seg(small, 128, ngr, "b2seg2")
                    y1v = y1b.rearrange("p (r c) -> p r c", c=S)
                    for g in range(ngr):
                        y0 = g * nr
                        t1 = b2b.tile([128, nr + 2, Spad], F32R, tag="t1s2")
                        nc.vector.memset(t1.bitcast(F32)[:, :, 0], 0.0)
                        nc.vector.memset(t1.bitcast(F32)[:, :, Spad - 1], 0.0)
                        r0, r1 = y0 - 1, y0 + nr + 1
                        if r0 < 0:
                            nc.vector.memset(t1.bitcast(F32)[:, 0, :], 0.0)
                            r0 = 0
                        if r1 > S:
                            nc.vector.memset(t1.bitcast(F32)[:, nr + 1, :], 0.0)
                            r1 = S
                        nc.scalar.activation(
                            out=t1[:, r0 - (y0 - 1):r1 - (y0 - 1), 1:1 + S],
                            in_=y1v[:, r0:r1, :], func=ACTF.Relu,
                            bias=ac1[:, 1:2], scale=ac1[:, 0:1])
                        ps = psp2.tile([128, nr, S], F32, tag="psB")
                        first = True
                        for dy in range(3):
                            for dx in range(3):
                                nc.tensor.matmul(
                                    ps, r32(w2l[:, dy * 3 + dx, :]),
                                    r32(t1[:, dy:dy + nr, dx:dx + S]),
                                    start=first, stop=(dy == 2 and dx == 2))
                                first = False
                        seg2.add(nc, ps.rearrange("p a b -> p (a b)"))
                        nc.scalar.copy(
                            y2b.rearrange("p (r c) -> p r c", c=S)[:, y0:y0 + nr, :], ps)
                    mv2 = seg2.finish(nc, small, "b2seg2")
                    pk2 = _pack_stats(nc, small, mv2, 128, "b2pk2")
                # y1b dead
                g2 = _allreduce(nc, small, [pk2], cc["ar2_2"], replica, "b2ar2")[0]
                ac2 = _unpack_stats(nc, small, g2, _load_gb(nc, small, wt["gb_2"], 256, 128, "gb2_2"), 128, "b2ac2")
                ccs = small.tile([128, 1], F32, tag="b2ccs")
                nc.vector.tensor_tensor(out=ccs, in0=acs[:, 1:2], in1=ac2[:, 1:2], op=OP.add)

                # ---- pass C ----
                in3 = _open_next_in(tc, nc, 3)
                with tc.tile_pool(name="b2c", bufs=1) as bc:
                    for j in range(16):
                        rb = 8 * j
                        ysc = bc.tile([128, 8, S], BF16, tag="yscC2")
                        nc.sync.dma_start(out=ysc, in_=yscD2[:, rb:rb + 8, :])
                        tmp = bc.tile([128, 8, S], F32, tag="tmpC2")
                        nc.scalar.activation(out=tmp, in_=ysc, func=ACTF.Identity,
                                             bias=ccs[:, 0:1], scale=acs[:, 0:1])
                        y2v = y2b.rearrange("p (r c) -> p r c", c=S)[:, rb:rb + 8, :]
                        nc.vector.scalar_tensor_tensor(
                            out=tmp, in0=y2v, scalar=ac2[:, 0:1], in1=tmp,
                            op0=OP.mult, op1=OP.add)
                        nc.scalar.activation(out=tmp, in_=tmp, func=ACTF.Relu)
                        nc.sync.dma_start(out=eout[2][:, rb:rb + 8, :], in_=tmp)
                        m1 = bc.tile([128, 8, 64], F32, tag="m1C2")
                        tv = tmp.rearrange("p r (c d) -> p r c d", d=2)
                        nc.vector.tensor_tensor(out=m1, in0=tv[:, :, :, 0],
                                                in1=tv[:, :, :, 1], op=OP.max)
                        m2 = bc.tile([128, 4, 64], F32, tag="m2C2")
                        m1v = m1.rearrange("p (r d) c -> p r d c", d=2)
                        nc.vector.tensor_tensor(out=m2, in0=m1v[:, :, 0, :],
                                                in1=m1v[:, :, 1, :], op=OP.max)
                        nc.sync.dma_start(out=in3[0][:, 1 + 4 * j:1 + 4 * j + 4, 1:65],
                                          in_=m2.bitcast(F32R))


def _emit_blockk(tc, nc, k, wt, eout, cc, replica, small, wav_d,
                 stream_w1=False, stream_w2=False):
    spec = BLOCKS[k]
    Cp, Cout, S, nr = spec["Cp"], spec["Cout"], spec["S"], spec["nr"]
    Spad = S + 2
    Mt, Ktp = Cout // 128, Cp // 128
    ngr = S // nr
    inP, inP_cm, inP_pool = _NEXT_IN[k]

    with tc.tile_pool(name=f"bk{k}w", bufs=1) as bkw:
        bkT_cm = tc.tile_pool(name=f"bk{k}T", bufs=1, side="right")
        bkT = bkT_cm.__enter__()
        # wavelet im2col from DRAM (padded implicitly via shifts + zero pad)
        T108 = bkT.tile([108, S, S], F32R, tag=f"T108_{k}")
        wpad = bkT.tile([12, Spad, Spad], F32R, tag=f"wpad{k}")
        _pad_memset(nc, wpad)
        nc.sync.dma_start(out=wpad[:, 1:1 + S, 1:1 + S], in_=wav_d[k][:, :, :].bitcast(F32R))
        for ti, t in enumerate(T_ORDER):
            dy, dx = t // 3, t % 3
            nc.sync.dma_start(out=T108[12 * ti:12 * ti + 12, :, :],
                              in_=wpad[:, dy:dy + S, dx:dx + S])
        w1wl = bkw.tile([108, Mt, 128], F32R, tag=f"w1wl{k}")
        nc.sync.dma_start(out=w1wl, in_=wt[f"w1w_{k}"][:, :, :].rearrange("a b c -> b a c").bitcast(F32R))
        wswl = bkw.tile([12, Mt, 128], F32R, tag=f"wswl{k}")
        nc.sync.dma_start(out=wswl, in_=wt[f"wsw_{k}"][:, :, :].rearrange("a b c -> b a c").bitcast(F32R))
        wspl = bkw.tile([128, Mt, Ktp, 128], F32R, tag=f"wspl{k}")
        nc.sync.dma_start(out=wspl, in_=wt[f"ws_{k}"][:, :, :, :].rearrange("a b c d -> c a b d").bitcast(F32R))
        if not stream_w1:
            w1pl = bkw.tile([128, Mt, Ktp, 9, 128], F32R, tag=f"w1pl{k}")
            nc.sync.dma_start(out=w1pl,
                              in_=wt[f"w1_{k}"][:, :, :, :, :].rearrange("a b c d e -> d a b c e").bitcast(F32R))

        with tc.tile_pool(name=f"bk{k}ys", bufs=1) as bkys:
            y1b = [inP_pool.tile([128, S * S], F32, tag=f"y1b{k}_{m}", name=f"y1b{k}_{m}") for m in range(Mt)]
            yscb = [bkys.tile([128, S * S], F32, tag=f"yscb{k}_{m}", name=f"yscb{k}_{m}") for m in range(Mt)]

            # ---- pass A ----
            with tc.tile_pool(name=f"b{k}ps", bufs=4, space="PSUM") as psp, \
                 tc.tile_pool(name=f"b{k}wst", bufs=2) as wstr:
                seg1 = [Seg(small, 128, ngr, f"b{k}seg1_{m}") for m in range(Mt)]
                segs = [Seg(small, 128, ngr, f"b{k}segs_{m}") for m in range(Mt)]
                for m in range(Mt):
                    w1m = {}
                    if stream_w1:
                        for kt in range(Ktp):
                            w1kt = wstr.tile([128, 9, 128], F32R, tag=f"w1m{kt % 2}",
                                             name=f"w1m_{m}_{kt}")
                            nc.sync.dma_start(out=w1kt,
                                              in_=wt[f"w1_{k}"][m, kt].rearrange("d e f -> e d f").bitcast(F32R))
                            w1m[kt] = w1kt
                    for g in range(ngr):
                        y0 = g * nr
                        ps = psp.tile([128, nr, S], F32, tag="psA")
                        first = True
                        for kt in range(Ktp):
                            for dy in range(3):
                                for dx in range(3):
                                    lw = (w1m[kt][:, dy * 3 + dx, :] if stream_w1
                                          else w1pl[:, m, kt, dy * 3 + dx, :])
                                    nc.tensor.matmul(
                                        ps, r32(lw),
                                        r32(inP[kt][:, y0 + dy:y0 + dy + nr, dx:dx + S]),
                                        start=first, stop=False)
                                    first = False
                        nc.tensor.matmul(ps, r32(w1wl[:, m, :]),
                                         r32(T108[:, y0:y0 + nr, :]),
                                         start=False, stop=True)
                        pss = psp.tile([128, nr, S], F32, tag="psS")
                        for kt in range(Ktp):
                            nc.tensor.matmul(pss, r32(wspl[:, m, kt, :]),
                                             r32(inP[kt][:, 1 + y0:1 + y0 + nr, 1:1 + S]),
                                             start=(kt == 0), stop=False)
                        nc.tensor.matmul(pss, r32(wswl[:, m, :]),
                                         r32(T108[0:12, y0:y0 + nr, :]),
                                         start=False, stop=True)
                        seg1[m].add(nc, ps.rearrange("p a b -> p (a b)"))
                        segs[m].add(nc, pss.rearrange("p a b -> p (a b)"))
                        nc.scalar.copy(y1b[m][:, y0 * S:(y0 + nr) * S],
                                       ps.rearrange("p a b -> p (a b)"))
                        nc.scalar.copy(yscb[m][:, y0 * S:(y0 + nr) * S],
                                       pss.rearrange("p a b -> p (a b)"))
                pks = [_pack_stats(nc, small, seg1[m].finish(nc, small, f"b{k}seg1_{m}"),
                                   128, f"b{k}p1{m}") for m in range(Mt)] + \
                      [_pack_stats(nc, small, segs[m].finish(nc, small, f"b{k}segs_{m}"),
                                   128, f"b{k}ps{m}") for m in range(Mt)]
            bkT_cm.__exit__(None, None, None)  # T108/wpad dead after pass A
            gl = _allreduce(nc, small, pks, cc[f"ar1_{k}"], replica, f"b{k}ar1")
            ac1 = [_unpack_stats(nc, small, gl[m],
                                 _load_gb(nc, small, wt[f"gb_{k}"], m * 128, 128, f"gbl{k}1{m}"),
                                 128, f"b{k}ac1{m}") for m in range(Mt)]
            acs = [_unpack_stats(nc, small, gl[Mt + m],
                                 _load_gb(nc, small, wt[f"gb_{k}"], Cout + m * 128, 128, f"gbl{k}s{m}"),
                                 128, f"b{k}acs{m}") for m in range(Mt)]

            # ---- pass B ----
            with tc.tile_pool(name=f"bk{k}y2", bufs=1) as bky2:
                y2b = [bky2.tile([128, S * S], F32, tag=f"y2b{k}_{m}", name=f"y2b{k}_{m}") for m in range(Mt)]
                with tc.tile_pool(name=f"b{k}t1s", bufs=2) as bt1s, \
                     tc.tile_pool(name=f"b{k}ps2", bufs=4, space="PSUM") as psp2, \
                     tc.tile_pool(name=f"b{k}wst2", bufs=2) as wstr2:
                    seg2 = [Seg(small, 128, ngr, f"b{k}seg2_{m}") for m in range(Mt)]
                    if not stream_w2:
                        w2lf = bkw.tile([128, Mt, Mt, 9, 128], F32R, tag=f"w2l{k}")
                        nc.sync.dma_start(out=w2lf,
                                          in_=wt[f"w2_{k}"][:, :, :, :, :].rearrange("a b c d e -> d a b c e").bitcast(F32R))
                    for m in range(Mt):
                        w2m = {}
                        if stream_w2:
                            for kt in range(Mt):
                                w2kt = wstr2.tile([128, 9, 128], F32R, tag=f"w2m{kt % 2}",
                                                  name=f"w2m_{m}_{kt}")
                                nc.sync.dma_start(out=w2kt,
                                                  in_=wt[f"w2_{k}"][m, kt].rearrange("d e f -> e d f").bitcast(F32R))
                                w2m[kt] = w2kt
                        for g in range(ngr):
                            y0 = g * nr
                            t1s = []
                            for kt in range(Mt):
                                t1k = bt1s.tile([128, nr + 2, Spad], F32R,
                                                tag=f"t1s{kt}", name=f"t1s{kt}")
                                nc.vector.memset(t1k.bitcast(F32)[:, :, 0], 0.0)
                                nc.vector.memset(t1k.bitcast(F32)[:, :, Spad - 1], 0.0)
                                r0, r1 = y0 - 1, y0 + nr + 1
                                if r0 < 0:
                                    nc.vector.memset(t1k.bitcast(F32)[:, 0, :], 0.0)
                                    r0 = 0
                                if r1 > S:
                                    nc.vector.memset(t1k.bitcast(F32)[:, nr + 1, :], 0.0)
                                    r1 = S
                                nc.scalar.activation(
                                    out=t1k[:, r0 - (y0 - 1):r1 - (y0 - 1), 1:1 + S],
                                    in_=y1b[kt].rearrange("p (r c) -> p r c", c=S)[:, r0:r1, :],
                                    func=ACTF.Relu, bias=ac1[kt][:, 1:2], scale=ac1[kt][:, 0:1])
                                t1s.append(t1k)
                            ps = psp2.tile([128, nr, S], F32, tag="psB")
                            first = True
                            for kt in range(Mt):
                                for dy in range(3):
                                    for dx in range(3):
                                        lw = (w2m[kt][:, dy * 3 + dx, :] if stream_w2
                                              else w2lf[:, m, kt, dy * 3 + dx, :])
                                        nc.tensor.matmul(
                                            ps, r32(lw),
                                            r32(t1s[kt][:, dy:dy + nr, dx:dx + S]),
                                            start=first,
                                            stop=(kt == Mt - 1 and dy == 2 and dx == 2))
                                        first = False
                            seg2[m].add(nc, ps.rearrange("p a b -> p (a b)"))
                            nc.scalar.copy(y2b[m][:, y0 * S:(y0 + nr) * S],
                                           ps.rearrange("p a b -> p (a b)"))
                    pk2 = [_pack_stats(nc, small, seg2[m].finish(nc, small, f"b{k}seg2_{m}"),
                                       128, f"b{k}p2{m}") for m in range(Mt)]
                inP_cm.__exit__(None, None, None)  # in-tiles + y1b dead
                gl2 = _allreduce(nc, small, pk2, cc[f"ar2_{k}"], replica, f"b{k}ar2")
                ac2 = [_unpack_stats(nc, small, gl2[m],
                                     _load_gb(nc, small, wt[f"gb_{k}"], 2 * Cout + m * 128, 128, f"gbl{k}2{m}"),
                                     128, f"b{k}ac2{m}") for m in range(Mt)]

                # ---- pass C ----
                outP = _open_next_in(tc, nc, k + 1) if k < 5 else None
                ncch = S // 16 if S >= 32 else 1   # row chunks
                rch = S // ncch
                with tc.tile_pool(name=f"b{k}c", bufs=2) as bc:
                    ccs_l = []
                    for m in range(Mt):
                        ccs = small.tile([128, 1], F32, tag=f"b{k}ccs{m}", name=f"b{k}ccs{m}")
                        nc.vector.tensor_tensor(out=ccs, in0=acs[m][:, 1:2],
                                                in1=ac2[m][:, 1:2], op=OP.add)
                        ccs_l.append(ccs)
                    for ch in range(ncch):
                        for m in range(Mt):
                            ccs = ccs_l[m]
                            rb = ch * rch
                            tmp = bc.tile([128, rch, S], F32, tag="tmpC")
                            nc.scalar.activation(
                                out=tmp,
                                in_=yscb[m].rearrange("p (r c) -> p r c", c=S)[:, rb:rb + rch, :],
                                func=ACTF.Identity, bias=ccs[:, 0:1], scale=acs[m][:, 0:1])
                            nc.vector.scalar_tensor_tensor(
                                out=tmp,
                                in0=y2b[m].rearrange("p (r c) -> p r c", c=S)[:, rb:rb + rch, :],
                                scalar=ac2[m][:, 0:1], in1=tmp, op0=OP.mult, op1=OP.add)
                            nc.scalar.activation(out=tmp, in_=tmp, func=ACTF.Relu)
                            nc.sync.dma_start(out=eout[k][m * 128:(m + 1) * 128, rb:rb + rch, :], in_=tmp)
                            if k < 5:
                                m1 = bc.tile([128, rch, S // 2], F32, tag="m1C")
                                tv = tmp.rearrange("p r (c d) -> p r c d", d=2)
                                nc.vector.tensor_tensor(out=m1, in0=tv[:, :, :, 0],
                                                        in1=tv[:, :, :, 1], op=OP.max)
                                m2 = bc.tile([128, rch // 2, S // 2], F32, tag="m2C")
                                m1v = m1.rearrange("p (r d) c -> p r d c", d=2)
                                nc.vector.tensor_tensor(out=m2, in0=m1v[:, :, 0, :],
                                                        in1=m1v[:, :, 1, :], op=OP.max)
                                nc.sync.dma_start(
                                    out=outP[m][:, 1 + rb // 2:1 + rb // 2 + rch // 2, 1:1 + S // 2],
                                    in_=m2.bitcast(F32R))



# ---------------------------------------------------------------------------
# entry point
# ---------------------------------------------------------------------------

_NC_CACHE = {}


def _get_nc():
    if "nc" not in _NC_CACHE:
        _NC_CACHE["nc"] = _build_nc()
    return _NC_CACHE["nc"]


def kernel(x_img, params):
    x_img = np.asarray(x_img, dtype=np.float32)
    P = _prep_weights(params)
    nc = _get_nc()
    in_maps = []
    for i in range(N_CORES):
        m = {"x": np.ascontiguousarray(x_img[i])}
        m.update(P)
        in_maps.append(m)
    res = run_bass_kernel_spmd(nc, in_maps, core_ids=list(range(N_CORES)))
    outs = []
    for k in range(1, 6):
        ek = np.stack([res.results[i][f"e{k}"] for i in range(N_CORES)], axis=0)
        outs.append(ek)
    return (x_img, *outs)


# revision 37
# speedup vs baseline: 1.0563x; 1.0042x over previous
"""Trainium2 Bass kernel for nn_EncoderWav (wavelet CNN encoder).

Strategy: pure data parallelism — 8 images, one per NeuronCore. Sync-BN
batch statistics are combined with tiny AllReduce collectives (2 per
residual block). Convolutions run as fp32r matmuls on the tensor engine
with taps accumulated in PSUM; the first block folds (dx, cin) into the
contraction dim to use the 128-wide PE array despite cin=15.
"""

import sys
from contextlib import ExitStack

sys.path.insert(0, "/opt/trn_rl_repo")

import numpy as np  # noqa: E402

import concourse.bass as bass  # noqa: E402
import concourse.bacc as bacc  # noqa: E402
import concourse.tile as tile  # noqa: E402
import concourse.mybir as mybir  # noqa: E402
from concourse.bass_utils import run_bass_kernel_spmd  # noqa: E402

F32 = mybir.dt.float32
F32R = mybir.dt.float32r
BF16 = mybir.dt.bfloat16
OP = mybir.AluOpType
ACTF = mybir.ActivationFunctionType
BN_EPS = 1e-5
N_CORES = 8

# block specs (k>=2): (Cp pool chans, Cout, S spatial, rows-per-matmul)
BLOCKS = {
    2: dict(Cp=64, Cout=128, S=128, nr=4),
    3: dict(Cp=128, Cout=256, S=64, nr=8),
    4: dict(Cp=256, Cout=512, S=32, nr=16),
    5: dict(Cp=512, Cout=1024, S=16, nr=16),
}
T_ORDER = [4, 0, 1, 2, 3, 5, 6, 7, 8]  # tap (1,1) first so sc rhs has base partition 0
OUT_SHAPES = {
    1: (64, 256, 256), 2: (128, 128, 128), 3: (256, 64, 64),
    4: (512, 32, 32), 5: (1024, 16, 16),
}


def r32(ap):
    return ap if ap.dtype == BF16 else ap.bitcast(F32R)


# ---------------------------------------------------------------------------
# host-side weight packing
# ---------------------------------------------------------------------------

def _prep_weights(params):
    """Transform conv weights into lhsT tensors for the kernel.

    Wavelet inputs are computed unnormalized on device (plain subband sums,
    no 0.5 factors); the 2^-k scale of level-k wavelets is folded into the
    conv weights that consume them. Conv biases are dropped entirely:
    train-mode BN directly follows every conv, and BN(y + b) == BN(y).
    """
    P = {}
    f32 = lambda a: np.ascontiguousarray(np.asarray(a, dtype=np.float32))
    import ml_dtypes
    _to_bf16 = lambda a: np.ascontiguousarray(a.astype(ml_dtypes.bfloat16))

    # ---- block 1 ----
    b = params["blk1"]
    w1 = f32(b["w1"]).copy()              # [64, 15, 3, 3]
    ws = f32(b["ws"])[:, :, 0, 0].copy()  # [64, 15]
    w1[:, 3:, :, :] *= 0.5                # wav1 channels carry 2x scale
    ws[:, 3:] *= 0.5
    w1_1 = np.zeros((3, 45, 128), np.float32)
    for dy in range(3):
        for dx in range(3):
            w1_1[dy, dx * 15:dx * 15 + 15, 0:64] = w1[:, :, dy, dx].T
    w1_1[1, 15:30, 64:128] = ws.T          # shortcut rides tap (dy=1, dx=1)
    P["w1_1"] = w1_1
    w2 = f32(b["w2"])                      # [64, 64, 3, 3]
    w2_1 = np.zeros((3, 128, 64), np.float32)   # pairs: taps (dy,0)+(dy,1)
    w2s_1 = np.zeros((3, 64, 64), np.float32)   # singles: taps (dy,2)
    for dy in range(3):
        w2_1[dy, 0:64] = w2[:, :, dy, 0].T
        w2_1[dy, 64:128] = w2[:, :, dy, 1].T
        w2s_1[dy] = w2[:, :, dy, 2].T
    P["w2_1"] = w2_1
    P["w2s_1"] = w2s_1
    gb = np.zeros((192, 2), np.float32)
    gb[0:64, 0], gb[0:64, 1] = f32(b["g1"]), f32(b["be1"])
    gb[64:128, 0], gb[64:128, 1] = f32(b["gs"]), f32(b["bes"])
    gb[128:192, 0], gb[128:192, 1] = f32(b["g2"]), f32(b["be2"])
    P["gb_1"] = gb

    # ---- block 2 ----
    b = params["blk2"]
    w1 = f32(b["w1"]).copy()               # [128, 76, 3, 3]
    ws = f32(b["ws"])[:, :, 0, 0].copy()
    w1[:, 64:, :, :] *= 0.25               # wav2 carries 4x
    ws[:, 64:] *= 0.25
    w1_2 = np.zeros((10, 76, 128), np.float32)
    for dy in range(3):
        for dx in range(3):
            w1_2[dy * 3 + dx] = w1[:, :, dy, dx].T
    w1_2[9] = ws.T
    P["w1_2"] = w1_2
    w2 = f32(b["w2"])
    w2_2 = np.zeros((9, 128, 128), np.float32)
    for dy in range(3):
        for dx in range(3):
            w2_2[dy * 3 + dx] = w2[:, :, dy, dx].T
    P["w2_2"] = w2_2
    gb = np.zeros((384, 2), np.float32)
    gb[0:128, 0], gb[0:128, 1] = f32(b["g1"]), f32(b["be1"])
    gb[128:256, 0], gb[128:256, 1] = f32(b["gs"]), f32(b["bes"])
    gb[256:384, 0], gb[256:384, 1] = f32(b["g2"]), f32(b["be2"])
    P["gb_2"] = gb

    # ---- blocks 3..5 ----
    for k in (3, 4, 5):
        spec = BLOCKS[k]
        Cp, Cout = spec["Cp"], spec["Cout"]
        Mt, Ktp = Cout // 128, Cp // 128
        b = params[f"blk{k}"]
        w1 = f32(b["w1"]).copy()
        ws = f32(b["ws"])[:, :, 0, 0].copy()
        wavscale = 0.5 ** k
        w1[:, Cp:, :, :] *= wavscale
        ws[:, Cp:] *= wavscale
        w1p = np.zeros((Mt, Ktp, 9, 128, 128), np.float32)
        w1w = np.zeros((Mt, 108, 128), np.float32)
        wsp = np.zeros((Mt, Ktp, 128, 128), np.float32)
        wsw = np.zeros((Mt, 12, 128), np.float32)
        for m in range(Mt):
            wm = w1[m * 128:(m + 1) * 128]
            for kt in range(Ktp):
                for dy in range(3):
                    for dx in range(3):
                        w1p[m, kt, dy * 3 + dx] = wm[:, kt * 128:(kt + 1) * 128, dy, dx].T
                wsp[m, kt] = ws[m * 128:(m + 1) * 128, kt * 128:(kt + 1) * 128].T
            for ti, t in enumerate(T_ORDER):
                dy, dx = t // 3, t % 3
                w1w[m, ti * 12:(ti + 1) * 12, :] = wm[:, Cp:, dy, dx].T
            wsw[m] = ws[m * 128:(m + 1) * 128, Cp:].T
        P[f"w1_{k}"] = w1p
        P[f"w1w_{k}"] = w1w
        P[f"ws_{k}"] = wsp
        P[f"wsw_{k}"] = wsw
        w2 = f32(b["w2"])
        w2p = np.zeros((Mt, Mt, 9, 128, 128), np.float32)
        for m in range(Mt):
            for kt in range(Mt):
                for dy in range(3):
                    for dx in range(3):
                        w2p[m, kt, dy * 3 + dx] = \
                            w2[m * 128:(m + 1) * 128, kt * 128:(kt + 1) * 128, dy, dx].T
        P[f"w2_{k}"] = w2p
        gb = np.zeros((3 * Cout, 2), np.float32)
        gb[0:Cout, 0], gb[0:Cout, 1] = f32(b["g1"]), f32(b["be1"])
        gb[Cout:2 * Cout, 0], gb[Cout:2 * Cout, 1] = f32(b["gs"]), f32(b["bes"])
        gb[2 * Cout:, 0], gb[2 * Cout:, 1] = f32(b["g2"]), f32(b["be2"])
        P[f"gb_{k}"] = gb
    return P


# ---------------------------------------------------------------------------
# device-side helpers
# ---------------------------------------------------------------------------

class Seg:
    """Accumulates bn_stats chunks for one [P, npix] conv-output segment."""

    def __init__(self, pool, P, nchunks, tag):
        self.P = P
        self.buf = pool.tile([P, max(nchunks, 1), 6], F32, tag=tag)
        self.n = 0

    def add(self, nc, src_flat):
        nc.vector.bn_stats(out=self.buf[:, self.n, :], in_=src_flat)
        self.n += 1

    def finish(self, nc, pool, tag):
        mv = pool.tile([self.P, 2], F32, tag=tag + "_mv")
        nc.vector.bn_aggr(out=mv, in_=self.buf[:, :self.n, :])
        return mv


def _pack_stats(nc, pool, mv, P, tag):
    """[P,2] (mean,var) -> [P,2] (mean/8, (var+mean^2)/8)."""
    pk = pool.tile([P, 2], F32, tag=tag + "_pk")
    tmp = pool.tile([P, 1], F32, tag=tag + "_tmp")
    nc.vector.tensor_tensor(out=tmp, in0=mv[:, 0:1], in1=mv[:, 0:1], op=OP.mult)
    nc.vector.tensor_tensor(out=pk[:, 1:2], in0=mv[:, 1:2], in1=tmp, op=OP.add)
    nc.scalar.mul(pk[:, 1:2], pk[:, 1:2], 1.0 / N_CORES)
    nc.scalar.mul(pk[:, 0:1], mv[:, 0:1], 1.0 / N_CORES)
    return pk


def _unpack_stats(nc, pool, g, gamma_beta, P, tag):
    """g [P,2] = (mean, E[x^2]) -> ac [P,2] = (a, c):
    a = gamma * rsqrt(var + eps), c = beta - mean * a."""
    ac = pool.tile([P, 2], F32, tag=tag + "_ac")
    tmp = pool.tile([P, 1], F32, tag=tag + "_t1")
    var = pool.tile([P, 1], F32, tag=tag + "_t2")
    nc.vector.tensor_tensor(out=tmp, in0=g[:, 0:1], in1=g[:, 0:1], op=OP.mult)
    nc.vector.tensor_tensor(out=var, in0=g[:, 1:2], in1=tmp, op=OP.subtract)
    eps = pool.tile([P, 1], F32, tag=tag + "_eps")
    nc.vector.memset(eps, BN_EPS)
    nc.scalar.activation(out=var, in_=var, func=ACTF.Sqrt, bias=eps, scale=1.0)
    nc.vector.reciprocal(out=var, in_=var)
    nc.vector.tensor_tensor(out=ac[:, 0:1], in0=var, in1=gamma_beta[:, 0:1], op=OP.mult)
    nc.vector.tensor_tensor(out=tmp, in0=g[:, 0:1], in1=ac[:, 0:1], op=OP.mult)
    nc.vector.tensor_tensor(out=ac[:, 1:2], in0=gamma_beta[:, 1:2], in1=tmp, op=OP.subtract)
    return ac


def _allreduce(nc, pool, segs_pk, cc_pair, replica, tag):
    """Pack per-segment [P,2] tiles into cci, AllReduce, read back tiles of
    (global mean, global E[x^2])."""
    cci, cco = cc_pair
    off = 0
    for pk in segs_pk:
        Pp = pk.shape[0]
        nc.sync.dma_start(out=cci[0, off:off + 2 * Pp], in_=pk)
        off += 2 * Pp
    nc.gpsimd.collective_compute(
        "AllReduce", OP.add, ins=[cci[:, :]], outs=[cco[:, :]],
        replica_groups=[replica])
    outs = []
    off = 0
    for i, pk in enumerate(segs_pk):
        Pp = pk.shape[0]
        g = pool.tile([Pp, 2], F32, tag=f"{tag}_g{i}")
        nc.sync.dma_start(out=g, in_=cco[0, off:off + 2 * Pp])
        off += 2 * Pp
        outs.append(g)
    return outs


def _load_gb(nc, pool, dram, row0, P, tag):
    t = pool.tile([P, 2], F32, tag=tag)
    nc.sync.dma_start(out=t, in_=dram[row0:row0 + P, :])
    return t


def _pad_memset(nc, t):
    S2 = t.shape[1]
    tb = t.bitcast(F32) if t.dtype == F32R else t
    nc.vector.memset(tb[:, 0, :], 0.0)
    nc.vector.memset(tb[:, S2 - 1, :], 0.0)
    nc.vector.memset(tb[:, :, 0], 0.0)
    nc.vector.memset(tb[:, :, S2 - 1], 0.0)


# ---------------------------------------------------------------------------
# kernel body
# ---------------------------------------------------------------------------

def _build_nc(num_devices=N_CORES, replica=None):
    if replica is None:
        replica = list(range(num_devices))
    nc = bacc.Bacc("TRN2", target_bir_lowering=False, debug=False,
                   num_devices=num_devices)

    x = nc.dram_tensor("x", [3, 512, 512], F32, kind="ExternalInput")
    wt = {}
    wt["w1_1"] = nc.dram_tensor("w1_1", [3, 45, 128], F32, kind="ExternalInput")
    wt["w2_1"] = nc.dram_tensor("w2_1", [3, 128, 64], F32, kind="ExternalInput")
    wt["w2s_1"] = nc.dram_tensor("w2s_1", [3, 64, 64], F32, kind="ExternalInput")
    wt["gb_1"] = nc.dram_tensor("gb_1", [192, 2], F32, kind="ExternalInput")
    wt["w1_2"] = nc.dram_tensor("w1_2", [10, 76, 128], F32, kind="ExternalInput")
    wt["w2_2"] = nc.dram_tensor("w2_2", [9, 128, 128], F32, kind="ExternalInput")
    wt["gb_2"] = nc.dram_tensor("gb_2", [384, 2], F32, kind="ExternalInput")
    for k in (3, 4, 5):
        Cp, Cout = BLOCKS[k]["Cp"], BLOCKS[k]["Cout"]
        Mt, Ktp = Cout // 128, Cp // 128
        wt[f"w1_{k}"] = nc.dram_tensor(f"w1_{k}", [Mt, Ktp, 9, 128, 128], F32, kind="ExternalInput")
        wt[f"w1w_{k}"] = nc.dram_tensor(f"w1w_{k}", [Mt, 108, 128], F32, kind="ExternalInput")
        wt[f"ws_{k}"] = nc.dram_tensor(f"ws_{k}", [Mt, Ktp, 128, 128], F32, kind="ExternalInput")
        wt[f"wsw_{k}"] = nc.dram_tensor(f"wsw_{k}", [Mt, 12, 128], F32, kind="ExternalInput")
        wt[f"w2_{k}"] = nc.dram_tensor(f"w2_{k}", [Mt, Mt, 9, 128, 128], F32, kind="ExternalInput")
        wt[f"gb_{k}"] = nc.dram_tensor(f"gb_{k}", [3 * Cout, 2], F32, kind="ExternalInput")

    eout = {k: nc.dram_tensor(f"e{k}", list(OUT_SHAPES[k]), F32, kind="ExternalOutput")
            for k in range(1, 6)}

    in1d = nc.dram_tensor("in1d", [45, 258, 256], F32)  # (dx,c), 1+256+1 rows, shifted cols
    ll_d = {1: nc.dram_tensor("ll1", [3, 256, 256], F32),
            2: nc.dram_tensor("ll2", [3, 128, 128], F32),
            3: nc.dram_tensor("ll3", [3, 64, 64], F32),
            4: nc.dram_tensor("ll4", [3, 32, 32], F32)}
    wav_d = {2: nc.dram_tensor("wav2", [12, 128, 128], F32),
             3: nc.dram_tensor("wav3", [12, 64, 64], F32),
             4: nc.dram_tensor("wav4", [12, 32, 32], F32),
             5: nc.dram_tensor("wav5", [12, 16, 16], F32)}
    yscD1 = nc.dram_tensor("yscD1", [64, 256, 256], BF16)
    y2D1 = nc.dram_tensor("y2D1", [64, 256, 256], BF16)
    yscD2 = nc.dram_tensor("yscD2", [128, 128, 128], BF16)
    cc = {}
    for tag, n in (("ar1_1", 256), ("ar2_1", 128), ("ar1_2", 512), ("ar2_2", 256),
                   ("ar1_3", 1024), ("ar2_3", 512), ("ar1_4", 2048), ("ar2_4", 1024),
                   ("ar1_5", 4096), ("ar2_5", 2048)):
        cc[tag] = (nc.dram_tensor(f"cci_{tag}", [1, n], F32),
                   nc.dram_tensor(f"cco_{tag}", [1, n], F32, addr_space="Shared"))

    with tile.TileContext(nc, pool_alloc_mode="queue") as tc:
        with ExitStack() as ctx:
            _emit(ctx, tc, nc, x, wt, eout, in1d, ll_d, wav_d,
                  yscD1, y2D1, yscD2, cc, replica)
    nc.compile()
    return nc


def _phase_wavelets(tc, nc, x, in1d, ll_d, wav_d, levels=(1, 2, 3, 4, 5)):
    with tc.tile_pool(name="wv", bufs=3) as wv:
        # zero pad rows (r=0, r=257) of the pre-shifted in1d45
        zr = wv.tile([128, 8192], F32, tag="zr", bufs=1)
        nc.vector.memset(zr, 0.0)
        nc.sync.dma_start(out=in1d[:, 0, :], in_=zr[0:45, 0:256])
        nc.sync.dma_start(out=in1d[:, 257, :], in_=zr[0:45, 0:256])
        # level 1: x [3,512,512] -> in1d channels (pool + wav1) + ll1
        for g in (range(6) if 1 in levels else ()):
            c, half = g // 2, g % 2
            y0 = 128 * half
            E = wv.tile([128, 512], F32, tag="E")
            Ot = wv.tile([128, 512], F32, tag="O")
            nc.sync.dma_start(out=E, in_=x[c, 2 * y0:2 * y0 + 256:2, :])
            nc.sync.dma_start(out=Ot, in_=x[c, 2 * y0 + 1:2 * y0 + 256:2, :])
            Ev = E.rearrange("p (a b) -> p a b", b=2)
            Ov = Ot.rearrange("p (a b) -> p a b", b=2)
            p_ = wv.tile([128, 256], F32, tag="p_")
            m_ = wv.tile([128, 256], F32, tag="m_")
            q_ = wv.tile([128, 256], F32, tag="q_")
            n_ = wv.tile([128, 256], F32, tag="n_")
            nc.vector.tensor_tensor(out=p_, in0=Ev[:, :, 0], in1=Ev[:, :, 1], op=OP.add)
            nc.vector.tensor_tensor(out=m_, in0=Ev[:, :, 0], in1=Ev[:, :, 1], op=OP.subtract)
            nc.vector.tensor_tensor(out=q_, in0=Ov[:, :, 0], in1=Ov[:, :, 1], op=OP.add)
            nc.vector.tensor_tensor(out=n_, in0=Ov[:, :, 0], in1=Ov[:, :, 1], op=OP.subtract)
            # subband/pool tiles carry zero cols at 0 and 257 so the three
            # dx-shifted DRAM writes are single full-width row DMAs
            sb = wv.tile([128, 4, 258], F32, tag="sb")
            nc.vector.memset(sb[:, :, 0], 0.0)
            nc.vector.memset(sb[:, :, 257], 0.0)
            nc.vector.tensor_tensor(out=sb[:, 0, 1:257], in0=p_, in1=q_, op=OP.add)
            nc.vector.tensor_tensor(out=sb[:, 1, 1:257], in0=p_, in1=q_, op=OP.subtract)
            nc.vector.tensor_tensor(out=sb[:, 2, 1:257], in0=m_, in1=n_, op=OP.add)
            nc.vector.tensor_tensor(out=sb[:, 3, 1:257], in0=m_, in1=n_, op=OP.subtract)
            pl = wv.tile([128, 258], F32, tag="pl")
            nc.vector.memset(pl[:, 0:1], 0.0)
            nc.vector.memset(pl[:, 257:258], 0.0)
            nc.vector.tensor_tensor(out=p_, in0=Ev[:, :, 0], in1=Ev[:, :, 1], op=OP.max)
            nc.vector.tensor_tensor(out=q_, in0=Ov[:, :, 0], in1=Ov[:, :, 1], op=OP.max)
            nc.vector.tensor_tensor(out=pl[:, 1:257], in0=p_, in1=q_, op=OP.max)
            # in1d45[(dx,c'), 1+r, x] = in1[c', r, x-1+dx]; src cols [dx, dx+256)
            for ch_, src in [(c, pl)] + [(3 + 4 * c + s, sb[:, s, :]) for s in range(4)]:
                for dx in range(3):
                    nc.sync.dma_start(
                        out=in1d[15 * dx + ch_, 1 + y0:1 + y0 + 128, :],
                        in_=src[:, dx:dx + 256])
            nc.sync.dma_start(out=ll_d[1][c, y0:y0 + 128, :], in_=sb[:, 0, 1:257])

        # levels 2..5 -> wav_d planes (+ ll chain)
        for lvl in (2, 3, 4, 5):
            if lvl not in levels:
                continue
            src = ll_d[lvl - 1]
            Sin = src.shape[1]
            Sout = Sin // 2
            for c in range(3):
                ng = (Sout + 127) // 128
                for gi in range(ng):
                    y0 = gi * 128
                    nrr = min(128, Sout - y0)
                    E = wv.tile([128, 512], F32, tag="E")
                    Ot = wv.tile([128, 512], F32, tag="O")
                    nc.sync.dma_start(out=E[:nrr, :Sin], in_=src[c, 2 * y0:2 * (y0 + nrr):2, :])
                    nc.sync.dma_start(out=Ot[:nrr, :Sin], in_=src[c, 2 * y0 + 1:2 * (y0 + nrr):2, :])
                    Ev = E[:nrr, :Sin].rearrange("p (a b) -> p a b", b=2)
                    Ov = Ot[:nrr, :Sin].rearrange("p (a b) -> p a b", b=2)
                    p_ = wv.tile([128, 256], F32, tag="p_")
                    m_ = wv.tile([128, 256], F32, tag="m_")
                    q_ = wv.tile([128, 256], F32, tag="q_")
                    n_ = wv.tile([128, 256], F32, tag="n_")
                    nc.vector.tensor_tensor(out=p_[:nrr, :Sout], in0=Ev[:, :, 0], in1=Ev[:, :, 1], op=OP.add)
                    nc.vector.tensor_tensor(out=m_[:nrr, :Sout], in0=Ev[:, :, 0], in1=Ev[:, :, 1], op=OP.subtract)
                    nc.vector.tensor_tensor(out=q_[:nrr, :Sout], in0=Ov[:, :, 0], in1=Ov[:, :, 1], op=OP.add)
                    nc.vector.tensor_tensor(out=n_[:nrr, :Sout], in0=Ov[:, :, 0], in1=Ov[:, :, 1], op=OP.subtract)
                    sb = wv.tile([128, 4, 256], F32, tag="sb")
                    nc.vector.tensor_tensor(out=sb[:nrr, 0, :Sout], in0=p_[:nrr, :Sout], in1=q_[:nrr, :Sout], op=OP.add)
                    nc.vector.tensor_tensor(out=sb[:nrr, 1, :Sout], in0=p_[:nrr, :Sout], in1=q_[:nrr, :Sout], op=OP.subtract)
                    nc.vector.tensor_tensor(out=sb[:nrr, 2, :Sout], in0=m_[:nrr, :Sout], in1=n_[:nrr, :Sout], op=OP.add)
                    nc.vector.tensor_tensor(out=sb[:nrr, 3, :Sout], in0=m_[:nrr, :Sout], in1=n_[:nrr, :Sout], op=OP.subtract)
                    for s in range(4):
                        nc.sync.dma_start(out=wav_d[lvl][4 * c + s, y0:y0 + nrr, :],
                                          in_=sb[:nrr, s, :Sout])
                    if lvl < 5:
                        nc.sync.dma_start(out=ll_d[lvl][c, y0:y0 + nrr, :],
                                          in_=sb[:nrr, 0, :Sout])


def _emit(ctx, tc, nc, x, wt, eout, in1d, ll_d, wav_d, yscD1, y2D1, yscD2,
          cc, replica):
    _NEXT_IN.clear()
    small = ctx.enter_context(tc.tile_pool(name="small", bufs=1))

    _phase_wavelets(tc, nc, x, in1d, ll_d, wav_d, levels=(1,))

    # =====================================================================
    # Block 1 (strip-tiled; y1 SBUF bf16; ysc & y2 staged in DRAM bf16)
    # =====================================================================

    with tc.tile_pool(name="b1w", bufs=1) as b1w:
        w1l = b1w.tile([45, 3, 128], F32R, tag="w1l")
        nc.sync.dma_start(out=w1l, in_=wt["w1_1"][:, :, :].rearrange("a b c -> b a c").bitcast(F32R))
        w2l = b1w.tile([128, 3, 64], F32R, tag="w2l")
        nc.sync.dma_start(out=w2l, in_=wt["w2_1"][:, :, :].rearrange("a b c -> b a c").bitcast(F32R))
        w2sl = b1w.tile([64, 3, 64], F32R, tag="w2sl")
        nc.sync.dma_start(out=w2sl, in_=wt["w2s_1"][:, :, :].rearrange("a b c -> b a c").bitcast(F32R))

        _phase_wavelets(tc, nc, x, in1d, ll_d, wav_d, levels=(2, 3, 4, 5))

        with tc.tile_pool(name="pY1", bufs=1) as pY1:
            Y1 = pY1.tile([128, 34816], BF16, tag="Y1")  # part 64h+c; free (r%128)*256+x

            # ---------------- pass A: conv1 + shortcut ----------------
            with tc.tile_pool(name="b1a", bufs=2) as b1a, \
                 tc.tile_pool(name="b1ps", bufs=4, space="PSUM") as b1ps, \
                 tc.tile_pool(name="b1st", bufs=2) as b1st:
                seg1 = Seg(small, 128, 128, "seg1")
                for s in range(8):
                    # T45[(dx,c), yy, x] = in1[c, 32s-1+yy, x-1+dx] (pre-shifted DRAM)
                    T45 = b1a.tile([45, 34, 256], F32R, tag="T45")
                    nc.sync.dma_start(out=T45[:, :, :],
                                      in_=in1d[:, 32 * s:32 * s + 34, :].bitcast(F32R))
                    yscS = b1st.tile([64, 32, 256], BF16, tag="yscS")
                    h = s // 4
                    for i in range(16):
                        yo = 2 * i
                        ps = b1ps.tile([128, 2, 256], F32, tag="ps1")
                        for dy in range(3):
                            nc.tensor.matmul(ps, r32(w1l[:, dy, :]),
                                             r32(T45[:, yo + dy:yo + dy + 2, :]),
                                             start=(dy == 0), stop=(dy == 2))
                        psf = ps.rearrange("p a b -> p (a b)")
                        seg1.add(nc, psf)
                        rr = (32 * s + yo) % 128
                        nc.scalar.copy(Y1[64 * h:64 * h + 64, rr * 256:(rr + 2) * 256],
                                       psf[0:64, :])
                        nc.scalar.copy(yscS[:, yo:yo + 2, :], ps[64:128, :, :])
                    nc.sync.dma_start(out=yscD1[:, 32 * s:32 * s + 32, :], in_=yscS)
                mv1 = seg1.finish(nc, small, "seg1")
                pk1 = _pack_stats(nc, small, mv1, 128, "pk1")
            g1 = _allreduce(nc, small, [pk1], cc["ar1_1"], replica, "ar11")[0]
            gb1a = _load_gb(nc, small, wt["gb_1"], 0, 128, "gb1a")
            ac1 = _unpack_stats(nc, small, g1, gb1a, 128, "ac1")
            acs_d = small.tile([128, 2], F32, tag="acs_d")
            nc.scalar.copy(acs_d[0:64, :], ac1[64:128, :])
            nc.scalar.copy(acs_d[64:128, :], ac1[64:128, :])

            # ---------------- pass B: t1 = relu(bn(y1)); conv2 ----------------
            with tc.tile_pool(name="b1b", bufs=2) as b1b, \
                 tc.tile_pool(name="b1ps2", bufs=4, space="PSUM") as b1ps2, \
                 tc.tile_pool(name="b1st2", bufs=2) as b1st2:
                seg2 = Seg(small, 64, 128, "seg2")
                for s in range(8):
                    # partitions 0:64 = t1; 64:128 = t1 shifted left one col
                    t1 = b1b.tile([128, 34, 258], F32R, tag="t1")
                    nc.vector.memset(t1.bitcast(F32)[:, :, 0], 0.0)
                    nc.vector.memset(t1.bitcast(F32)[:, :, 257], 0.0)
                    r0, r1 = 32 * s - 1, 32 * s + 33
                    if r0 < 0:
                        nc.vector.memset(t1.bitcast(F32)[:, 0, :], 0.0)
                        r0 = 0
                    if r1 > 256:
                        nc.vector.memset(t1.bitcast(F32)[:, 33, :], 0.0)
                        r1 = 256
                    spans = []
                    if r0 < 128:
                        spans.append((0, r0, min(r1, 128)))
                    if r1 > 128:
                        spans.append((1, max(r0, 128), r1))
                    for h, a, bnd in spans:
                        Yv = Y1[64 * h:64 * h + 64,
                                (a % 128) * 256:((a % 128) + (bnd - a)) * 256]
                        yy = a - (32 * s - 1)
                        nc.scalar.activation(
                            out=t1[0:64, yy:yy + (bnd - a), 1:257],
                            in_=Yv.rearrange("p (r c) -> p r c", c=256),
                            func=ACTF.Relu,
                            bias=ac1[0:64, 1:2], scale=ac1[0:64, 0:1])
                    # duplicate t1 into partitions 64:128 shifted left one col
                    nc.sync.dma_start(out=t1[64:128, :, 0:257],
                                      in_=t1[0:64, :, 1:258])
                    y2S = b1st2.tile([64, 32, 256], BF16, tag="y2S")
                    for i in range(16):
                        yo = 2 * i
                        ps = b1ps2.tile([64, 2, 256], F32, tag="ps2")
                        for dy in range(3):
                            nc.tensor.matmul(
                                ps, r32(w2l[:, dy, :]),
                                r32(t1[:, yo + dy:yo + dy + 2, 0:256]),
                                start=(dy == 0), stop=False)
                        for dy in range(3):
                            nc.tensor.matmul(
                                ps, r32(w2sl[:, dy, :]),
                                r32(t1[0:64, yo + dy:yo + dy + 2, 2:258]),
                                start=False, stop=(dy == 2))
                        psf = ps.rearrange("p a b -> p (a b)")
                        seg2.add(nc, psf)
                        nc.scalar.copy(y2S[:, yo:yo + 2, :], ps)
                    nc.sync.dma_start(out=y2D1[:, 32 * s:32 * s + 32, :], in_=y2S)
                mv2 = seg2.finish(nc, small, "seg2")
                pk2 = _pack_stats(nc, small, mv2, 64, "pk2")
        # Y1 pool closed here
        g2 = _allreduce(nc, small, [pk2], cc["ar2_1"], replica, "ar21")[0]
        gb1b = _load_gb(nc, small, wt["gb_1"], 128, 64, "gb1b")
        ac2 = _unpack_stats(nc, small, g2, gb1b, 64, "ac2")
        ac2_d = small.tile([128, 2], F32, tag="ac2_d")
        nc.scalar.copy(ac2_d[0:64, :], ac2)
        nc.scalar.copy(ac2_d[64:128, :], ac2)
        ccs1 = small.tile([128, 1], F32, tag="ccs1")
        nc.vector.tensor_tensor(out=ccs1, in0=acs_d[:, 1:2], in1=ac2_d[:, 1:2], op=OP.add)

    # in2sb spans blk1 pass C .. blk2 pass A
    p_in2 = tc.tile_pool(name="p_in2", bufs=1, side="right")
    in2p = p_in2.__enter__()
    in2sb = in2p.tile([76, 130, 130], F32R, tag="in2sb")
    _pad_memset(nc, in2sb)
    # wav2 channels
    nc.sync.dma_start(out=in2sb[64:76, 1:129, 1:129], in_=wav_d[2][:, :, :].bitcast(F32R))

    # ---------------- blk1 pass C ----------------
    with tc.tile_pool(name="b1c", bufs=2) as b1c:
        for j in range(16):
            # chunk covers rows [8j,8j+8) of each half
            y2c = b1c.tile([128, 8, 256], BF16, tag="y2c")
            ysc = b1c.tile([128, 8, 256], BF16, tag="ysc")
            for h in range(2):
                rb = 128 * h + 8 * j
                nc.sync.dma_start(out=y2c[64 * h:64 * h + 64, :, :],
                                  in_=y2D1[:, rb:rb + 8, :])
                nc.sync.dma_start(out=ysc[64 * h:64 * h + 64, :, :],
                                  in_=yscD1[:, rb:rb + 8, :])
            tmp = b1c.tile([128, 8, 256], F32, tag="tmpc")
            nc.scalar.activation(out=tmp, in_=ysc, func=ACTF.Identity,
                                 bias=ccs1[:, 0:1], scale=acs_d[:, 0:1])
            nc.vector.scalar_tensor_tensor(
                out=tmp, in0=y2c, scalar=ac2_d[:, 0:1], in1=tmp,
                op0=OP.mult, op1=OP.add)
            nc.scalar.activation(out=tmp, in_=tmp, func=ACTF.Relu)
            for h in range(2):
                rb = 128 * h + 8 * j
                nc.sync.dma_start(out=eout[1][:, rb:rb + 8, :],
                                  in_=tmp[64 * h:64 * h + 64, :, :])
            m1 = b1c.tile([128, 8, 128], F32, tag="m1")
            tv = tmp.rearrange("p r (c d) -> p r c d", d=2)
            nc.vector.tensor_tensor(out=m1, in0=tv[:, :, :, 0], in1=tv[:, :, :, 1], op=OP.max)
            m2 = b1c.tile([128, 4, 128], F32, tag="m2")
            m1v = m1.rearrange("p (r d) c -> p r d c", d=2)
            nc.vector.tensor_tensor(out=m2, in0=m1v[:, :, 0, :], in1=m1v[:, :, 1, :], op=OP.max)
            for h in range(2):
                rb = 64 * h + 4 * j
                nc.sync.dma_start(out=in2sb[0:64, 1 + rb:1 + rb + 4, 1:129],
                                  in_=m2[64 * h:64 * h + 64, :, :].bitcast(F32R))

    # =====================================================================
    # Block 2
    # =====================================================================
    _emit_block2(tc, nc, wt, eout, in2sb, p_in2, yscD2, wav_d, cc, replica, small)

    # =====================================================================
    # Blocks 3..5
    # =====================================================================
    # in3a was created by _emit_block2 pass C (returned via small registry)
    _emit_blockk(tc, nc, 3, wt, eout, cc, replica, small, wav_d,
                 stream_w1=False, stream_w2=False)
    _emit_blockk(tc, nc, 4, wt, eout, cc, replica, small, wav_d,
                 stream_w1=True, stream_w2=True)
    _emit_blockk(tc, nc, 5, wt, eout, cc, replica, small, wav_d,
                 stream_w1=True, stream_w2=True)


_NEXT_IN = {}  # k -> list of [128, Spad, Spad] pool-input tiles (built by k-1)


def _open_next_in(tc, nc, k):
    """Create block k's pool-channel input tiles (padded, borders zeroed)."""
    spec = BLOCKS[k]
    S, Ktp = spec["S"], spec["Cp"] // 128 if k >= 3 else 1
    Spad = S + 2
    pool_cm = tc.tile_pool(name=f"p_in{k}", bufs=1, side="right")
    p = pool_cm.__enter__()
    nt = max(1, spec["Cp"] // 128)
    npart = 128 if spec["Cp"] >= 128 else spec["Cp"]
    tiles = []
    for i in range(nt):
        t = p.tile([npart, Spad, Spad], F32R, tag=f"in{k}_{i}", name=f"in{k}_{i}")
        tiles.append(t)
    for t in tiles:
        _pad_memset(nc, t)
    _NEXT_IN[k] = (tiles, pool_cm, p)
    return tiles


def _emit_block2(tc, nc, wt, eout, in2sb, p_in2_cm, yscD2, wav_d, cc, replica, small):
    S, Spad, nr = 128, 130, 4
    ngr = S // nr

    with tc.tile_pool(name="b2y", bufs=1) as b2y:
        y1b = b2y.tile([128, S * S], F32, tag="y1b2")
        with tc.tile_pool(name="b2w", bufs=1) as b2w:
            w1l = b2w.tile([76, 10, 128], F32R, tag="w1l2")
            nc.sync.dma_start(out=w1l, in_=wt["w1_2"][:, :, :].rearrange("a b c -> b a c").bitcast(F32R))
            w2l = b2w.tile([128, 9, 128], F32R, tag="w2l2")
            nc.sync.dma_start(out=w2l, in_=wt["w2_2"][:, :, :].rearrange("a b c -> b a c").bitcast(F32R))

            # ---- pass A: conv1 + sc ----
            with tc.tile_pool(name="b2ps", bufs=4, space="PSUM") as psp, \
                 tc.tile_pool(name="b2st", bufs=2) as stp:
                seg1 = Seg(small, 128, ngr, "b2seg1")
                segs = Seg(small, 128, ngr, "b2segs")
                for g in range(ngr):
                    y0 = g * nr
                    ps = psp.tile([128, nr, S], F32, tag="psA")
                    first = True
                    for dy in range(3):
                        for dx in range(3):
                            nc.tensor.matmul(
                                ps, r32(w1l[:, dy * 3 + dx, :]),
                                r32(in2sb[:, y0 + dy:y0 + dy + nr, dx:dx + S]),
                                start=first, stop=(dy == 2 and dx == 2))
                            first = False
                    pss = psp.tile([128, nr, S], F32, tag="psS")
                    nc.tensor.matmul(pss, r32(w1l[:, 9, :]),
                                     r32(in2sb[:, 1 + y0:1 + y0 + nr, 1:1 + S]),
                                     start=True, stop=True)
                    seg1.add(nc, ps.rearrange("p a b -> p (a b)"))
                    segs.add(nc, pss.rearrange("p a b -> p (a b)"))
                    nc.scalar.copy(y1b[:, y0 * S:(y0 + nr) * S],
                                   ps.rearrange("p a b -> p (a b)"))
                    yscS = stp.tile([128, nr, S], BF16, tag="yscS2")
                    nc.scalar.copy(yscS, pss)
                    nc.sync.dma_start(out=yscD2[:, y0:y0 + nr, :], in_=yscS)
                mv1 = seg1.finish(nc, small, "b2seg1")
                mvs = segs.finish(nc, small, "b2segs")
                pk1 = _pack_stats(nc, small, mv1, 128, "b2pk1")
                pks = _pack_stats(nc, small, mvs, 128, "b2pks")
            # in2sb dead from here
            p_in2_cm.__exit__(None, None, None)
            gars = _allreduce(nc, small, [pk1, pks], cc["ar1_2"], replica, "b2ar1")
            ac1 = _unpack_stats(nc, small, gars[0], _load_gb(nc, small, wt["gb_2"], 0, 128, "gb2_1"), 128, "b2ac1")
            acs = _unpack_stats(nc, small, gars[1], _load_gb(nc, small, wt["gb_2"], 128, 128, "gb2_s"), 128, "b2acs")

            # ---- pass B: t1 strips + conv2 ----
            with tc.tile_pool(name="b2y2", bufs=1) as b2y2:
                y2b = b2y2.tile([128, S * S], BF16, tag="y2b2")
                with tc.tile_pool(name="b2b", bufs=3) as b2b, \
                     tc.tile_pool(name="b2ps2", bufs=4, space="PSUM") as psp2:
                    seg2 = Seg(small, 128, ngr, "b2seg2")
                    y1v = y1b.rearrange("p (r c) -> p r c", c=S)
                    for g in range(ngr):
                        y0 = g * nr
                        t1 = b2b.tile([128, nr + 2, Spad], F32R, tag="t1s2")
                        nc.vector.memset(t1.bitcast(F32)[:, :, 0], 0.0)
                        nc.vector.memset(t1.bitcast(F32)[:, :, Spad - 1], 0.0)
                        r0, r1 = y0 - 1, y0 + nr + 1
                        if r0 < 0:
                            nc.vector.memset(t1.bitcast(F32)[:, 0, :], 0.0)
                            r0 = 0
                        if r1 > S:
                            nc.vector.memset(t1.bitcast(F32)[:, nr + 1, :], 0.0)
                            r1 = S
                        nc.scalar.activation(
                            out=t1[:, r0 - (y0 - 1):r1 - (y0 - 1), 1:1 + S],
                            in_=y1v[:, r0:r1, :], func=ACTF.Relu,
                            bias=ac1[:, 1:2], scale=ac1[:, 0:1])
                        ps = psp2.tile([128, nr, S], F32, tag="psB")
                        first = True
                        for dy in range(3):
                            for dx in range(3):
                                nc.tensor.matmul(
                                    ps, r32(w2l[:, dy * 3 + dx, :]),
                                    r32(t1[:, dy:dy + nr, dx:dx + S]),
                                    start=first, stop=(dy == 2 and dx == 2))
                                first = False
                        seg2.add(nc, ps.rearrange("p a b -> p (a b)"))
                        nc.scalar.copy(
                            y2b.rearrange("p (r c) -> p r c", c=S)[:, y0:y0 + nr, :], ps)
                    mv2 = seg2.finish(nc, small, "b2seg2")
                    pk2 = _pack_stats(nc, small, mv2, 128, "b2pk2")
                # y1b dead
                g2 = _allreduce(nc, small, [pk2], cc["ar2_2"], replica, "b2ar2")[0]
                ac2 = _unpack_stats(nc, small, g2, _load_gb(nc, small, wt["gb_2"], 256, 128, "gb2_2"), 128, "b2ac2")
                ccs = small.tile([128, 1], F32, tag="b2ccs")
                nc.vector.tensor_tensor(out=ccs, in0=acs[:, 1:2], in1=ac2[:, 1:2], op=OP.add)

                # ---- pass C ----
                in3 = _open_next_in(tc, nc, 3)
                with tc.tile_pool(name="b2c", bufs=1) as bc:
                    for j in range(16):
                        rb = 8 * j
                        ysc = bc.tile([128, 8, S], BF16, tag="yscC2")
                        nc.sync.dma_start(out=ysc, in_=yscD2[:, rb:rb + 8, :])
                        tmp = bc.tile([128, 8, S], F32, tag="tmpC2")
                        nc.scalar.activation(out=tmp, in_=ysc, func=ACTF.Identity,
                                             bias=ccs[:, 0:1], scale=acs[:, 0:1])
                        y2v = y2b.rearrange("p (r c) -> p r c", c=S)[:, rb:rb + 8, :]
                        nc.vector.scalar_tensor_tensor(
                            out=tmp, in0=y2v, scalar=ac2[:, 0:1], in1=tmp,
                            op0=OP.mult, op1=OP.add)
                        nc.scalar.activation(out=tmp, in_=tmp, func=ACTF.Relu)
                        nc.sync.dma_start(out=eout[2][:, rb:rb + 8, :], in_=tmp)
                        m1 = bc.tile([128, 8, 64], F32, tag="m1C2")
                        tv = tmp.rearrange("p r (c d) -> p r c d", d=2)
                        nc.vector.tensor_tensor(out=m1, in0=tv[:, :, :, 0],
                                                in1=tv[:, :, :, 1], op=OP.max)
                        m2 = bc.tile([128, 4, 64], F32, tag="m2C2")
                        m1v = m1.rearrange("p (r d) c -> p r d c", d=2)
                        nc.vector.tensor_tensor(out=m2, in0=m1v[:, :, 0, :],
                                                in1=m1v[:, :, 1, :], op=OP.max)
                        nc.sync.dma_start(out=in3[0][:, 1 + 4 * j:1 + 4 * j + 4, 1:65],
                                          in_=m2.bitcast(F32R))


def _emit_blockk(tc, nc, k, wt, eout, cc, replica, small, wav_d,
                 stream_w1=False, stream_w2=False):
    spec = BLOCKS[k]
    Cp, Cout, S, nr = spec["Cp"], spec["Cout"], spec["S"], spec["nr"]
    Spad = S + 2
    Mt, Ktp = Cout // 128, Cp // 128
    ngr = S // nr
    inP, inP_cm, inP_pool = _NEXT_IN[k]

    with tc.tile_pool(name=f"bk{k}w", bufs=1) as bkw:
        bkT_cm = tc.tile_pool(name=f"bk{k}T", bufs=1, side="right")
        bkT = bkT_cm.__enter__()
        # wavelet im2col from DRAM (padded implicitly via shifts + zero pad)
        T108 = bkT.tile([108, S, S], F32R, tag=f"T108_{k}")
        wpad = bkT.tile([12, Spad, Spad], F32R, tag=f"wpad{k}")
        _pad_memset(nc, wpad)
        nc.sync.dma_start(out=wpad[:, 1:1 + S, 1:1 + S], in_=wav_d[k][:, :, :].bitcast(F32R))
        for ti, t in enumerate(T_ORDER):
            dy, dx = t // 3, t % 3
            nc.sync.dma_start(out=T108[12 * ti:12 * ti + 12, :, :],
                              in_=wpad[:, dy:dy + S, dx:dx + S])
        w1wl = bkw.tile([108, Mt, 128], F32R, tag=f"w1wl{k}")
        nc.sync.dma_start(out=w1wl, in_=wt[f"w1w_{k}"][:, :, :].rearrange("a b c -> b a c").bitcast(F32R))
        wswl = bkw.tile([12, Mt, 128], F32R, tag=f"wswl{k}")
        nc.sync.dma_start(out=wswl, in_=wt[f"wsw_{k}"][:, :, :].rearrange("a b c -> b a c").bitcast(F32R))
        wspl = bkw.tile([128, Mt, Ktp, 128], F32R, tag=f"wspl{k}")
        nc.sync.dma_start(out=wspl, in_=wt[f"ws_{k}"][:, :, :, :].rearrange("a b c d -> c a b d").bitcast(F32R))
        if not stream_w1:
            w1pl = bkw.tile([128, Mt, Ktp, 9, 128], F32R, tag=f"w1pl{k}")
            nc.sync.dma_start(out=w1pl,
                              in_=wt[f"w1_{k}"][:, :, :, :, :].rearrange("a b c d e -> d a b c e").bitcast(F32R))

        with tc.tile_pool(name=f"bk{k}ys", bufs=1) as bkys:
            y1b = [inP_pool.tile([128, S * S], F32, tag=f"y1b{k}_{m}", name=f"y1b{k}_{m}") for m in range(Mt)]
            yscb = [bkys.tile([128, S * S], F32, tag=f"yscb{k}_{m}", name=f"yscb{k}_{m}") for m in range(Mt)]

            # ---- pass A ----
            with tc.tile_pool(name=f"b{k}ps", bufs=4, space="PSUM") as psp, \
                 tc.tile_pool(name=f"b{k}wst", bufs=2) as wstr:
                seg1 = [Seg(small, 128, ngr, f"b{k}seg1_{m}") for m in range(Mt)]
                segs = [Seg(small, 128, ngr, f"b{k}segs_{m}") for m in range(Mt)]
                for m in range(Mt):
                    w1m = {}
                    if stream_w1:
                        for kt in range(Ktp):
                            w1kt = wstr.tile([128, 9, 128], F32R, tag=f"w1m{kt % 2}",
                                             name=f"w1m_{m}_{kt}")
                            nc.sync.dma_start(out=w1kt,
                                              in_=wt[f"w1_{k}"][m, kt].rearrange("d e f -> e d f").bitcast(F32R))
                            w1m[kt] = w1kt
                    for g in range(ngr):
                        y0 = g * nr
                        ps = psp.tile([128, nr, S], F32, tag="psA")
                        first = True
                        for kt in range(Ktp):
                            for dy in range(3):
                                for dx in range(3):
                                    lw = (w1m[kt][:, dy * 3 + dx, :] if stream_w1
                                          else w1pl[:, m, kt, dy * 3 + dx, :])
                                    nc.tensor.matmul(
                                        ps, r32(lw),
                                        r32(inP[kt][:, y0 + dy:y0 + dy + nr, dx:dx + S]),
                                        start=first, stop=False)
                                    first = False
                        nc.tensor.matmul(ps, r32(w1wl[:, m, :]),
                                         r32(T108[:, y0:y0 + nr, :]),
                                         start=False, stop=True)
                        pss = psp.tile([128, nr, S], F32, tag="psS")
                        for kt in range(Ktp):
                            nc.tensor.matmul(pss, r32(wspl[:, m, kt, :]),
                                             r32(inP[kt][:, 1 + y0:1 + y0 + nr, 1:1 + S]),
                                             start=(kt == 0), stop=False)
                        nc.tensor.matmul(pss, r32(wswl[:, m, :]),
                                         r32(T108[0:12, y0:y0 + nr, :]),
                                         start=False, stop=True)
                        seg1[m].add(nc, ps.rearrange("p a b -> p (a b)"))
                        segs[m].add(nc, pss.rearrange("p a b -> p (a b)"))
                        nc.scalar.copy(y1b[m][:, y0 * S:(y0 + nr) * S],
                                       ps.rearrange("p a b -> p (a b)"))
                        nc.scalar.copy(yscb[m][:, y0 * S:(y0 + nr) * S],
                                       pss.rearrange("p a b -> p (a b)"))
                pks = [_pack_stats(nc, small, seg1[m].finish(nc, small, f"b{k}seg1_{m}"),
                                   128, f"b{k}p1{m}") for m in range(Mt)] + \
                      [_pack_stats(nc, small, segs[m].finish(nc, small, f"b{k}segs_{m}"),
                                   128, f"b{k}ps{m}") for m in range(Mt)]
            bkT_cm.__exit__(None, None, None)  # T108/wpad dead after pass A
            gl = _allreduce(nc, small, pks, cc[f"ar1_{k}"], replica, f"b{k}ar1")
            ac1 = [_unpack_stats(nc, small, gl[m],
                                 _load_gb(nc, small, wt[f"gb_{k}"], m * 128, 128, f"gbl{k}1{m}"),
                                 128, f"b{k}ac1{m}") for m in range(Mt)]
            acs = [_unpack_stats(nc, small, gl[Mt + m],
                                 _load_gb(nc, small, wt[f"gb_{k}"], Cout + m * 128, 128, f"gbl{k}s{m}"),
                                 128, f"b{k}acs{m}") for m in range(Mt)]

            # ---- pass B ----
            with tc.tile_pool(name=f"bk{k}y2", bufs=1) as bky2:
                y2b = [bky2.tile([128, S * S], F32, tag=f"y2b{k}_{m}", name=f"y2b{k}_{m}") for m in range(Mt)]
                with tc.tile_pool(name=f"b{k}t1s", bufs=2) as bt1s, \
                     tc.tile_pool(name=f"b{k}ps2", bufs=4, space="PSUM") as psp2, \
                     tc.tile_pool(name=f"b{k}wst2", bufs=2) as wstr2:
                    seg2 = [Seg(small, 128, ngr, f"b{k}seg2_{m}") for m in range(Mt)]
                    if not stream_w2:
                        w2lf = bkw.tile([128, Mt, Mt, 9, 128], F32R, tag=f"w2l{k}")
                        nc.sync.dma_start(out=w2lf,
                                          in_=wt[f"w2_{k}"][:, :, :, :, :].rearrange("a b c d e -> d a b c e").bitcast(F32R))
                    for m in range(Mt):
                        w2m = {}
                        if stream_w2:
                            w2dt = F32R
                            for kt in range(Mt):
                                w2kt = wstr2.tile([128, 9, 128], w2dt, tag=f"w2m{kt % 2}",
                                                  name=f"w2m_{m}_{kt}")
                                src = wt[f"w2_{k}"][m, kt].rearrange("d e f -> e d f").bitcast(F32R)
                                nc.sync.dma_start(out=w2kt, in_=src)
                                w2m[kt] = w2kt
                        for g in range(ngr):
                            y0 = g * nr
                            t1s = []
                            t1dt = F32R
                            for kt in range(Mt):
                                t1k = bt1s.tile([128, nr + 2, Spad], t1dt,
                                                tag=f"t1s{kt}", name=f"t1s{kt}")
                                t1m = t1k.bitcast(F32) if t1dt == F32R else t1k
                                nc.vector.memset(t1m[:, :, 0], 0.0)
                                nc.vector.memset(t1m[:, :, Spad - 1], 0.0)
                                r0, r1 = y0 - 1, y0 + nr + 1
                                if r0 < 0:
                                    nc.vector.memset(t1m[:, 0, :], 0.0)
                                    r0 = 0
                                if r1 > S:
                                    nc.vector.memset(t1m[:, nr + 1, :], 0.0)
                                    r1 = S
                                nc.scalar.activation(
                                    out=t1k[:, r0 - (y0 - 1):r1 - (y0 - 1), 1:1 + S],
                                    in_=y1b[kt].rearrange("p (r c) -> p r c", c=S)[:, r0:r1, :],
                                    func=ACTF.Relu, bias=ac1[kt][:, 1:2], scale=ac1[kt][:, 0:1])
                                t1s.append(t1k)
                            ps = psp2.tile([128, nr, S], F32, tag="psB")
                            first = True
                            for kt in range(Mt):
                                for dy in range(3):
                                    for dx in range(3):
                                        lw = (w2m[kt][:, dy * 3 + dx, :] if stream_w2
                                              else w2lf[:, m, kt, dy * 3 + dx, :])
                                        nc.tensor.matmul(
                                            ps, r32(lw),
                                            r32(t1s[kt][:, dy:dy + nr, dx:dx + S]),
                                            start=first,
                                            stop=(kt == Mt - 1 and dy == 2 and dx == 2))
                                        first = False
                            seg2[m].add(nc, ps.rearrange("p a b -> p (a b)"))
                            nc.scalar.copy(y2b[m][:, y0 * S:(y0 + nr) * S],
                                           ps.rearrange("p a b -> p (a b)"))
                    pk2 = [_pack_stats(nc, small, seg2[m].finish(nc, small, f"b{k}seg2_{m}"),
                                       128, f"b{k}p2{m}") for m in range(Mt)]
                inP_cm.__exit__(None, None, None)  # in-tiles + y1b dead
                gl2 = _allreduce(nc, small, pk2, cc[f"ar2_{k}"], replica, f"b{k}ar2")
                ac2 = [_unpack_stats(nc, small, gl2[m],
                                     _load_gb(nc, small, wt[f"gb_{k}"], 2 * Cout + m * 128, 128, f"gbl{k}2{m}"),
                                     128, f"b{k}ac2{m}") for m in range(Mt)]

                # ---- pass C ----
                outP = _open_next_in(tc, nc, k + 1) if k < 5 else None
                ncch = S // 16 if S >= 32 else 1   # row chunks
                rch = S // ncch
                with tc.tile_pool(name=f"b{k}c", bufs=2) as bc:
                    ccs_l = []
                    for m in range(Mt):
                        ccs = small.tile([128, 1], F32, tag=f"b{k}ccs{m}", name=f"b{k}ccs{m}")
                        nc.vector.tensor_tensor(out=ccs, in0=acs[m][:, 1:2],
                                                in1=ac2[m][:, 1:2], op=OP.add)
                        ccs_l.append(ccs)
                    for ch in range(ncch):
                        for m in range(Mt):
                            ccs = ccs_l[m]
                            rb = ch * rch
                            tmp = bc.tile([128, rch, S], F32, tag="tmpC")
                            nc.scalar.activation(
                                out=tmp,
                                in_=yscb[m].rearrange("p (r c) -> p r c", c=S)[:, rb:rb + rch, :],
                                func=ACTF.Identity, bias=ccs[:, 0:1], scale=acs[m][:, 0:1])
                            nc.vector.scalar_tensor_tensor(
                                out=tmp,
                                in0=y2b[m].rearrange("p (r c) -> p r c", c=S)[:, rb:rb + rch, :],
                                scalar=ac2[m][:, 0:1], in1=tmp, op0=OP.mult, op1=OP.add)
                            nc.scalar.activation(out=tmp, in_=tmp, func=ACTF.Relu)
                            nc.sync.dma_start(out=eout[k][m * 128:(m + 1) * 128, rb:rb + rch, :], in_=tmp)
                            if k < 5:
                                m1 = bc.tile([128, rch, S // 2], F32, tag="m1C")
                                tv = tmp.rearrange("p r (c d) -> p r c d", d=2)
                                nc.vector.tensor_tensor(out=m1, in0=tv[:, :, :, 0],
                                                        in1=tv[:, :, :, 1], op=OP.max)
                                m2 = bc.tile([128, rch // 2, S // 2], F32, tag="m2C")
                                m1v = m1.rearrange("p (r d) c -> p r d c", d=2)
                                nc.vector.tensor_tensor(out=m2, in0=m1v[:, :, 0, :],
                                                        in1=m1v[:, :, 1, :], op=OP.max)
                                nc.sync.dma_start(
                                    out=outP[m][:, 1 + rb // 2:1 + rb // 2 + rch // 2, 1:1 + S // 2],
                                    in_=m2.bitcast(F32R))



# ---------------------------------------------------------------------------
# entry point
# ---------------------------------------------------------------------------

_NC_CACHE = {}


def _get_nc():
    if "nc" not in _NC_CACHE:
        _NC_CACHE["nc"] = _build_nc()
    return _NC_CACHE["nc"]


def kernel(x_img, params):
    x_img = np.asarray(x_img, dtype=np.float32)
    P = _prep_weights(params)
    nc = _get_nc()
    in_maps = []
    for i in range(N_CORES):
        m = {"x": np.ascontiguousarray(x_img[i])}
        m.update(P)
        in_maps.append(m)
    res = run_bass_kernel_spmd(nc, in_maps, core_ids=list(range(N_CORES)))
    outs = []
    for k in range(1, 6):
        ek = np.stack([res.results[i][f"e{k}"] for i in range(N_CORES)], axis=0)
        outs.append(ek)
    return (x_img, *outs)
